# revision 35
# baseline (speedup 1.0000x reference)
"""GatedDeltaNet block kernel for 8 Trainium2 cores (Bass/Tile), bf16 rework.

Sharding: DP2 (batch) x TP4 (heads / MLP-inter). Core c: group g=c//4 runs
batch g; member m=c%4 owns heads [8m,8m+8), q/k cols [384m,..), v/g cols
[768m,..), INTER [1408m,..). Two half-token AllReduces per 4-core group after
o_proj (overlapped with o_proj compute); final down-proj partials summed on
the host.

Everything runs feature-major (host passes x transposed, takes y transposed)
so there are no PE transposes outside the delta-rule inner loop. All big
GEMM operands are bf16 (host-cast weights); psum accumulation, the delta-rule
state, decay/beta math and norms stay fp32.

Per-core dataflow:
  A: xT [D,T] -> rmsnorm via matmul-accumulated column sumsq -> hT bf16 (SBUF)
  B: bf16 projections off hT; q/k feature-major (heads padded to 64 rows)
     -> conv+silu+l2norm -> SBUF (+ token-major copies of k, v via PE
     transposes); gate token-major; a/b -> batched decay prep for all chunks
  C: chunked gated delta rule (C=128, UT transform via log-doubling inverse,
     bf16 matmuls / fp32 state); writes normed+gated o feature-major to SBUF
  D: o_projT in two token halves, each followed by its AllReduce (overlapped)
  E: h2T = xT + oT; rmsnorm -> ffT bf16 (reuses hT); MLP bf16; yT partials
"""
import sys
sys.path.insert(0, '/opt/trn_rl_repo')
import numpy as np
import ml_dtypes

import concourse.bass as bass
import concourse.bacc as bacc
import concourse.mybir as mybir
import concourse.tile as tile
from concourse.bass_isa import ReduceOp
from concourse.bass_utils import run_bass_kernel_spmd

F32 = mybir.dt.float32
BF16 = mybir.dt.bfloat16
AF = mybir.ActivationFunctionType
OP = mybir.AluOpType

B, T, D = 2, 1024, 2048
H, DK, DV = 32, 48, 96
HP = 8
QKP = 512
VD_C = 768
VP = 1024
INT_C = 1408
C = 128
NCHUNK = T // C
KT = D // 128
NTOK = T // 128

_cache = {}
import os
PHASES = os.environ.get("DN_PHASES", "ABCDE")
NCH = int(os.environ.get("DN_NCHUNK", str(T // C)))
DUMP = os.environ.get("DN_DUMP", "")


class _SkipRest(Exception):
    pass


class _Dumped(Exception):
    pass


def _build(n_cores=8):
    groups = [[0, 1, 2, 3], [4, 5, 6, 7]] if n_cores == 8 else [[0]]
    nc = bacc.Bacc("TRN2", target_bir_lowering=False, debug=False, num_devices=n_cores)

    xT_d = nc.dram_tensor("xT", [D, T], F32, kind="ExternalInput")
    wq_d = nc.dram_tensor("wq", [D, QKP], BF16, kind="ExternalInput")
    wk_d = nc.dram_tensor("wk", [D, QKP], BF16, kind="ExternalInput")
    wv_d = nc.dram_tensor("wv", [D, VP], BF16, kind="ExternalInput")
    wg_d = nc.dram_tensor("wg", [D, VD_C], BF16, kind="ExternalInput")
    wab_d = nc.dram_tensor("wab", [D, 16], BF16, kind="ExternalInput")
    cq_d = nc.dram_tensor("cq", [QKP, 4], F32, kind="ExternalInput")
    ck_d = nc.dram_tensor("ck", [QKP, 4], F32, kind="ExternalInput")
    cv_d = nc.dram_tensor("cv", [VP, 4], F32, kind="ExternalInput")
    dtb_d = nc.dram_tensor("dtb", [1, 64], F32, kind="ExternalInput")
    nega_d = nc.dram_tensor("nega", [1, 64], F32, kind="ExternalInput")
    wo_d = nc.dram_tensor("wo", [VD_C, D], BF16, kind="ExternalInput")
    w1_d = nc.dram_tensor("w1", [D, INT_C], BF16, kind="ExternalInput")
    w3_d = nc.dram_tensor("w3", [D, INT_C], BF16, kind="ExternalInput")
    w2_d = nc.dram_tensor("w2", [INT_C, D], BF16, kind="ExternalInput")
    y_d = nc.dram_tensor("y", [D, T], F32, kind="ExternalOutput")

    ones = np.ones((128, 128), np.float32)
    idn_c = nc.inline_tensor(np.eye(128, dtype=np.float32), "idn_c")
    idnb_c = nc.inline_tensor(np.eye(128, dtype=ml_dtypes.bfloat16), "idnb_c")
    cum_c = nc.inline_tensor(np.triu(ones).copy(), "cum_c")
    mst_c = nc.inline_tensor(np.triu(ones, 1).astype(ml_dtypes.bfloat16), "mst_c")
    msi_c = nc.inline_tensor(np.triu(ones).copy(), "msi_c")
    negl_c = nc.inline_tensor((np.tril(ones, -1) * -1e30).copy(), "negl_c")
    # SELJ[r, 128j+p] = 1 iff (r%8==2j and p<48) or (r%8==2j+1 and 64<=p<112)
    selj_np = np.zeros((64, 512), np.float32)
    for r in range(64):
        for j in range(4):
            if r % 8 == 2 * j:
                selj_np[r, 128 * j:128 * j + 48] = 1.0
            if r % 8 == 2 * j + 1:
                selj_np[r, 128 * j + 64:128 * j + 112] = 1.0
    selj_c = nc.inline_tensor(selj_np, "selj_c")
    # CHK[8ci+h, ci] = 1
    chk_np = np.zeros((64, 8), np.float32)
    for ci in range(8):
        chk_np[8 * ci:8 * ci + 8, ci] = 1.0
    chk_c = nc.inline_tensor(chk_np, "chk_c")
    on48_np = np.zeros((128, 2), ml_dtypes.bfloat16)
    on48_np[0:48, 0] = 1.0
    on48_np[64:112, 1] = 1.0
    on48_c = nc.inline_tensor(on48_np, "on48_c")
    ones1_np = np.ones((128, 1), ml_dtypes.bfloat16)
    ones1_c = nc.inline_tensor(ones1_np, "ones1_c")

    with tile.TileContext(nc) as tc:
      live_pools = []

      def _pool(**kw):
          p = tc.alloc_tile_pool(**kw)
          live_pools.append(p)
          return p

      def _rel(p):
          p.release()
          live_pools.remove(p)

      try:
        cpool = _pool(name="consts", bufs=1)
        ps = _pool(name="ps", bufs=8, space="PSUM")

        def pst(p=128, f=512, dt=F32):
            return ps.tile([p, f], dt, tag="ps", name="pst")

        def dump_y(items):
            # items: list of (y_block_index, ap [p, <=1024]) — copy (cast) to y
            dbg = _pool(name="dbg", bufs=4)
            for bi, ap in items:
                p, n = ap.shape[0], ap.shape[1]
                st = dbg.tile([128, 1024], F32, tag="dbg", name="st")
                nc.vector.tensor_copy(st[0:p, 0:n], ap)
                nc.sync.dma_start(y_d[128 * bi:128 * bi + p, 0:n], st[0:p, 0:n])
            _rel(dbg)
            raise _Dumped()

        idn = cpool.tile([128, 128], F32)
        idnb = cpool.tile([128, 128], BF16)
        cum = cpool.tile([128, 128], F32)
        mstb = cpool.tile([128, 128], BF16)
        msi = cpool.tile([128, 128], F32)
        negl = cpool.tile([128, 128], F32)
        selj = cpool.tile([64, 512], F32)
        chk = cpool.tile([64, 8], F32)
        on48 = cpool.tile([128, 2], BF16)
        ones1 = cpool.tile([128, 1], BF16)
        for t_, s_ in [(idn, idn_c), (idnb, idnb_c), (cum, cum_c), (mstb, mst_c),
                       (msi, msi_c), (negl, negl_c), (selj, selj_c), (chk, chk_c),
                       (on48, on48_c), (ones1, ones1_c)]:
            nc.sync.dma_start(t_[:], s_[:])
        eps1 = cpool.tile([128, 1], F32)
        nc.vector.memset(eps1[:], 1e-5)
        epsq = cpool.tile([128, 1], F32)
        nc.vector.memset(epsq[:], 48e-6)
        epsk = cpool.tile([128, 1], F32)
        nc.vector.memset(epsk[:], 1e-6)
        epsg = cpool.tile([128, 1], F32)
        nc.vector.memset(epsg[:], 1e-5)
        dtb_r = cpool.tile([1, 64], F32)
        nega_r = cpool.tile([1, 64], F32)
        nc.sync.dma_start(dtb_r[:], dtb_d[:])
        nc.sync.dma_start(nega_r[:], nega_d[:])
        dtb_bc = cpool.tile([128, 64], F32)
        nega_bc = cpool.tile([128, 64], F32)
        nc.gpsimd.partition_broadcast(dtb_bc[:], dtb_r[:])
        nc.gpsimd.partition_broadcast(nega_bc[:], nega_r[:])
        cqw = cpool.tile([128, 16], F32)
        ckw = cpool.tile([128, 16], F32)
        cvw = cpool.tile([128, 32], F32)
        for j in range(4):
            nc.sync.dma_start(cqw[:, 4 * j:4 * j + 4], cq_d[128 * j:128 * j + 128, :])
            nc.sync.dma_start(ckw[:, 4 * j:4 * j + 4], ck_d[128 * j:128 * j + 128, :])
        for j in range(8):
            nc.sync.dma_start(cvw[:, 4 * j:4 * j + 4], cv_d[128 * j:128 * j + 128, :])
        ab_fm = cpool.tile([16, 1024], F32)

        big = _pool(name="big", bufs=1)
        hT = big.tile([128, KT * 1024], BF16)       # also ffT in phase E
        osb = big.tile([128, 6 * 1024], BF16)       # feature-major o: [feat%128, 1024*(f//128)+tok]
        pg = _pool(name="pg", bufs=1)
        g_tok = pg.tile([128, NTOK * VD_C], BF16, tag="gtok")

        wp = _pool(name="wp", bufs=4)
        dram = _pool(name="dram", bufs=1, space="DRAM")
        oT_in = [dram.tile([D, 512], BF16, name=f"oT_in{i}") for i in range(2)]
        oT_out = [dram.tile([D, 512], BF16, name=f"oT_out{i}") for i in range(2)]
        h2T_scr = dram.tile([D, T], F32)
        bfm_scr = dram.tile([64, 128], F32)

        bigq = _pool(name="bigq", bufs=1)
        qsb = bigq.tile([128, 4 * 1024], BF16)
        ksb = bigq.tile([128, 4 * 1024], BF16)
        ktok = bigq.tile([128, 8 * 512], BF16)      # token-major k: [tok, 512ci+128j]
        vtok = bigq.tile([128, 8 * VD_C], BF16)     # token-major v: [tok, 768ci+96h]
        nc.vector.memset(qsb[:], 0.0)
        nc.vector.memset(ksb[:], 0.0)

        # ============ Phase A: hT = rmsnorm(x)^T in bf16 ============
        stA = _pool(name="stA", bufs=16)
        sqp = _pool(name="sqp", bufs=3)
        p_ss = [pst(1, 512) for _ in range(2)]
        xts = []
        for k in range(KT):
            xa = stA.tile([128, 1024], F32, tag="xT")
            nc.sync.dma_start(xa[:], xT_d[128 * k:128 * k + 128, :])
            xts.append(xa)
            sq = sqp.tile([128, 1024], BF16, tag="sq")
            nc.vector.tensor_mul(sq[:], xa[:], xa[:])
            for n in range(2):
                nc.tensor.matmul(p_ss[n][:], ones1[:], sq[:, 512 * n:512 * n + 512],
                                 start=(k == 0), stop=(k == KT - 1))
        srowA = sqp.tile([1, 1024], F32, tag="srowA")
        for n in range(2):
            nc.scalar.activation(srowA[:, 512 * n:512 * n + 512], p_ss[n][:],
                                 AF.Sqrt, bias=eps1[0:1, :], scale=1.0 / D)
        nc.vector.reciprocal_approx_fast(srowA[:], srowA[:])
        sbcA = sqp.tile([128, 1024], F32, tag="sbcA")
        nc.gpsimd.partition_broadcast(sbcA[:], srowA[:])
        for k in range(KT):
            nc.vector.tensor_mul(hT[:, 1024 * k:1024 * k + 1024], xts[k][:], sbcA[:])
        _rel(sqp)
        _rel(stA)
        if DUMP == "hT":
            dump_y([(k, hT[:, 1024 * k:1024 * k + 1024]) for k in range(KT)])

        # ============ Phase B ============
        if "B" not in PHASES:
            raise _SkipRest()
        dk = _pool(name="dk", bufs=1)
        pb = _pool(name="pb", bufs=6)

        def conv_acc(pp0, pp1, cw, j):
            # conv taps read the two 512-wide psum halves directly
            acc = pb.tile([128, 1024], F32, tag="s1k")
            def tap(dst_ap, src_half, lo, hi, s):
                # copy*scale psum half [lo:hi) into dst
                nc.scalar.activation(dst_ap, (pp0 if src_half == 0 else pp1)[:, lo:hi],
                                     AF.Copy, scale=cw[:, 4 * j + 3 - s:4 * j + 4 - s])
            tap(acc[:, 0:512], 0, 0, 512, 0)
            tap(acc[:, 512:1024], 1, 0, 512, 0)
            for s in (1, 2, 3):
                tmp = pb.tile([128, 1024], F32, tag="s1k")
                tap(tmp[:, 0:512], 0, 0, 512, s)
                tap(tmp[:, 512:1024], 1, 0, 512, s)
                nc.gpsimd.tensor_add(acc[:, s:1024], acc[:, s:1024], tmp[:, 0:1024 - s])
            return acc

        def qkv_pass(w_dram, cw, eps_col, mult, kind, jbase, wcol0):
            pps = [[pst() for n in range(2)] for j in range(4)]
            for k in range(KT):
                wt = wp.tile([128, 512], BF16, tag="wwide")
                nc.sync.dma_start(wt[:], w_dram[128 * k:128 * k + 128, wcol0:wcol0 + 512])
                for j in range(4):
                    for n in range(2):
                        nc.tensor.matmul(
                            pps[j][n][:], wt[:, 128 * j:128 * j + 128],
                            hT[:, 1024 * k + 512 * n:1024 * k + 512 * n + 512],
                            start=(k == 0), stop=(k == KT - 1))
            for j in range(4):
                jj = jbase + j
                acc = conv_acc(pps[j][0], pps[j][1], cw, jj)
                if kind == "v":
                    vb = pb.tile([128, 1024], BF16, tag="vb16", bufs=2)
                    nc.scalar.activation(vb[:], acc[:], AF.Silu)
                    for ci in range(8):
                        pv = pst(128, 96, BF16)
                        nc.tensor.transpose(pv[:], vb[0:96, 128 * ci:128 * ci + 128],
                                            idnb[0:96, 0:96])
                        nc.scalar.copy(
                            vtok[:, VD_C * ci + 96 * jj:VD_C * ci + 96 * jj + 96], pv[:])
                else:
                    blk = pb.tile([128, 1024], F32, tag="s1k")
                    nc.scalar.activation(blk[:], acc[:], AF.Silu)
                    sq = pb.tile([128, 1024], BF16, tag="sqb", bufs=2)
                    nc.vector.tensor_mul(sq[:], blk[:], blk[:])
                    dst = qsb if kind == "q" else ksb
                    for hh, rh in ((0, 0), (1, 64)):
                        srow = pb.tile([1, 1024], F32, tag="srow", bufs=2)
                        for n2 in range(2):
                            p_ssq = pst(1, 512)
                            nc.tensor.matmul(
                                p_ssq[:], on48[:, hh:hh + 1], sq[:, 512 * n2:512 * n2 + 512],
                                start=True, stop=True)
                            nc.scalar.activation(srow[:, 512 * n2:512 * n2 + 512], p_ssq[:],
                                                 AF.Sqrt, bias=eps_col[0:1, :], scale=mult)
                        nc.vector.reciprocal_approx_fast(srow[:], srow[:])
                        sbc = pb.tile([128, 1024], F32, tag="sbc", bufs=2)
                        nc.gpsimd.partition_broadcast(sbc[:], srow[:])
                        nc.vector.tensor_mul(dst[rh:rh + 48, 1024 * jj:1024 * jj + 1024],
                                             blk[rh:rh + 48, :], sbc[rh:rh + 48, :])
                    if kind == "k":
                        for ci in range(8):
                            pk = pst(128, 128, BF16)
                            nc.tensor.transpose(
                                pk[:], ksb[:, 1024 * jj + 128 * ci:1024 * jj + 128 * ci + 128],
                                idnb[:])
                            nc.scalar.copy(
                                ktok[:, 512 * ci + 128 * jj:512 * ci + 128 * jj + 128], pk[:])

        qkv_pass(wq_d, cqw, epsq, 48.0, "q", 0, 0)
        qkv_pass(wk_d, ckw, epsk, 1.0, "k", 0, 0)
        qkv_pass(wv_d, cvw, None, None, "v", 0, 0)
        qkv_pass(wv_d, cvw, None, None, "v", 4, 512)
        if DUMP == "qkv":
            dump_y([(j, qsb[:, 1024 * j:1024 * j + 1024]) for j in range(4)]
                   + [(4 + j, ksb[:, 1024 * j:1024 * j + 1024]) for j in range(4)]
                   + [(8 + b, vtok[:, 1024 * b:1024 * b + 1024]) for b in range(6)]
                   + [(14 + b, ktok[:, 1024 * b:1024 * b + 1024]) for b in range(2)])

        # gate token-major
        for n in range(2):
            pgs = [pst(128, 384) for _ in range(NTOK)]
            for k in range(KT):
                wt = wp.tile([128, 384], BF16, tag="wg384")
                nc.sync.dma_start(wt[:], wg_d[128 * k:128 * k + 128, 384 * n:384 * n + 384])
                for i in range(NTOK):
                    nc.tensor.matmul(
                        pgs[i][:], hT[:, 1024 * k + 128 * i:1024 * k + 128 * i + 128], wt[:],
                        start=(k == 0), stop=(k == KT - 1))
            for i in range(NTOK):
                nc.scalar.activation(
                    g_tok[:, VD_C * i + 384 * n:VD_C * i + 384 * n + 384], pgs[i][:], AF.Silu)

        # a/b projections, feature-major [16, 1024]
        ppab = [pst(16, 512) for _ in range(2)]
        for k in range(KT):
            wt = wp.tile([128, 16], BF16, tag="wab")
            nc.sync.dma_start(wt[:], wab_d[128 * k:128 * k + 128, :])
            for n in range(2):
                nc.tensor.matmul(ppab[n][:], wt[:], hT[:, 1024 * k + 512 * n:1024 * k + 512 * n + 512],
                                 start=(k == 0), stop=(k == KT - 1))
        for n in range(2):
            nc.vector.tensor_copy(ab_fm[:, 512 * n:512 * n + 512], ppab[n][:])

        # -------- batched decay prep for all chunks --------
        gta = dk.tile([128, 64], F32)
        bta = dk.tile([128, 64], F32)
        for ci in range(8):
            p_ab = pst(128, 16)
            nc.tensor.transpose(p_ab[:], ab_fm[:, 128 * ci:128 * ci + 128], idn[0:16, 0:16])
            nc.vector.tensor_copy(gta[:, 8 * ci:8 * ci + 8], p_ab[:, 0:8])
            nc.vector.tensor_copy(bta[:, 8 * ci:8 * ci + 8], p_ab[:, 8:16])
        nc.vector.tensor_add(gta[:], gta[:], dtb_bc[:])
        nc.scalar.activation(gta[:], gta[:], AF.Exp)
        nc.vector.tensor_scalar_add(gta[:], gta[:], 1.0)
        nc.scalar.activation(gta[:], gta[:], AF.Ln)
        nc.vector.tensor_mul(gta[:], gta[:], nega_bc[:])        # gt_all [128,64]
        beta_all = dk.tile([128, 64], F32)
        nc.scalar.activation(beta_all[:], bta[:], AF.Sigmoid)
        nbeta_all = dk.tile([128, 64], F32)
        nc.vector.tensor_scalar_mul(nbeta_all[:], beta_all[:], -1.0)
        p_bc = pst(128, 64)
        nc.tensor.matmul(p_bc[:], cum[:], gta[:], start=True, stop=True)
        bcum_tok = dk.tile([128, 64], F32)
        nc.vector.tensor_copy(bcum_tok[:], p_bc[:])
        lam_all = dk.tile([128, 64], F32)
        nc.scalar.activation(lam_all[:], p_bc[:], AF.Exp)
        p_bf = pst(64, 128)
        nc.tensor.transpose(p_bf[:], bcum_tok[:], idn[:])
        b_fm = dk.tile([64, 128], F32)
        nc.vector.tensor_copy(b_fm[:], p_bf[:])
        nc.scalar.dma_start(bfm_scr[:], b_fm[:])
        wfm = dk.tile([64, 128], F32)
        nc.vector.tensor_scalar(wfm[:], b_fm[:], b_fm[:, 127:128], None, OP.subtract)
        nc.scalar.activation(wfm[:], wfm[:], AF.Exp, scale=-1.0)
        p_wt = pst(128, 64)
        nc.tensor.transpose(p_wt[:], wfm[:], idn[0:64, 0:64])
        w_tok = dk.tile([128, 64], F32)
        nc.vector.tensor_copy(w_tok[:], p_wt[:])
        ebc_all = dk.tile([64, 1], F32)
        nc.scalar.activation(ebc_all[:], b_fm[:, 127:128], AF.Exp)
        # EB[8ci+h, ci] = ebc_all[8ci+h]; ebcJ[j][p, ci] = per-(ci,j) state-decay col
        EB = dk.tile([64, 8], F32)
        nc.vector.tensor_scalar_mul(EB[:], chk[:], ebc_all[:, 0:1])
        ebcJ = []
        for j in range(4):
            p_ebj = pst(128, 8)
            nc.tensor.matmul(p_ebj[:], selj[:, 128 * j:128 * j + 128], EB[:],
                             start=True, stop=True)
            ej = dk.tile([128, 8], F32, tag=f"ebj{j}", name=f"ebj{j}")
            nc.vector.tensor_copy(ej[:], p_ebj[:])
            ebcJ.append(ej)
        _rel(pb)

        # ============ Phase C ============
        if "C" not in PHASES:
            raise _SkipRest()
        dput = _pool(name="dput", bufs=16)
        dpa = _pool(name="dpa", bufs=10)
        dpx = _pool(name="dpx", bufs=10)
        dpf = _pool(name="dpf", bufs=6)
        dp2 = _pool(name="dp2", bufs=2)
        spool = _pool(name="spool", bufs=2)

        S_cur = {}
        for j in range(4):
            S_cur[j] = spool.tile([128, DV], F32, tag=f"s{j}", name=f"s{j}")
            nc.vector.memset(S_cur[j][:], 0.0)

        for ci in range(NCH):
            cs = slice(128 * ci, 128 * ci + 128)
            # ---- prep all 8 heads: abar, xx, xt ----
            ABAR, XX, XT = {}, {}, {}
            for j in range(4):
                for hh in range(2):
                    h = 2 * j + hh
                    rh = 64 * hh
                    kts = ksb[rh:rh + 48, 1024 * j + 128 * ci:1024 * j + 128 * ci + 128]
                    qts = qsb[rh:rh + 48, 1024 * j + 128 * ci:1024 * j + 128 * ci + 128]
                    p_kk = pst(128, 128)
                    nc.tensor.matmul(p_kk[:], kts, kts, start=True, stop=True)
                    p_kq = pst(128, 128)
                    nc.tensor.matmul(p_kq[:], kts, qts, start=True, stop=True)
                    bc128 = dpf.tile([128, 128], F32, tag="bc", name="bc128")
                    nc.gpsimd.dma_start(
                        bc128[:],
                        bfm_scr[8 * ci + h:8 * ci + h + 1, :].to_broadcast((128, 128)))
                    dte = dpf.tile([128, 128], F32, tag="dte", name="dte")
                    nc.gpsimd.tensor_scalar(dte[:], bc128[:],
                                            bcum_tok[:, 8 * ci + h:8 * ci + h + 1],
                                            None, OP.subtract)
                    nc.gpsimd.tensor_mul(dte[:], dte[:], msi[:])
                    nc.gpsimd.tensor_add(dte[:], dte[:], negl[:])
                    dincl = dput.tile([128, 128], BF16, tag="ut", name="dincl")
                    nc.scalar.activation(dincl[:], dte[:], AF.Exp)
                    abar = dpa.tile([128, 128], BF16, tag="abar", name="abar")
                    nc.vector.tensor_mul(abar[:], p_kq[:], dincl[:])
                    dstr = dput.tile([128, 128], BF16, tag="ut", name="dstr")
                    nc.gpsimd.tensor_mul(dstr[:], dincl[:], mstb[:])
                    x0 = dput.tile([128, 128], BF16, tag="ut", name="x0")
                    nc.vector.tensor_mul(x0[:], p_kk[:], dstr[:])
                    xx = dpx.tile([128, 128], BF16, tag="xx", name="xx")
                    nc.gpsimd.tensor_scalar_mul(xx[:], x0[:],
                                                nbeta_all[:, 8 * ci + h:8 * ci + h + 1])
                    p_x = pst(128, 128, BF16)
                    nc.tensor.transpose(p_x[:], xx[:], idnb[:])
                    xt = dpx.tile([128, 128], BF16, tag="xt", name="xt")
                    nc.vector.tensor_copy(xt[:], p_x[:])
                    ABAR[h], XX[h], XT[h] = abar, xx, xt

            ot_all = dp2.tile([128, VD_C], F32, tag="otall", name="ot_all")
            # ---- per-head UT inverse + state/output ----
            for j in range(4):
                S_bf = dp2.tile([128, DV], BF16, tag=f"sbf{j}", name="S_bf")
                nc.gpsimd.tensor_copy(S_bf[:], S_cur[j][:])
                p_s = pst(128, DV)
                for hh in range(2):
                    h = 2 * j + hh
                    rh = 64 * hh
                    kts = ksb[rh:rh + 48, 1024 * j + 128 * ci:1024 * j + 128 * ci + 128]
                    qts = qsb[rh:rh + 48, 1024 * j + 128 * ci:1024 * j + 128 * ci + 128]
                    xx, xt = XX[h], XT[h]
                    pmat = dput.tile([128, 128], BF16, tag="ut", name="pmat")
                    nc.vector.tensor_add(pmat[:], xx[:], idnb[:])
                    for lvl in range(6):
                        last = lvl == 5
                        if not last:
                            p_sq = pst(128, 128)
                            nc.tensor.matmul(p_sq[:], xt[:], xx[:], start=True, stop=True)
                            x2 = dput.tile([128, 128], BF16, tag="ut", name="x2")
                            nc.scalar.copy(x2[:], p_sq[:])
                        p_sqt = pst(128, 128)
                        nc.tensor.matmul(p_sqt[:], xx[:], xt[:], start=True, stop=True)
                        xt2 = dput.tile([128, 128], BF16, tag="ut", name="xt2")
                        nc.scalar.copy(xt2[:], p_sqt[:])
                        p_pr = pst(128, 128)
                        nc.tensor.matmul(p_pr[:], xt2[:], pmat[:], start=True, stop=True)
                        pnew = dput.tile([128, 128], BF16, tag="ut", name="pnew")
                        nc.vector.tensor_add(pnew[:], pmat[:], p_pr[:])
                        pmat = pnew
                        if not last:
                            xx, xt = x2, xt2

                    p_ks = pst(128, DV)
                    nc.tensor.matmul(p_ks[:], kts, S_bf[rh:rh + 48, :], start=True, stop=True)
                    r_ = dp2.tile([128, DV], BF16, tag="rr", name="r_")
                    nc.vector.tensor_scalar_mul(r_[:], p_ks[:],
                                                lam_all[:, 8 * ci + h:8 * ci + h + 1])
                    nc.vector.tensor_sub(r_[:], vtok[:, VD_C * ci + 96 * h:VD_C * ci + 96 * h + 96],
                                         r_[:])
                    p_w = pst(128, DV)
                    nc.tensor.matmul(p_w[:], pmat[:], r_[:], start=True, stop=True)
                    u_ = dp2.tile([128, DV], BF16, tag="uu", name="u_")
                    nc.vector.tensor_scalar_mul(u_[:], p_w[:],
                                                beta_all[:, 8 * ci + h:8 * ci + h + 1])
                    p_oi = pst(128, DV)
                    nc.tensor.matmul(p_oi[:], ABAR[h][:], u_[:], start=True, stop=True)
                    p_qs = pst(128, DV)
                    nc.tensor.matmul(p_qs[:], qts, S_bf[rh:rh + 48, :], start=True, stop=True)
                    ots = ot_all[:, 96 * h:96 * h + 96]
                    nc.vector.tensor_scalar_mul(ots, p_qs[:],
                                                lam_all[:, 8 * ci + h:8 * ci + h + 1])
                    nc.vector.tensor_add(ots, ots, p_oi[:])
                    kw = dp2.tile([128, 48], BF16, tag="kw", name="kw")
                    nc.gpsimd.tensor_scalar_mul(
                        kw[:], ktok[:, 512 * ci + 128 * j + rh:512 * ci + 128 * j + rh + 48],
                        w_tok[:, 8 * ci + h:8 * ci + h + 1])
                    nc.tensor.matmul(p_s[rh:rh + 48, :], kw[:], u_[:], start=True, stop=True)

                s_new = spool.tile([128, DV], F32, tag=f"s{j}", name="s_new")
                for rh2 in (0, 64):
                    nc.vector.tensor_scalar_mul(
                        s_new[rh2:rh2 + 48, :], S_cur[j][rh2:rh2 + 48, :],
                        ebcJ[j][rh2:rh2 + 48, ci:ci + 1])
                    nc.vector.tensor_add(
                        s_new[rh2:rh2 + 48, :], s_new[rh2:rh2 + 48, :], p_s[rh2:rh2 + 48, :])
                S_cur[j] = s_new

            # ---- batched gated rmsnorm + gate + transpose to osb ----
            osq = dp2.tile([128, VD_C], F32, tag="osq", name="osq")
            nc.gpsimd.tensor_mul(osq[:], ot_all[:], ot_all[:])
            rcol8 = dp2.tile([128, 8], F32, tag="rc8", name="rcol8")
            for h in range(HP):
                nc.vector.tensor_reduce(rcol8[:, h:h + 1], osq[:, 96 * h:96 * h + 96],
                                        mybir.AxisListType.X, OP.add)
            nc.scalar.activation(rcol8[:], rcol8[:], AF.Sqrt, bias=epsg[:], scale=1.0 / DV)
            nc.vector.reciprocal_approx_fast(rcol8[:], rcol8[:])
            for h in range(HP):
                nc.gpsimd.tensor_scalar_mul(ot_all[:, 96 * h:96 * h + 96],
                                            ot_all[:, 96 * h:96 * h + 96], rcol8[:, h:h + 1])
            ob = dp2.tile([128, VD_C], BF16, tag="ob", name="ob")
            nc.gpsimd.tensor_mul(ob[:], ot_all[:], g_tok[:, VD_C * ci:VD_C * ci + VD_C])
            for b6 in range(6):
                p_ot = pst(128, 128, BF16)
                nc.tensor.transpose(p_ot[:], ob[:, 128 * b6:128 * b6 + 128], idnb[:])
                nc.scalar.copy(osb[:, 1024 * b6 + 128 * ci:1024 * b6 + 128 * ci + 128], p_ot[:])

        for p in (spool, dp2, dpf, dpx, dpa, dput):
            _rel(p)
        if DUMP == "o":
            dump_y([(b, osb[:, 1024 * b:1024 * b + 1024]) for b in range(6)]
                   + [(6 + b, g_tok[:, 1024 * b:1024 * b + 1024]) for b in range(6)]
                   + [(12 + b, ktok[:, 1024 * b:1024 * b + 1024]) for b in range(4)])
        if DUMP == "dk":
            dump_y([(0, gta[:]), (1, beta_all[:]), (2, bcum_tok[:]),
                    (3, lam_all[:]), (4, w_tok[:]), (5, b_fm[:]),
                    (6, ebc_all[:]), (7, ebcJ[0][:]), (8, ebcJ[3][:])])
        _rel(dk)
        _rel(bigq)

        # ============ Phase D: o_projT halves + overlapped AllReduce ============
        if "D" not in PHASES:
            raise _SkipRest()
        wod = _pool(name="wod", bufs=14)
        pd = _pool(name="pd", bufs=4)
        for half in range(2):
            t0 = 512 * half
            for db in range(16):
                wts = []
                for fb in range(6):
                    wt = wod.tile([128, 128], BF16, tag="wo", name="wo_t")
                    nc.sync.dma_start(wt[:], wo_d[128 * fb:128 * fb + 128,
                                                  128 * db:128 * db + 128])
                    wts.append(wt)
                pp = pst()
                for fb in range(6):
                    nc.tensor.matmul(pp[:], wts[fb][:],
                                     osb[:, 1024 * fb + t0:1024 * fb + t0 + 512],
                                     start=(fb == 0), stop=(fb == 5))
                stg = pd.tile([128, 512], BF16, tag="s512b", name="stg")
                nc.scalar.copy(stg[:], pp[:])
                nc.sync.dma_start(oT_in[half][128 * db:128 * db + 128, :], stg[:])
            nc.gpsimd.collective_compute(
                "AllReduce", OP.add, ins=[oT_in[half][:]], outs=[oT_out[half][:]],
                replica_groups=groups)
        if DUMP in ("ar", "oin"):
            src = oT_out if DUMP == "ar" else oT_in
            dbg = _pool(name="dbg", bufs=4)
            for bi in range(16):
                st = dbg.tile([128, 1024], F32, tag="dbg", name="st")
                for half in range(2):
                    so = dbg.tile([128, 512], BF16, tag="dbg2", name="so")
                    nc.sync.dma_start(so[:], src[half][128 * bi:128 * bi + 128, :])
                    nc.vector.tensor_copy(st[:, 512 * half:512 * half + 512], so[:])
                nc.sync.dma_start(y_d[128 * bi:128 * bi + 128, :], st[:])
            _rel(dbg)
            raise _Dumped()

        # ============ Phase E ============
        if "E" not in PHASES:
            raise _SkipRest()
        seq = _pool(name="seq", bufs=3)
        ffT = hT
        p_s2 = [pst(1, 512) for _ in range(2)]
        for k in range(KT):
            xe = seq.tile([128, 1024], F32, tag="xe")
            nc.sync.dma_start(xe[:], xT_d[128 * k:128 * k + 128, :])
            oe = seq.tile([128, 1024], BF16, tag="oe")
            for half in range(2):
                nc.sync.dma_start(oe[:, 512 * half:512 * half + 512],
                                  oT_out[half][128 * k:128 * k + 128, :])
            h2 = seq.tile([128, 1024], F32, tag="h2T")
            nc.vector.tensor_add(h2[:], xe[:], oe[:])
            nc.sync.dma_start(h2T_scr[128 * k:128 * k + 128, :], h2[:])
            sqe = seq.tile([128, 1024], BF16, tag="sqe")
            nc.vector.tensor_mul(sqe[:], h2[:], h2[:])
            for n in range(2):
                nc.tensor.matmul(p_s2[n][:], ones1[:], sqe[:, 512 * n:512 * n + 512],
                                 start=(k == 0), stop=(k == KT - 1))
        srowE = seq.tile([1, 1024], F32, tag="srowE", bufs=1)
        for n in range(2):
            nc.scalar.activation(srowE[:, 512 * n:512 * n + 512], p_s2[n][:],
                                 AF.Sqrt, bias=eps1[0:1, :], scale=1.0 / D)
        nc.vector.reciprocal_approx_fast(srowE[:], srowE[:])
        sbcE = seq.tile([128, 1024], F32, tag="sbcE", bufs=1)
        nc.gpsimd.partition_broadcast(sbcE[:], srowE[:])
        for k in range(KT):
            h2r = seq.tile([128, 1024], F32, tag="h2r", bufs=4)
            nc.sync.dma_start(h2r[:], h2T_scr[128 * k:128 * k + 128, :])
            nc.vector.tensor_mul(ffT[:, 1024 * k:1024 * k + 1024], h2r[:], sbcE[:])
        if DUMP == "ffT":
            dump_y([(k, ffT[:, 1024 * k:1024 * k + 1024]) for k in range(KT)])

        mida = pg.tile([128, 6 * 1024], BF16, tag="gtok")
        pmid = _pool(name="pmid", bufs=1)
        midb = pmid.tile([128, 5 * 1024], BF16, tag="midb")

        def mid_ap(m, off, ln):
            if m < 6:
                return mida[:, 1024 * m + off:1024 * m + off + ln]
            return midb[:, 1024 * (m - 6) + off:1024 * (m - 6) + off + ln]

        for m in range(11):
            pu1 = [pst() for _ in range(2)]
            pu3 = [pst() for _ in range(2)]
            for k in range(KT):
                wt1 = wp.tile([128, 128], BF16, tag="w")
                nc.sync.dma_start(wt1[:], w1_d[128 * k:128 * k + 128, 128 * m:128 * m + 128])
                wt3 = wp.tile([128, 128], BF16, tag="w")
                nc.sync.dma_start(wt3[:], w3_d[128 * k:128 * k + 128, 128 * m:128 * m + 128])
                for n in range(2):
                    rhs = ffT[:, 1024 * k + 512 * n:1024 * k + 512 * n + 512]
                    nc.tensor.matmul(pu1[n][:], wt1[:], rhs, start=(k == 0), stop=(k == KT - 1))
                    nc.tensor.matmul(pu3[n][:], wt3[:], rhs, start=(k == 0), stop=(k == KT - 1))
            for n in range(2):
                u1s = pd.tile([128, 512], F32, tag="s512", name="u1s")
                nc.scalar.activation(u1s[:], pu1[n][:], AF.Silu)
                nc.vector.tensor_mul(mid_ap(m, 512 * n, 512), u1s[:], pu3[n][:])

        if DUMP == "mid":
            dump_y([(m, mid_ap(m, 0, 1024)) for m in range(11)])
        wp2 = _pool(name="wp2", bufs=12)
        for db in range(16):
            wts = []
            for m in range(11):
                wt = wp2.tile([128, 128], BF16, tag="w2", name="w2_t")
                nc.sync.dma_start(wt[:], w2_d[128 * m:128 * m + 128, 128 * db:128 * db + 128])
                wts.append(wt)
            for half in range(2):
                pps = pst()
                for m in range(11):
                    nc.tensor.matmul(pps[:], wts[m][:], mid_ap(m, 512 * half, 512),
                                     start=(m == 0), stop=(m == 10))
                h2t = pd.tile([128, 512], F32, tag="s512", name="h2t")
                nc.sync.dma_start(h2t[:], h2T_scr[128 * db:128 * db + 128,
                                                  512 * half:512 * half + 512])
                yst = pd.tile([128, 512], F32, tag="s512", name="yst")
                nc.vector.tensor_scalar_mul(yst[:], h2t[:], 0.25)
                nc.vector.tensor_add(yst[:], yst[:], pps[:])
                nc.sync.dma_start(y_d[128 * db:128 * db + 128, 512 * half:512 * half + 512],
                                  yst[:])

        for p in (wp2, pmid, seq, pd, wod, dram, wp, pg, big, ps, cpool):
            _rel(p)
      except _SkipRest:
        zst = _pool(name="zst", bufs=1)
        zt = zst.tile([128, 512], F32)
        nc.vector.memset(zt[:], 0.0)
        for i in range(16):
            for dh in range(2):
                nc.sync.dma_start(y_d[128 * i:128 * i + 128, 512 * dh:512 * dh + 512], zt[:])
        for p in reversed(live_pools):
            p.release()
      except _Dumped:
        for p in reversed(live_pools):
            p.release()

    nc.compile()
    return nc


def _shard(inputs):
    f32 = np.float32
    bf = ml_dtypes.bfloat16
    rms1 = np.asarray(inputs["rms1_w"], f32)
    rms2 = np.asarray(inputs["rms2_w"], f32)
    gn = np.asarray(inputs["gnorm_w"], f32)
    in_maps = []
    for c in range(8):
        g, m = c // 4, c % 4
        qs = slice(384 * m, 384 * m + 384)
        vs = slice(768 * m, 768 * m + 768)
        hs = slice(8 * m, 8 * m + 8)
        isl = slice(1408 * m, 1408 * m + 1408)

        def padqk(w):
            wp_ = np.zeros((D, QKP), f32)
            for h in range(8):
                wp_[:, 64 * h:64 * h + 48] = w[:, 48 * h:48 * h + 48]
            return wp_

        def padcw(w):
            cp = np.zeros((QKP, 4), f32)
            for h in range(8):
                cp[64 * h:64 * h + 48] = w[48 * h:48 * h + 48]
            return cp

        def padv(w):
            colpad = w.shape[0] == D
            out = np.zeros((D, VP) if colpad else (VP, w.shape[1]), f32)
            for h in range(8):
                if colpad:
                    out[:, 128 * h:128 * h + 96] = w[:, 96 * h:96 * h + 96]
                else:
                    out[128 * h:128 * h + 96] = w[96 * h:96 * h + 96]
            return out

        dtb8 = np.asarray(inputs["dt_bias"], f32)[hs]
        nega8 = -np.exp(np.asarray(inputs["A_log"], f32)[hs])
        in_maps.append(dict(
            xT=np.ascontiguousarray(np.asarray(inputs["x"], f32)[g].T),
            wq=padqk(np.asarray(inputs["Wq"], f32)[:, qs] * rms1[:, None]).astype(bf),
            wk=padqk(np.asarray(inputs["Wk"], f32)[:, qs] * rms1[:, None]).astype(bf),
            wv=padv(np.asarray(inputs["Wv"], f32)[:, vs] * rms1[:, None]).astype(bf),
            wg=np.ascontiguousarray(
                np.asarray(inputs["Wg"], f32)[:, vs] * rms1[:, None]).astype(bf),
            wab=np.ascontiguousarray(np.concatenate(
                [np.asarray(inputs["Wa"], f32)[:, hs],
                 np.asarray(inputs["Wb"], f32)[:, hs]], 1) * rms1[:, None]).astype(bf),
            cq=padcw(np.asarray(inputs["conv_q_w"], f32)[qs]),
            ck=padcw(np.asarray(inputs["conv_k_w"], f32)[qs]),
            cv=padv(np.asarray(inputs["conv_v_w"], f32)[vs]),
            dtb=np.tile(dtb8, 8).reshape(1, 64).copy(),
            nega=np.tile(nega8, 8).reshape(1, 64).copy(),
            wo=np.ascontiguousarray(
                np.asarray(inputs["Wo"], f32)[vs] * np.tile(gn, 8)[:, None]).astype(bf),
            w1=np.ascontiguousarray(
                np.asarray(inputs["W1"], f32)[:, isl] * rms2[:, None]).astype(bf),
            w3=np.ascontiguousarray(
                np.asarray(inputs["W3"], f32)[:, isl] * rms2[:, None]).astype(bf),
            w2=np.ascontiguousarray(np.asarray(inputs["W2"], f32)[isl]).astype(bf),
        ))
    return in_maps


def kernel(**inputs):
    if "nc" not in _cache:
        _cache["nc"] = _build(8)
    res = run_bass_kernel_spmd(_cache["nc"], _shard(inputs), list(range(8)))
    out = np.zeros((B, T, D), np.float32)
    for g in range(2):
        yT = sum(res.results[4 * g + m]["y"] for m in range(4))
        out[g] = yT.T
    return out


# revision 36
# speedup vs baseline: 1.2692x; 1.2692x over previous
"""GatedDeltaNet block kernel for 8 Trainium2 cores (Bass/Tile), bf16 rework.

Sharding: DP2 (batch) x TP4 (heads / MLP-inter). Core c: group g=c//4 runs
batch g; member m=c%4 owns heads [8m,8m+8), q/k cols [384m,..), v/g cols
[768m,..), INTER [1408m,..). Two half-token AllReduces per 4-core group after
o_proj (overlapped with o_proj compute); final down-proj partials summed on
the host.

Everything runs feature-major (host passes x transposed, takes y transposed)
so there are no PE transposes outside the delta-rule inner loop. All big
GEMM operands are bf16 (host-cast weights); psum accumulation, the delta-rule
state, decay/beta math and norms stay fp32.

Per-core dataflow:
  A: xT [D,T] -> rmsnorm via matmul-accumulated column sumsq -> hT bf16 (SBUF)
  B: bf16 projections off hT; q/k feature-major (heads padded to 64 rows)
     -> conv+silu+l2norm -> SBUF (+ token-major copies of k, v via PE
     transposes); gate token-major; a/b -> batched decay prep for all chunks
  C: chunked gated delta rule (C=128, UT transform via log-doubling inverse,
     bf16 matmuls / fp32 state); writes normed+gated o feature-major to SBUF
  D: o_projT in two token halves, each followed by its AllReduce (overlapped)
  E: h2T = xT + oT; rmsnorm -> ffT bf16 (reuses hT); MLP bf16; yT partials
"""
import sys
sys.path.insert(0, '/opt/trn_rl_repo')
import numpy as np
import ml_dtypes

import concourse.bass as bass
import concourse.bacc as bacc
import concourse.mybir as mybir
import concourse.tile as tile
from concourse.bass_isa import ReduceOp
from concourse.bass_utils import run_bass_kernel_spmd

F32 = mybir.dt.float32
BF16 = mybir.dt.bfloat16
AF = mybir.ActivationFunctionType
OP = mybir.AluOpType

B, T, D = 2, 1024, 2048
H, DK, DV = 32, 48, 96
HP = 8
QKP = 512
VD_C = 768
VP = 1024
INT_C = 1408
C = 128
NCHUNK = T // C
KT = D // 128
NTOK = T // 128

_cache = {}
import os
PHASES = os.environ.get("DN_PHASES", "ABCDE")
NCH = int(os.environ.get("DN_NCHUNK", str(T // C)))
DUMP = os.environ.get("DN_DUMP", "")


class _SkipRest(Exception):
    pass


class _Dumped(Exception):
    pass


def _build(n_cores=8):
    groups = [[0, 1, 2, 3], [4, 5, 6, 7]] if n_cores == 8 else [[0]]
    nc = bacc.Bacc("TRN2", target_bir_lowering=False, debug=False, num_devices=n_cores)

    xT_d = nc.dram_tensor("xT", [D, T], F32, kind="ExternalInput")
    wq_d = nc.dram_tensor("wq", [D, QKP], BF16, kind="ExternalInput")
    wk_d = nc.dram_tensor("wk", [D, QKP], BF16, kind="ExternalInput")
    wv_d = nc.dram_tensor("wv", [D, VP], BF16, kind="ExternalInput")
    wg_d = nc.dram_tensor("wg", [D, VD_C], BF16, kind="ExternalInput")
    wab_d = nc.dram_tensor("wab", [D, 16], BF16, kind="ExternalInput")
    cq_d = nc.dram_tensor("cq", [QKP, 4], F32, kind="ExternalInput")
    ck_d = nc.dram_tensor("ck", [QKP, 4], F32, kind="ExternalInput")
    cv_d = nc.dram_tensor("cv", [VP, 4], F32, kind="ExternalInput")
    dtb_d = nc.dram_tensor("dtb", [1, 64], F32, kind="ExternalInput")
    nega_d = nc.dram_tensor("nega", [1, 64], F32, kind="ExternalInput")
    wo_d = nc.dram_tensor("wo", [VD_C, D], BF16, kind="ExternalInput")
    w1_d = nc.dram_tensor("w1", [D, INT_C], BF16, kind="ExternalInput")
    w3_d = nc.dram_tensor("w3", [D, INT_C], BF16, kind="ExternalInput")
    w2_d = nc.dram_tensor("w2", [INT_C, D], BF16, kind="ExternalInput")
    y_d = nc.dram_tensor("y", [D, T], F32, kind="ExternalOutput")

    ones = np.ones((128, 128), np.float32)
    idn_c = nc.inline_tensor(np.eye(128, dtype=np.float32), "idn_c")
    idnb_c = nc.inline_tensor(np.eye(128, dtype=ml_dtypes.bfloat16), "idnb_c")
    cum_c = nc.inline_tensor(np.triu(ones).copy(), "cum_c")
    mst_c = nc.inline_tensor(np.triu(ones, 1).astype(ml_dtypes.bfloat16), "mst_c")
    msi_c = nc.inline_tensor(np.triu(ones).copy(), "msi_c")
    negl_c = nc.inline_tensor((np.tril(ones, -1) * -1e30).copy(), "negl_c")
    # SELJ[r, 128j+p] = 1 iff (r%8==2j and p<48) or (r%8==2j+1 and 64<=p<112)
    selj_np = np.zeros((64, 512), np.float32)
    for r in range(64):
        for j in range(4):
            if r % 8 == 2 * j:
                selj_np[r, 128 * j:128 * j + 48] = 1.0
            if r % 8 == 2 * j + 1:
                selj_np[r, 128 * j + 64:128 * j + 112] = 1.0
    selj_c = nc.inline_tensor(selj_np, "selj_c")
    # CHK[8ci+h, ci] = 1
    chk_np = np.zeros((64, 8), np.float32)
    for ci in range(8):
        chk_np[8 * ci:8 * ci + 8, ci] = 1.0
    chk_c = nc.inline_tensor(chk_np, "chk_c")
    on48_np = np.zeros((128, 2), ml_dtypes.bfloat16)
    on48_np[0:48, 0] = 1.0
    on48_np[64:112, 1] = 1.0
    on48_c = nc.inline_tensor(on48_np, "on48_c")
    ones1_np = np.ones((128, 1), ml_dtypes.bfloat16)
    ones1_c = nc.inline_tensor(ones1_np, "ones1_c")

    with tile.TileContext(nc) as tc:
      live_pools = []

      def _pool(**kw):
          p = tc.alloc_tile_pool(**kw)
          live_pools.append(p)
          return p

      def _rel(p):
          p.release()
          live_pools.remove(p)

      try:
        cpool = _pool(name="consts", bufs=1)
        ps = _pool(name="ps", bufs=8, space="PSUM")

        def pst(p=128, f=512, dt=F32):
            return ps.tile([p, f], dt, tag="ps", name="pst")

        def dump_y(items):
            # items: list of (y_block_index, ap [p, <=1024]) — copy (cast) to y
            dbg = _pool(name="dbg", bufs=4)
            for bi, ap in items:
                p, n = ap.shape[0], ap.shape[1]
                st = dbg.tile([128, 1024], F32, tag="dbg", name="st")
                nc.vector.tensor_copy(st[0:p, 0:n], ap)
                nc.sync.dma_start(y_d[128 * bi:128 * bi + p, 0:n], st[0:p, 0:n])
            _rel(dbg)
            raise _Dumped()

        idn = cpool.tile([128, 128], F32)
        idnb = cpool.tile([128, 128], BF16)
        cum = cpool.tile([128, 128], F32)
        mstb = cpool.tile([128, 128], BF16)
        msi = cpool.tile([128, 128], F32)
        negl = cpool.tile([128, 128], F32)
        selj = cpool.tile([64, 512], F32)
        chk = cpool.tile([64, 8], F32)
        on48 = cpool.tile([128, 2], BF16)
        ones1 = cpool.tile([128, 1], BF16)
        for t_, s_ in [(idn, idn_c), (idnb, idnb_c), (cum, cum_c), (mstb, mst_c),
                       (msi, msi_c), (negl, negl_c), (selj, selj_c), (chk, chk_c),
                       (on48, on48_c), (ones1, ones1_c)]:
            nc.sync.dma_start(t_[:], s_[:])
        eps1 = cpool.tile([128, 1], F32)
        nc.vector.memset(eps1[:], 1e-5)
        epsq = cpool.tile([128, 1], F32)
        nc.vector.memset(epsq[:], 48e-6)
        epsk = cpool.tile([128, 1], F32)
        nc.vector.memset(epsk[:], 1e-6)
        epsg = cpool.tile([128, 1], F32)
        nc.vector.memset(epsg[:], 1e-5)
        dtb_r = cpool.tile([1, 64], F32)
        nega_r = cpool.tile([1, 64], F32)
        nc.sync.dma_start(dtb_r[:], dtb_d[:])
        nc.sync.dma_start(nega_r[:], nega_d[:])
        dtb_bc = cpool.tile([128, 64], F32)
        nega_bc = cpool.tile([128, 64], F32)
        nc.gpsimd.partition_broadcast(dtb_bc[:], dtb_r[:])
        nc.gpsimd.partition_broadcast(nega_bc[:], nega_r[:])
        cqw = cpool.tile([128, 16], F32)
        ckw = cpool.tile([128, 16], F32)
        cvw = cpool.tile([128, 32], F32)
        for j in range(4):
            nc.sync.dma_start(cqw[:, 4 * j:4 * j + 4], cq_d[128 * j:128 * j + 128, :])
            nc.sync.dma_start(ckw[:, 4 * j:4 * j + 4], ck_d[128 * j:128 * j + 128, :])
        for j in range(8):
            nc.sync.dma_start(cvw[:, 4 * j:4 * j + 4], cv_d[128 * j:128 * j + 128, :])
        ab_fm = cpool.tile([16, 1024], F32)

        big = _pool(name="big", bufs=1)
        hT = big.tile([128, KT * 1024], BF16)       # also ffT in phase E
        osb = big.tile([128, 6 * 1024], BF16)       # feature-major o: [feat%128, 1024*(f//128)+tok]
        pg = _pool(name="pg", bufs=1)
        g_tok = pg.tile([128, NTOK * VD_C], BF16, tag="gtok")

        wp = _pool(name="wp", bufs=4)
        dram = _pool(name="dram", bufs=1, space="DRAM")
        oT_in = [dram.tile([D, 512], BF16, name=f"oT_in{i}") for i in range(2)]
        oT_out = [dram.tile([D, 512], BF16, name=f"oT_out{i}") for i in range(2)]
        h2T_scr = dram.tile([D, T], F32)
        bfm_scr = dram.tile([64, 128], F32)

        bigq = _pool(name="bigq", bufs=1)
        qsb = bigq.tile([128, 4 * 1024], BF16)
        ksb = bigq.tile([128, 4 * 1024], BF16)
        ktok = bigq.tile([128, 8 * 512], BF16)      # token-major k: [tok, 512ci+128j]
        vtok = bigq.tile([128, 8 * VD_C], BF16)     # token-major v: [tok, 768ci+96h]
        nc.vector.memset(qsb[:], 0.0)
        nc.vector.memset(ksb[:], 0.0)

        # ============ Phase A: hT = rmsnorm(x)^T in bf16 ============
        stA = _pool(name="stA", bufs=16)
        sqp = _pool(name="sqp", bufs=3)
        p_ss = [pst(1, 512) for _ in range(2)]
        xts = []
        for k in range(KT):
            xa = stA.tile([128, 1024], F32, tag="xT")
            nc.sync.dma_start(xa[:], xT_d[128 * k:128 * k + 128, :])
            xts.append(xa)
            sq = sqp.tile([128, 1024], BF16, tag="sq")
            nc.vector.tensor_mul(sq[:], xa[:], xa[:])
            for n in range(2):
                nc.tensor.matmul(p_ss[n][:], ones1[:], sq[:, 512 * n:512 * n + 512],
                                 start=(k == 0), stop=(k == KT - 1))
        srowA = sqp.tile([1, 1024], F32, tag="srowA")
        for n in range(2):
            nc.scalar.activation(srowA[:, 512 * n:512 * n + 512], p_ss[n][:],
                                 AF.Sqrt, bias=eps1[0:1, :], scale=1.0 / D)
        nc.vector.reciprocal_approx_fast(srowA[:], srowA[:])
        sbcA = sqp.tile([128, 1024], F32, tag="sbcA")
        nc.gpsimd.partition_broadcast(sbcA[:], srowA[:])
        for k in range(KT):
            nc.vector.tensor_mul(hT[:, 1024 * k:1024 * k + 1024], xts[k][:], sbcA[:])
        _rel(sqp)
        _rel(stA)
        if DUMP == "hT":
            dump_y([(k, hT[:, 1024 * k:1024 * k + 1024]) for k in range(KT)])

        # ============ Phase B ============
        if "B" not in PHASES:
            raise _SkipRest()
        dk = _pool(name="dk", bufs=1)
        pb = _pool(name="pb", bufs=6)

        def conv_acc(pre, cw, j):
            acc = pb.tile([128, 1024], F32, tag="s1k")
            nc.scalar.activation(acc[:], pre[:], AF.Copy, scale=cw[:, 4 * j + 3:4 * j + 4])
            for s in (1, 2, 3):
                tmp = pb.tile([128, 1024], F32, tag="s1k")
                nc.scalar.activation(tmp[:, 0:1024 - s], pre[:, 0:1024 - s],
                                     AF.Copy, scale=cw[:, 4 * j + 3 - s:4 * j + 4 - s])
                nc.vector.tensor_add(acc[:, s:1024], acc[:, s:1024], tmp[:, 0:1024 - s])
            return acc

        def qkv_pass(w_dram, cw, eps_col, mult, kind, jbase, wcol0):
            pps = [[pst() for n in range(2)] for j in range(4)]
            for k in range(KT):
                wt = wp.tile([128, 512], BF16, tag="wwide")
                nc.sync.dma_start(wt[:], w_dram[128 * k:128 * k + 128, wcol0:wcol0 + 512])
                for j in range(4):
                    for n in range(2):
                        nc.tensor.matmul(
                            pps[j][n][:], wt[:, 128 * j:128 * j + 128],
                            hT[:, 1024 * k + 512 * n:1024 * k + 512 * n + 512],
                            start=(k == 0), stop=(k == KT - 1))
            for j in range(4):
                jj = jbase + j
                pre = pb.tile([128, 1024], F32, tag="s1k")
                for n in range(2):
                    nc.vector.tensor_copy(pre[:, 512 * n:512 * n + 512], pps[j][n][:])
                acc = conv_acc(pre, cw, jj)
                if kind == "v":
                    vb = pb.tile([128, 1024], BF16, tag="vb16", bufs=2)
                    nc.scalar.activation(vb[:], acc[:], AF.Silu)
                    for ci in range(8):
                        pv = pst(128, 96, BF16)
                        nc.tensor.transpose(pv[:], vb[0:96, 128 * ci:128 * ci + 128],
                                            idnb[0:96, 0:96])
                        nc.scalar.copy(
                            vtok[:, VD_C * ci + 96 * jj:VD_C * ci + 96 * jj + 96], pv[:])
                else:
                    blk = pb.tile([128, 1024], F32, tag="s1k")
                    nc.scalar.activation(blk[:], acc[:], AF.Silu)
                    sq = pb.tile([128, 1024], BF16, tag="sqb", bufs=2)
                    nc.vector.tensor_mul(sq[:], blk[:], blk[:])
                    dst = qsb if kind == "q" else ksb
                    for hh, rh in ((0, 0), (1, 64)):
                        srow = pb.tile([1, 1024], F32, tag="srow", bufs=2)
                        for n2 in range(2):
                            p_ssq = pst(1, 512)
                            nc.tensor.matmul(
                                p_ssq[:], on48[:, hh:hh + 1], sq[:, 512 * n2:512 * n2 + 512],
                                start=True, stop=True)
                            nc.scalar.activation(srow[:, 512 * n2:512 * n2 + 512], p_ssq[:],
                                                 AF.Sqrt, bias=eps_col[0:1, :], scale=mult)
                        nc.vector.reciprocal_approx_fast(srow[:], srow[:])
                        sbc = pb.tile([128, 1024], F32, tag="sbc", bufs=2)
                        nc.gpsimd.partition_broadcast(sbc[:], srow[:])
                        nc.vector.tensor_mul(dst[rh:rh + 48, 1024 * jj:1024 * jj + 1024],
                                             blk[rh:rh + 48, :], sbc[rh:rh + 48, :])
                    if kind == "k":
                        for ci in range(8):
                            pk = pst(128, 128, BF16)
                            nc.tensor.transpose(
                                pk[:], ksb[:, 1024 * jj + 128 * ci:1024 * jj + 128 * ci + 128],
                                idnb[:])
                            nc.scalar.copy(
                                ktok[:, 512 * ci + 128 * jj:512 * ci + 128 * jj + 128], pk[:])

        qkv_pass(wq_d, cqw, epsq, 48.0, "q", 0, 0)
        qkv_pass(wk_d, ckw, epsk, 1.0, "k", 0, 0)
        qkv_pass(wv_d, cvw, None, None, "v", 0, 0)
        qkv_pass(wv_d, cvw, None, None, "v", 4, 512)
        if DUMP == "qkv":
            dump_y([(j, qsb[:, 1024 * j:1024 * j + 1024]) for j in range(4)]
                   + [(4 + j, ksb[:, 1024 * j:1024 * j + 1024]) for j in range(4)]
                   + [(8 + b, vtok[:, 1024 * b:1024 * b + 1024]) for b in range(6)]
                   + [(14 + b, ktok[:, 1024 * b:1024 * b + 1024]) for b in range(2)])

        # gate token-major
        for n in range(2):
            pgs = [pst(128, 384) for _ in range(NTOK)]
            for k in range(KT):
                wt = wp.tile([128, 384], BF16, tag="wg384")
                nc.sync.dma_start(wt[:], wg_d[128 * k:128 * k + 128, 384 * n:384 * n + 384])
                for i in range(NTOK):
                    nc.tensor.matmul(
                        pgs[i][:], hT[:, 1024 * k + 128 * i:1024 * k + 128 * i + 128], wt[:],
                        start=(k == 0), stop=(k == KT - 1))
            for i in range(NTOK):
                nc.scalar.activation(
                    g_tok[:, VD_C * i + 384 * n:VD_C * i + 384 * n + 384], pgs[i][:], AF.Silu)

        # a/b projections, feature-major [16, 1024]
        ppab = [pst(16, 512) for _ in range(2)]
        for k in range(KT):
            wt = wp.tile([128, 16], BF16, tag="wab")
            nc.sync.dma_start(wt[:], wab_d[128 * k:128 * k + 128, :])
            for n in range(2):
                nc.tensor.matmul(ppab[n][:], wt[:], hT[:, 1024 * k + 512 * n:1024 * k + 512 * n + 512],
                                 start=(k == 0), stop=(k == KT - 1))
        for n in range(2):
            nc.vector.tensor_copy(ab_fm[:, 512 * n:512 * n + 512], ppab[n][:])

        # -------- batched decay prep for all chunks --------
        gta = dk.tile([128, 64], F32)
        bta = dk.tile([128, 64], F32)
        for ci in range(8):
            p_ab = pst(128, 16)
            nc.tensor.transpose(p_ab[:], ab_fm[:, 128 * ci:128 * ci + 128], idn[0:16, 0:16])
            nc.vector.tensor_copy(gta[:, 8 * ci:8 * ci + 8], p_ab[:, 0:8])
            nc.vector.tensor_copy(bta[:, 8 * ci:8 * ci + 8], p_ab[:, 8:16])
        nc.vector.tensor_add(gta[:], gta[:], dtb_bc[:])
        nc.scalar.activation(gta[:], gta[:], AF.Exp)
        nc.vector.tensor_scalar_add(gta[:], gta[:], 1.0)
        nc.scalar.activation(gta[:], gta[:], AF.Ln)
        nc.vector.tensor_mul(gta[:], gta[:], nega_bc[:])        # gt_all [128,64]
        beta_all = dk.tile([128, 64], F32)
        nc.scalar.activation(beta_all[:], bta[:], AF.Sigmoid)
        nbeta_all = dk.tile([128, 64], F32)
        nc.vector.tensor_scalar_mul(nbeta_all[:], beta_all[:], -1.0)
        p_bc = pst(128, 64)
        nc.tensor.matmul(p_bc[:], cum[:], gta[:], start=True, stop=True)
        bcum_tok = dk.tile([128, 64], F32)
        nc.vector.tensor_copy(bcum_tok[:], p_bc[:])
        lam_all = dk.tile([128, 64], F32)
        nc.scalar.activation(lam_all[:], p_bc[:], AF.Exp)
        p_bf = pst(64, 128)
        nc.tensor.transpose(p_bf[:], bcum_tok[:], idn[:])
        b_fm = dk.tile([64, 128], F32)
        nc.vector.tensor_copy(b_fm[:], p_bf[:])
        nc.scalar.dma_start(bfm_scr[:], b_fm[:])
        wfm = dk.tile([64, 128], F32)
        nc.vector.tensor_scalar(wfm[:], b_fm[:], b_fm[:, 127:128], None, OP.subtract)
        nc.scalar.activation(wfm[:], wfm[:], AF.Exp, scale=-1.0)
        p_wt = pst(128, 64)
        nc.tensor.transpose(p_wt[:], wfm[:], idn[0:64, 0:64])
        w_tok = dk.tile([128, 64], F32)
        nc.vector.tensor_copy(w_tok[:], p_wt[:])
        ebc_all = dk.tile([64, 1], F32)
        nc.scalar.activation(ebc_all[:], b_fm[:, 127:128], AF.Exp)
        # EB[8ci+h, ci] = ebc_all[8ci+h]; ebcJ[j][p, ci] = per-(ci,j) state-decay col
        EB = dk.tile([64, 8], F32)
        nc.vector.tensor_scalar_mul(EB[:], chk[:], ebc_all[:, 0:1])
        ebcJ = []
        for j in range(4):
            p_ebj = pst(128, 8)
            nc.tensor.matmul(p_ebj[:], selj[:, 128 * j:128 * j + 128], EB[:],
                             start=True, stop=True)
            ej = dk.tile([128, 8], F32, tag=f"ebj{j}", name=f"ebj{j}")
            nc.vector.tensor_copy(ej[:], p_ebj[:])
            ebcJ.append(ej)
        _rel(pb)

        # ============ Phase C ============
        if "C" not in PHASES:
            raise _SkipRest()
        dput = _pool(name="dput", bufs=16)
        dpa = _pool(name="dpa", bufs=10)
        dpx = _pool(name="dpx", bufs=10)
        dpf = _pool(name="dpf", bufs=6)
        dp2 = _pool(name="dp2", bufs=2)
        spool = _pool(name="spool", bufs=2)

        S_cur = {}
        for j in range(4):
            S_cur[j] = spool.tile([128, DV], F32, tag=f"s{j}", name=f"s{j}")
            nc.vector.memset(S_cur[j][:], 0.0)

        for ci in range(NCH):
            cs = slice(128 * ci, 128 * ci + 128)
            # ---- prep all 8 heads: abar, xx, xt ----
            ABAR, XX, XT = {}, {}, {}
            for j in range(4):
                for hh in range(2):
                    h = 2 * j + hh
                    rh = 64 * hh
                    kts = ksb[rh:rh + 48, 1024 * j + 128 * ci:1024 * j + 128 * ci + 128]
                    qts = qsb[rh:rh + 48, 1024 * j + 128 * ci:1024 * j + 128 * ci + 128]
                    p_kk = pst(128, 128)
                    nc.tensor.matmul(p_kk[:], kts, kts, start=True, stop=True)
                    p_kq = pst(128, 128)
                    nc.tensor.matmul(p_kq[:], kts, qts, start=True, stop=True)
                    bc128 = dpf.tile([128, 128], F32, tag="bc", name="bc128")
                    nc.gpsimd.dma_start(
                        bc128[:],
                        bfm_scr[8 * ci + h:8 * ci + h + 1, :].to_broadcast((128, 128)))
                    dte = dpf.tile([128, 128], F32, tag="dte", name="dte")
                    nc.vector.tensor_scalar(dte[:], bc128[:],
                                            bcum_tok[:, 8 * ci + h:8 * ci + h + 1],
                                            None, OP.subtract)
                    nc.vector.tensor_mul(dte[:], dte[:], msi[:])
                    nc.vector.tensor_add(dte[:], dte[:], negl[:])
                    dincl = dput.tile([128, 128], BF16, tag="ut", name="dincl")
                    nc.scalar.activation(dincl[:], dte[:], AF.Exp)
                    abar = dpa.tile([128, 128], BF16, tag="abar", name="abar")
                    nc.vector.tensor_mul(abar[:], p_kq[:], dincl[:])
                    dstr = dput.tile([128, 128], BF16, tag="ut", name="dstr")
                    nc.vector.tensor_mul(dstr[:], dincl[:], mstb[:])
                    x0 = dput.tile([128, 128], BF16, tag="ut", name="x0")
                    nc.vector.tensor_mul(x0[:], p_kk[:], dstr[:])
                    xx = dpx.tile([128, 128], BF16, tag="xx", name="xx")
                    nc.vector.tensor_scalar_mul(xx[:], x0[:],
                                                nbeta_all[:, 8 * ci + h:8 * ci + h + 1])
                    p_x = pst(128, 128, BF16)
                    nc.tensor.transpose(p_x[:], xx[:], idnb[:])
                    xt = dpx.tile([128, 128], BF16, tag="xt", name="xt")
                    nc.vector.tensor_copy(xt[:], p_x[:])
                    ABAR[h], XX[h], XT[h] = abar, xx, xt

            ot_all = dp2.tile([128, VD_C], F32, tag="otall", name="ot_all")
            # ---- per-head UT inverse + state/output ----
            for j in range(4):
                S_bf = dp2.tile([128, DV], BF16, tag=f"sbf{j}", name="S_bf")
                nc.vector.tensor_copy(S_bf[:], S_cur[j][:])
                p_s = pst(128, DV)
                for hh in range(2):
                    h = 2 * j + hh
                    rh = 64 * hh
                    kts = ksb[rh:rh + 48, 1024 * j + 128 * ci:1024 * j + 128 * ci + 128]
                    qts = qsb[rh:rh + 48, 1024 * j + 128 * ci:1024 * j + 128 * ci + 128]
                    xx, xt = XX[h], XT[h]
                    pmat = dput.tile([128, 128], BF16, tag="ut", name="pmat")
                    nc.vector.tensor_add(pmat[:], xx[:], idnb[:])
                    for lvl in range(6):
                        last = lvl == 5
                        if not last:
                            p_sq = pst(128, 128)
                            nc.tensor.matmul(p_sq[:], xt[:], xx[:], start=True, stop=True)
                            x2 = dput.tile([128, 128], BF16, tag="ut", name="x2")
                            nc.scalar.copy(x2[:], p_sq[:])
                        p_sqt = pst(128, 128)
                        nc.tensor.matmul(p_sqt[:], xx[:], xt[:], start=True, stop=True)
                        xt2 = dput.tile([128, 128], BF16, tag="ut", name="xt2")
                        nc.scalar.copy(xt2[:], p_sqt[:])
                        p_pr = pst(128, 128)
                        nc.tensor.matmul(p_pr[:], xt2[:], pmat[:], start=True, stop=True)
                        pnew = dput.tile([128, 128], BF16, tag="ut", name="pnew")
                        nc.vector.tensor_add(pnew[:], pmat[:], p_pr[:])
                        pmat = pnew
                        if not last:
                            xx, xt = x2, xt2

                    p_ks = pst(128, DV)
                    nc.tensor.matmul(p_ks[:], kts, S_bf[rh:rh + 48, :], start=True, stop=True)
                    r_ = dp2.tile([128, DV], BF16, tag="rr", name="r_")
                    nc.vector.tensor_scalar_mul(r_[:], p_ks[:],
                                                lam_all[:, 8 * ci + h:8 * ci + h + 1])
                    nc.vector.tensor_sub(r_[:], vtok[:, VD_C * ci + 96 * h:VD_C * ci + 96 * h + 96],
                                         r_[:])
                    p_w = pst(128, DV)
                    nc.tensor.matmul(p_w[:], pmat[:], r_[:], start=True, stop=True)
                    u_ = dp2.tile([128, DV], BF16, tag="uu", name="u_")
                    nc.vector.tensor_scalar_mul(u_[:], p_w[:],
                                                beta_all[:, 8 * ci + h:8 * ci + h + 1])
                    p_oi = pst(128, DV)
                    nc.tensor.matmul(p_oi[:], ABAR[h][:], u_[:], start=True, stop=True)
                    p_qs = pst(128, DV)
                    nc.tensor.matmul(p_qs[:], qts, S_bf[rh:rh + 48, :], start=True, stop=True)
                    ots = ot_all[:, 96 * h:96 * h + 96]
                    nc.vector.tensor_scalar_mul(ots, p_qs[:],
                                                lam_all[:, 8 * ci + h:8 * ci + h + 1])
                    nc.vector.tensor_add(ots, ots, p_oi[:])
                    kw = dp2.tile([128, 48], BF16, tag="kw", name="kw")
                    nc.vector.tensor_scalar_mul(
                        kw[:], ktok[:, 512 * ci + 128 * j + rh:512 * ci + 128 * j + rh + 48],
                        w_tok[:, 8 * ci + h:8 * ci + h + 1])
                    nc.tensor.matmul(p_s[rh:rh + 48, :], kw[:], u_[:], start=True, stop=True)

                s_new = spool.tile([128, DV], F32, tag=f"s{j}", name="s_new")
                for rh2 in (0, 64):
                    nc.vector.tensor_scalar_mul(
                        s_new[rh2:rh2 + 48, :], S_cur[j][rh2:rh2 + 48, :],
                        ebcJ[j][rh2:rh2 + 48, ci:ci + 1])
                    nc.vector.tensor_add(
                        s_new[rh2:rh2 + 48, :], s_new[rh2:rh2 + 48, :], p_s[rh2:rh2 + 48, :])
                S_cur[j] = s_new

            # ---- batched gated rmsnorm + gate + transpose to osb ----
            osq = dp2.tile([128, VD_C], F32, tag="osq", name="osq")
            nc.vector.tensor_mul(osq[:], ot_all[:], ot_all[:])
            rcol8 = dp2.tile([128, 8], F32, tag="rc8", name="rcol8")
            for h in range(HP):
                nc.vector.tensor_reduce(rcol8[:, h:h + 1], osq[:, 96 * h:96 * h + 96],
                                        mybir.AxisListType.X, OP.add)
            nc.scalar.activation(rcol8[:], rcol8[:], AF.Sqrt, bias=epsg[:], scale=1.0 / DV)
            nc.vector.reciprocal_approx_fast(rcol8[:], rcol8[:])
            for h in range(HP):
                nc.vector.tensor_scalar_mul(ot_all[:, 96 * h:96 * h + 96],
                                            ot_all[:, 96 * h:96 * h + 96], rcol8[:, h:h + 1])
            ob = dp2.tile([128, VD_C], BF16, tag="ob", name="ob")
            nc.vector.tensor_mul(ob[:], ot_all[:], g_tok[:, VD_C * ci:VD_C * ci + VD_C])
            for b6 in range(6):
                p_ot = pst(128, 128, BF16)
                nc.tensor.transpose(p_ot[:], ob[:, 128 * b6:128 * b6 + 128], idnb[:])
                nc.scalar.copy(osb[:, 1024 * b6 + 128 * ci:1024 * b6 + 128 * ci + 128], p_ot[:])

        for p in (spool, dp2, dpf, dpx, dpa, dput):
            _rel(p)
        if DUMP == "o":
            dump_y([(b, osb[:, 1024 * b:1024 * b + 1024]) for b in range(6)]
                   + [(6 + b, g_tok[:, 1024 * b:1024 * b + 1024]) for b in range(6)]
                   + [(12 + b, ktok[:, 1024 * b:1024 * b + 1024]) for b in range(4)])
        if DUMP == "dk":
            dump_y([(0, gta[:]), (1, beta_all[:]), (2, bcum_tok[:]),
                    (3, lam_all[:]), (4, w_tok[:]), (5, b_fm[:]),
                    (6, ebc_all[:]), (7, ebcJ[0][:]), (8, ebcJ[3][:])])
        _rel(dk)
        _rel(bigq)

        # ============ Phase D: o_projT halves + overlapped AllReduce ============
        if "D" not in PHASES:
            raise _SkipRest()
        wod = _pool(name="wod", bufs=14)
        pd = _pool(name="pd", bufs=4)
        for half in range(2):
            t0 = 512 * half
            for db in range(16):
                wts = []
                for fb in range(6):
                    wt = wod.tile([128, 128], BF16, tag="wo", name="wo_t")
                    nc.sync.dma_start(wt[:], wo_d[128 * fb:128 * fb + 128,
                                                  128 * db:128 * db + 128])
                    wts.append(wt)
                pp = pst()
                for fb in range(6):
                    nc.tensor.matmul(pp[:], wts[fb][:],
                                     osb[:, 1024 * fb + t0:1024 * fb + t0 + 512],
                                     start=(fb == 0), stop=(fb == 5))
                stg = pd.tile([128, 512], BF16, tag="s512b", name="stg")
                nc.scalar.copy(stg[:], pp[:])
                nc.sync.dma_start(oT_in[half][128 * db:128 * db + 128, :], stg[:])
            nc.gpsimd.collective_compute(
                "AllReduce", OP.add, ins=[oT_in[half][:]], outs=[oT_out[half][:]],
                replica_groups=groups)
        if DUMP in ("ar", "oin"):
            src = oT_out if DUMP == "ar" else oT_in
            dbg = _pool(name="dbg", bufs=4)
            for bi in range(16):
                st = dbg.tile([128, 1024], F32, tag="dbg", name="st")
                for half in range(2):
                    so = dbg.tile([128, 512], BF16, tag="dbg2", name="so")
                    nc.sync.dma_start(so[:], src[half][128 * bi:128 * bi + 128, :])
                    nc.vector.tensor_copy(st[:, 512 * half:512 * half + 512], so[:])
                nc.sync.dma_start(y_d[128 * bi:128 * bi + 128, :], st[:])
            _rel(dbg)
            raise _Dumped()

        # ============ Phase E ============
        if "E" not in PHASES:
            raise _SkipRest()
        seq = _pool(name="seq", bufs=3)
        ffT = hT
        p_s2 = [pst(1, 512) for _ in range(2)]
        for k in range(KT):
            xe = seq.tile([128, 1024], F32, tag="xe")
            nc.sync.dma_start(xe[:], xT_d[128 * k:128 * k + 128, :])
            oe = seq.tile([128, 1024], BF16, tag="oe")
            for half in range(2):
                nc.sync.dma_start(oe[:, 512 * half:512 * half + 512],
                                  oT_out[half][128 * k:128 * k + 128, :])
            h2 = seq.tile([128, 1024], F32, tag="h2T")
            nc.vector.tensor_add(h2[:], xe[:], oe[:])
            nc.sync.dma_start(h2T_scr[128 * k:128 * k + 128, :], h2[:])
            sqe = seq.tile([128, 1024], BF16, tag="sqe")
            nc.vector.tensor_mul(sqe[:], h2[:], h2[:])
            for n in range(2):
                nc.tensor.matmul(p_s2[n][:], ones1[:], sqe[:, 512 * n:512 * n + 512],
                                 start=(k == 0), stop=(k == KT - 1))
        srowE = seq.tile([1, 1024], F32, tag="srowE", bufs=1)
        for n in range(2):
            nc.scalar.activation(srowE[:, 512 * n:512 * n + 512], p_s2[n][:],
                                 AF.Sqrt, bias=eps1[0:1, :], scale=1.0 / D)
        nc.vector.reciprocal_approx_fast(srowE[:], srowE[:])
        sbcE = seq.tile([128, 1024], F32, tag="sbcE", bufs=1)
        nc.gpsimd.partition_broadcast(sbcE[:], srowE[:])
        for k in range(KT):
            h2r = seq.tile([128, 1024], F32, tag="h2r", bufs=4)
            nc.sync.dma_start(h2r[:], h2T_scr[128 * k:128 * k + 128, :])
            nc.vector.tensor_mul(ffT[:, 1024 * k:1024 * k + 1024], h2r[:], sbcE[:])
        if DUMP == "ffT":
            dump_y([(k, ffT[:, 1024 * k:1024 * k + 1024]) for k in range(KT)])

        mida = pg.tile([128, 6 * 1024], BF16, tag="gtok")
        pmid = _pool(name="pmid", bufs=1)
        midb = pmid.tile([128, 5 * 1024], BF16, tag="midb")

        def mid_ap(m, off, ln):
            if m < 6:
                return mida[:, 1024 * m + off:1024 * m + off + ln]
            return midb[:, 1024 * (m - 6) + off:1024 * (m - 6) + off + ln]

        for m in range(11):
            pu1 = [pst() for _ in range(2)]
            pu3 = [pst() for _ in range(2)]
            for k in range(KT):
                wt1 = wp.tile([128, 128], BF16, tag="w")
                nc.sync.dma_start(wt1[:], w1_d[128 * k:128 * k + 128, 128 * m:128 * m + 128])
                wt3 = wp.tile([128, 128], BF16, tag="w")
                nc.sync.dma_start(wt3[:], w3_d[128 * k:128 * k + 128, 128 * m:128 * m + 128])
                for n in range(2):
                    rhs = ffT[:, 1024 * k + 512 * n:1024 * k + 512 * n + 512]
                    nc.tensor.matmul(pu1[n][:], wt1[:], rhs, start=(k == 0), stop=(k == KT - 1))
                    nc.tensor.matmul(pu3[n][:], wt3[:], rhs, start=(k == 0), stop=(k == KT - 1))
            for n in range(2):
                u1s = pd.tile([128, 512], F32, tag="s512", name="u1s")
                nc.scalar.activation(u1s[:], pu1[n][:], AF.Silu)
                nc.vector.tensor_mul(mid_ap(m, 512 * n, 512), u1s[:], pu3[n][:])

        if DUMP == "mid":
            dump_y([(m, mid_ap(m, 0, 1024)) for m in range(11)])
        wp2 = _pool(name="wp2", bufs=12)
        for db in range(16):
            wts = []
            for m in range(11):
                wt = wp2.tile([128, 128], BF16, tag="w2", name="w2_t")
                nc.sync.dma_start(wt[:], w2_d[128 * m:128 * m + 128, 128 * db:128 * db + 128])
                wts.append(wt)
            for half in range(2):
                pps = pst()
                for m in range(11):
                    nc.tensor.matmul(pps[:], wts[m][:], mid_ap(m, 512 * half, 512),
                                     start=(m == 0), stop=(m == 10))
                h2t = pd.tile([128, 512], F32, tag="s512", name="h2t")
                nc.sync.dma_start(h2t[:], h2T_scr[128 * db:128 * db + 128,
                                                  512 * half:512 * half + 512])
                yst = pd.tile([128, 512], F32, tag="s512", name="yst")
                nc.vector.tensor_scalar_mul(yst[:], h2t[:], 0.25)
                nc.vector.tensor_add(yst[:], yst[:], pps[:])
                nc.sync.dma_start(y_d[128 * db:128 * db + 128, 512 * half:512 * half + 512],
                                  yst[:])

        for p in (wp2, pmid, seq, pd, wod, dram, wp, pg, big, ps, cpool):
            _rel(p)
      except _SkipRest:
        zst = _pool(name="zst", bufs=1)
        zt = zst.tile([128, 512], F32)
        nc.vector.memset(zt[:], 0.0)
        for i in range(16):
            for dh in range(2):
                nc.sync.dma_start(y_d[128 * i:128 * i + 128, 512 * dh:512 * dh + 512], zt[:])
        for p in reversed(live_pools):
            p.release()
      except _Dumped:
        for p in reversed(live_pools):
            p.release()

    nc.compile()
    return nc


def _shard(inputs):
    f32 = np.float32
    bf = ml_dtypes.bfloat16
    rms1 = np.asarray(inputs["rms1_w"], f32)
    rms2 = np.asarray(inputs["rms2_w"], f32)
    gn = np.asarray(inputs["gnorm_w"], f32)
    in_maps = []
    for c in range(8):
        g, m = c // 4, c % 4
        qs = slice(384 * m, 384 * m + 384)
        vs = slice(768 * m, 768 * m + 768)
        hs = slice(8 * m, 8 * m + 8)
        isl = slice(1408 * m, 1408 * m + 1408)

        def padqk(w):
            wp_ = np.zeros((D, QKP), f32)
            for h in range(8):
                wp_[:, 64 * h:64 * h + 48] = w[:, 48 * h:48 * h + 48]
            return wp_

        def padcw(w):
            cp = np.zeros((QKP, 4), f32)
            for h in range(8):
                cp[64 * h:64 * h + 48] = w[48 * h:48 * h + 48]
            return cp

        def padv(w):
            colpad = w.shape[0] == D
            out = np.zeros((D, VP) if colpad else (VP, w.shape[1]), f32)
            for h in range(8):
                if colpad:
                    out[:, 128 * h:128 * h + 96] = w[:, 96 * h:96 * h + 96]
                else:
                    out[128 * h:128 * h + 96] = w[96 * h:96 * h + 96]
            return out

        dtb8 = np.asarray(inputs["dt_bias"], f32)[hs]
        nega8 = -np.exp(np.asarray(inputs["A_log"], f32)[hs])
        in_maps.append(dict(
            xT=np.ascontiguousarray(np.asarray(inputs["x"], f32)[g].T),
            wq=padqk(np.asarray(inputs["Wq"], f32)[:, qs] * rms1[:, None]).astype(bf),
            wk=padqk(np.asarray(inputs["Wk"], f32)[:, qs] * rms1[:, None]).astype(bf),
            wv=padv(np.asarray(inputs["Wv"], f32)[:, vs] * rms1[:, None]).astype(bf),
            wg=np.ascontiguousarray(
                np.asarray(inputs["Wg"], f32)[:, vs] * rms1[:, None]).astype(bf),
            wab=np.ascontiguousarray(np.concatenate(
                [np.asarray(inputs["Wa"], f32)[:, hs],
                 np.asarray(inputs["Wb"], f32)[:, hs]], 1) * rms1[:, None]).astype(bf),
            cq=padcw(np.asarray(inputs["conv_q_w"], f32)[qs]),
            ck=padcw(np.asarray(inputs["conv_k_w"], f32)[qs]),
            cv=padv(np.asarray(inputs["conv_v_w"], f32)[vs]),
            dtb=np.tile(dtb8, 8).reshape(1, 64).copy(),
            nega=np.tile(nega8, 8).reshape(1, 64).copy(),
            wo=np.ascontiguousarray(
                np.asarray(inputs["Wo"], f32)[vs] * np.tile(gn, 8)[:, None]).astype(bf),
            w1=np.ascontiguousarray(
                np.asarray(inputs["W1"], f32)[:, isl] * rms2[:, None]).astype(bf),
            w3=np.ascontiguousarray(
                np.asarray(inputs["W3"], f32)[:, isl] * rms2[:, None]).astype(bf),
            w2=np.ascontiguousarray(np.asarray(inputs["W2"], f32)[isl]).astype(bf),
        ))
    return in_maps


def kernel(**inputs):
    if "nc" not in _cache:
        _cache["nc"] = _build(8)
    res = run_bass_kernel_spmd(_cache["nc"], _shard(inputs), list(range(8)))
    out = np.zeros((B, T, D), np.float32)
    for g in range(2):
        yT = sum(res.results[4 * g + m]["y"] for m in range(4))
        out[g] = yT.T
    return out


# revision 38
# speedup vs baseline: 1.2958x; 1.0210x over previous
"""GatedDeltaNet block kernel for 8 Trainium2 cores (Bass/Tile), bf16 rework.

Sharding: DP2 (batch) x TP4 (heads / MLP-inter). Core c: group g=c//4 runs
batch g; member m=c%4 owns heads [8m,8m+8), q/k cols [384m,..), v/g cols
[768m,..), INTER [1408m,..). Two half-token AllReduces per 4-core group after
o_proj (overlapped with o_proj compute); final down-proj partials summed on
the host.

Everything runs feature-major (host passes x transposed, takes y transposed)
so there are no PE transposes outside the delta-rule inner loop. All big
GEMM operands are bf16 (host-cast weights); psum accumulation, the delta-rule
state, decay/beta math and norms stay fp32.

Per-core dataflow:
  A: xT [D,T] -> rmsnorm via matmul-accumulated column sumsq -> hT bf16 (SBUF)
  B: bf16 projections off hT; q/k feature-major (heads padded to 64 rows)
     -> conv+silu+l2norm -> SBUF (+ token-major copies of k, v via PE
     transposes); gate token-major; a/b -> batched decay prep for all chunks
  C: chunked gated delta rule (C=128, UT transform via log-doubling inverse,
     bf16 matmuls / fp32 state); writes normed+gated o feature-major to SBUF
  D: o_projT in two token halves, each followed by its AllReduce (overlapped)
  E: h2T = xT + oT; rmsnorm -> ffT bf16 (reuses hT); MLP bf16; yT partials
"""
import sys
sys.path.insert(0, '/opt/trn_rl_repo')
import numpy as np
import ml_dtypes

import concourse.bass as bass
import concourse.bacc as bacc
import concourse.mybir as mybir
import concourse.tile as tile
from concourse.bass_isa import ReduceOp
from concourse.bass_utils import run_bass_kernel_spmd

F32 = mybir.dt.float32
BF16 = mybir.dt.bfloat16
AF = mybir.ActivationFunctionType
OP = mybir.AluOpType

B, T, D = 2, 1024, 2048
H, DK, DV = 32, 48, 96
HP = 8
QKP = 512
VD_C = 768
VP = 1024
INT_C = 1408
C = 128
NCHUNK = T // C
KT = D // 128
NTOK = T // 128

_cache = {}
import os
PHASES = os.environ.get("DN_PHASES", "ABCDE")
NCH = int(os.environ.get("DN_NCHUNK", str(T // C)))
DUMP = os.environ.get("DN_DUMP", "")


class _SkipRest(Exception):
    pass


class _Dumped(Exception):
    pass


def _build(n_cores=8):
    groups = [[0, 1, 2, 3], [4, 5, 6, 7]] if n_cores == 8 else [[0]]
    nc = bacc.Bacc("TRN2", target_bir_lowering=False, debug=False, num_devices=n_cores)

    xT_d = nc.dram_tensor("xT", [D, T], F32, kind="ExternalInput")
    wq_d = nc.dram_tensor("wq", [D, QKP], BF16, kind="ExternalInput")
    wk_d = nc.dram_tensor("wk", [D, QKP], BF16, kind="ExternalInput")
    wv_d = nc.dram_tensor("wv", [D, VP], BF16, kind="ExternalInput")
    wg_d = nc.dram_tensor("wg", [D, VD_C], BF16, kind="ExternalInput")
    wab_d = nc.dram_tensor("wab", [D, 16], BF16, kind="ExternalInput")
    cq_d = nc.dram_tensor("cq", [QKP, 4], F32, kind="ExternalInput")
    ck_d = nc.dram_tensor("ck", [QKP, 4], F32, kind="ExternalInput")
    cv_d = nc.dram_tensor("cv", [VP, 4], F32, kind="ExternalInput")
    dtb_d = nc.dram_tensor("dtb", [1, 64], F32, kind="ExternalInput")
    nega_d = nc.dram_tensor("nega", [1, 64], F32, kind="ExternalInput")
    wo_d = nc.dram_tensor("wo", [128, 16 * VD_C], BF16, kind="ExternalInput")
    w13_d = nc.dram_tensor("w13", [128, 11 * 4096], BF16, kind="ExternalInput")
    w2_d = nc.dram_tensor("w2", [128, 16 * INT_C], BF16, kind="ExternalInput")
    y_d = nc.dram_tensor("y", [D, T], F32, kind="ExternalOutput")

    ones = np.ones((128, 128), np.float32)
    idn_c = nc.inline_tensor(np.eye(128, dtype=np.float32), "idn_c")
    idnb_c = nc.inline_tensor(np.eye(128, dtype=ml_dtypes.bfloat16), "idnb_c")
    cum_c = nc.inline_tensor(np.triu(ones).copy(), "cum_c")
    mst_c = nc.inline_tensor(np.triu(ones, 1).astype(ml_dtypes.bfloat16), "mst_c")
    msi_c = nc.inline_tensor(np.triu(ones).copy(), "msi_c")
    negl_c = nc.inline_tensor((np.tril(ones, -1) * -1e30).copy(), "negl_c")
    # SELJ[r, 128j+p] = 1 iff (r%8==2j and p<48) or (r%8==2j+1 and 64<=p<112)
    selj_np = np.zeros((64, 512), np.float32)
    for r in range(64):
        for j in range(4):
            if r % 8 == 2 * j:
                selj_np[r, 128 * j:128 * j + 48] = 1.0
            if r % 8 == 2 * j + 1:
                selj_np[r, 128 * j + 64:128 * j + 112] = 1.0
    selj_c = nc.inline_tensor(selj_np, "selj_c")
    # CHK[8ci+h, ci] = 1
    chk_np = np.zeros((64, 8), np.float32)
    for ci in range(8):
        chk_np[8 * ci:8 * ci + 8, ci] = 1.0
    chk_c = nc.inline_tensor(chk_np, "chk_c")
    on48_np = np.zeros((128, 2), ml_dtypes.bfloat16)
    on48_np[0:48, 0] = 1.0
    on48_np[64:112, 1] = 1.0
    on48_c = nc.inline_tensor(on48_np, "on48_c")
    ones1_np = np.ones((128, 1), ml_dtypes.bfloat16)
    ones1_c = nc.inline_tensor(ones1_np, "ones1_c")

    with tile.TileContext(nc) as tc:
      live_pools = []

      def _pool(**kw):
          p = tc.alloc_tile_pool(**kw)
          live_pools.append(p)
          return p

      def _rel(p):
          p.release()
          live_pools.remove(p)

      try:
        cpool = _pool(name="consts", bufs=1)
        ps = _pool(name="ps", bufs=8, space="PSUM")

        def pst(p=128, f=512, dt=F32):
            return ps.tile([p, f], dt, tag="ps", name="pst")

        def dump_y(items):
            # items: list of (y_block_index, ap [p, <=1024]) — copy (cast) to y
            dbg = _pool(name="dbg", bufs=4)
            for bi, ap in items:
                p, n = ap.shape[0], ap.shape[1]
                st = dbg.tile([128, 1024], F32, tag="dbg", name="st")
                nc.vector.tensor_copy(st[0:p, 0:n], ap)
                nc.sync.dma_start(y_d[128 * bi:128 * bi + p, 0:n], st[0:p, 0:n])
            _rel(dbg)
            raise _Dumped()

        idn = cpool.tile([128, 128], F32)
        idnb = cpool.tile([128, 128], BF16)
        cum = cpool.tile([128, 128], F32)
        mstb = cpool.tile([128, 128], BF16)
        msi = cpool.tile([128, 128], F32)
        negl = cpool.tile([128, 128], F32)
        selj = cpool.tile([64, 512], F32)
        chk = cpool.tile([64, 8], F32)
        on48 = cpool.tile([128, 2], BF16)
        ones1 = cpool.tile([128, 1], BF16)
        for t_, s_ in [(idn, idn_c), (idnb, idnb_c), (cum, cum_c), (mstb, mst_c),
                       (msi, msi_c), (negl, negl_c), (selj, selj_c), (chk, chk_c),
                       (on48, on48_c), (ones1, ones1_c)]:
            nc.sync.dma_start(t_[:], s_[:])
        eps1 = cpool.tile([128, 1], F32)
        nc.vector.memset(eps1[:], 1e-5)
        epsq = cpool.tile([128, 1], F32)
        nc.vector.memset(epsq[:], 48e-6)
        epsk = cpool.tile([128, 1], F32)
        nc.vector.memset(epsk[:], 1e-6)
        epsg = cpool.tile([128, 1], F32)
        nc.vector.memset(epsg[:], 1e-5)
        dtb_r = cpool.tile([1, 64], F32)
        nega_r = cpool.tile([1, 64], F32)
        nc.sync.dma_start(dtb_r[:], dtb_d[:])
        nc.sync.dma_start(nega_r[:], nega_d[:])
        dtb_bc = cpool.tile([128, 64], F32)
        nega_bc = cpool.tile([128, 64], F32)
        nc.gpsimd.partition_broadcast(dtb_bc[:], dtb_r[:])
        nc.gpsimd.partition_broadcast(nega_bc[:], nega_r[:])
        cqw = cpool.tile([128, 16], F32)
        ckw = cpool.tile([128, 16], F32)
        cvw = cpool.tile([128, 32], F32)
        for j in range(4):
            nc.sync.dma_start(cqw[:, 4 * j:4 * j + 4], cq_d[128 * j:128 * j + 128, :])
            nc.sync.dma_start(ckw[:, 4 * j:4 * j + 4], ck_d[128 * j:128 * j + 128, :])
        for j in range(8):
            nc.sync.dma_start(cvw[:, 4 * j:4 * j + 4], cv_d[128 * j:128 * j + 128, :])
        ab_fm = cpool.tile([16, 1024], F32)

        big = _pool(name="big", bufs=1)
        hT = big.tile([128, KT * 1024], BF16)       # also ffT in phase E
        osb = big.tile([128, 6 * 1024], BF16)       # feature-major o: [feat%128, 1024*(f//128)+tok]
        pg = _pool(name="pg", bufs=1)
        g_tok = pg.tile([128, NTOK * VD_C], BF16, tag="gtok")

        wp = _pool(name="wp", bufs=4)
        dram = _pool(name="dram", bufs=1, space="DRAM")
        oT_in = [dram.tile([D, 512], BF16, name=f"oT_in{i}") for i in range(2)]
        oT_out = [dram.tile([D, 512], BF16, name=f"oT_out{i}") for i in range(2)]
        h2T_scr = dram.tile([D, T], F32)
        bfm_scr = dram.tile([64, 128], F32)

        bigq = _pool(name="bigq", bufs=1)
        qsb = bigq.tile([128, 4 * 1024], BF16)
        ksb = bigq.tile([128, 4 * 1024], BF16)
        ktok = bigq.tile([128, 8 * 512], BF16)      # token-major k: [tok, 512ci+128j]
        vtok = bigq.tile([128, 8 * VD_C], BF16)     # token-major v: [tok, 768ci+96h]
        nc.vector.memset(qsb[:], 0.0)
        nc.vector.memset(ksb[:], 0.0)

        # ============ Phase A: hT = rmsnorm(x)^T in bf16 ============
        stA = _pool(name="stA", bufs=16)
        sqp = _pool(name="sqp", bufs=3)
        p_ss = [pst(1, 512) for _ in range(2)]
        xts = []
        for k in range(KT):
            xa = stA.tile([128, 1024], F32, tag="xT")
            nc.sync.dma_start(xa[:], xT_d[128 * k:128 * k + 128, :])
            xts.append(xa)
            sq = sqp.tile([128, 1024], BF16, tag="sq")
            nc.vector.tensor_mul(sq[:], xa[:], xa[:])
            for n in range(2):
                nc.tensor.matmul(p_ss[n][:], ones1[:], sq[:, 512 * n:512 * n + 512],
                                 start=(k == 0), stop=(k == KT - 1))
        srowA = sqp.tile([1, 1024], F32, tag="srowA", bufs=1)
        for n in range(2):
            nc.scalar.activation(srowA[:, 512 * n:512 * n + 512], p_ss[n][:],
                                 AF.Sqrt, bias=eps1[0:1, :], scale=1.0 / D)
        nc.vector.reciprocal_approx_fast(srowA[:], srowA[:])
        sbcA = sqp.tile([128, 1024], F32, tag="sbcA", bufs=1)
        nc.gpsimd.partition_broadcast(sbcA[:], srowA[:])
        for k in range(KT):
            nc.vector.tensor_mul(hT[:, 1024 * k:1024 * k + 1024], xts[k][:], sbcA[:])
        _rel(sqp)
        _rel(stA)
        if DUMP == "hT":
            dump_y([(k, hT[:, 1024 * k:1024 * k + 1024]) for k in range(KT)])

        # ============ Phase B ============
        if "B" not in PHASES:
            raise _SkipRest()
        dk = _pool(name="dk", bufs=1)
        pb = _pool(name="pb", bufs=6)

        def conv_acc(pre, cw, j):
            acc = pb.tile([128, 1024], F32, tag="s1k")
            nc.scalar.activation(acc[:], pre[:], AF.Copy, scale=cw[:, 4 * j + 3:4 * j + 4])
            for s in (1, 2, 3):
                tmp = pb.tile([128, 1024], F32, tag="s1k")
                nc.scalar.activation(tmp[:, 0:1024 - s], pre[:, 0:1024 - s],
                                     AF.Copy, scale=cw[:, 4 * j + 3 - s:4 * j + 4 - s])
                nc.vector.tensor_add(acc[:, s:1024], acc[:, s:1024], tmp[:, 0:1024 - s])
            return acc

        def qkv_pass(w_dram, cw, eps_col, mult, kind, jbase, wcol0):
            pps = [[pst() for n in range(2)] for j in range(4)]
            for k in range(KT):
                wt = wp.tile([128, 512], BF16, tag="wwide")
                nc.sync.dma_start(wt[:], w_dram[128 * k:128 * k + 128, wcol0:wcol0 + 512])
                for j in range(4):
                    for n in range(2):
                        nc.tensor.matmul(
                            pps[j][n][:], wt[:, 128 * j:128 * j + 128],
                            hT[:, 1024 * k + 512 * n:1024 * k + 512 * n + 512],
                            start=(k == 0), stop=(k == KT - 1))
            for j in range(4):
                jj = jbase + j
                pre = pb.tile([128, 1024], F32, tag="s1k")
                for n in range(2):
                    nc.vector.tensor_copy(pre[:, 512 * n:512 * n + 512], pps[j][n][:])
                acc = conv_acc(pre, cw, jj)
                if kind == "v":
                    vb = pb.tile([128, 1024], BF16, tag="vb16", bufs=2)
                    nc.scalar.activation(vb[:], acc[:], AF.Silu)
                    for ci in range(8):
                        pv = pst(128, 96, BF16)
                        nc.tensor.transpose(pv[:], vb[0:96, 128 * ci:128 * ci + 128],
                                            idnb[0:96, 0:96])
                        nc.scalar.copy(
                            vtok[:, VD_C * ci + 96 * jj:VD_C * ci + 96 * jj + 96], pv[:])
                else:
                    blk = pb.tile([128, 1024], F32, tag="s1k")
                    nc.scalar.activation(blk[:], acc[:], AF.Silu)
                    sq = pb.tile([128, 1024], BF16, tag="sqb", bufs=2)
                    nc.vector.tensor_mul(sq[:], blk[:], blk[:])
                    dst = qsb if kind == "q" else ksb
                    for hh, rh in ((0, 0), (1, 64)):
                        srow = pb.tile([1, 1024], F32, tag="srow", bufs=2)
                        for n2 in range(2):
                            p_ssq = pst(1, 512)
                            nc.tensor.matmul(
                                p_ssq[:], on48[:, hh:hh + 1], sq[:, 512 * n2:512 * n2 + 512],
                                start=True, stop=True)
                            nc.scalar.activation(srow[:, 512 * n2:512 * n2 + 512], p_ssq[:],
                                                 AF.Sqrt, bias=eps_col[0:1, :], scale=mult)
                        nc.vector.reciprocal_approx_fast(srow[:], srow[:])
                        sbc = pb.tile([128, 1024], F32, tag="sbc", bufs=2)
                        nc.gpsimd.partition_broadcast(sbc[:], srow[:])
                        nc.vector.tensor_mul(dst[rh:rh + 48, 1024 * jj:1024 * jj + 1024],
                                             blk[rh:rh + 48, :], sbc[rh:rh + 48, :])
                    if kind == "k":
                        for ci in range(8):
                            pk = pst(128, 128, BF16)
                            nc.tensor.transpose(
                                pk[:], ksb[:, 1024 * jj + 128 * ci:1024 * jj + 128 * ci + 128],
                                idnb[:])
                            nc.scalar.copy(
                                ktok[:, 512 * ci + 128 * jj:512 * ci + 128 * jj + 128], pk[:])

        qkv_pass(wq_d, cqw, epsq, 48.0, "q", 0, 0)
        qkv_pass(wk_d, ckw, epsk, 1.0, "k", 0, 0)
        qkv_pass(wv_d, cvw, None, None, "v", 0, 0)
        qkv_pass(wv_d, cvw, None, None, "v", 4, 512)
        if DUMP == "qkv":
            dump_y([(j, qsb[:, 1024 * j:1024 * j + 1024]) for j in range(4)]
                   + [(4 + j, ksb[:, 1024 * j:1024 * j + 1024]) for j in range(4)]
                   + [(8 + b, vtok[:, 1024 * b:1024 * b + 1024]) for b in range(6)]
                   + [(14 + b, ktok[:, 1024 * b:1024 * b + 1024]) for b in range(2)])

        # gate token-major
        for n in range(2):
            pgs = [pst(128, 384) for _ in range(NTOK)]
            for k in range(KT):
                wt = wp.tile([128, 384], BF16, tag="wg384")
                nc.sync.dma_start(wt[:], wg_d[128 * k:128 * k + 128, 384 * n:384 * n + 384])
                for i in range(NTOK):
                    nc.tensor.matmul(
                        pgs[i][:], hT[:, 1024 * k + 128 * i:1024 * k + 128 * i + 128], wt[:],
                        start=(k == 0), stop=(k == KT - 1))
            for i in range(NTOK):
                nc.scalar.activation(
                    g_tok[:, VD_C * i + 384 * n:VD_C * i + 384 * n + 384], pgs[i][:], AF.Silu)

        # a/b projections, feature-major [16, 1024]
        ppab = [pst(16, 512) for _ in range(2)]
        for k in range(KT):
            wt = wp.tile([128, 16], BF16, tag="wab")
            nc.sync.dma_start(wt[:], wab_d[128 * k:128 * k + 128, :])
            for n in range(2):
                nc.tensor.matmul(ppab[n][:], wt[:], hT[:, 1024 * k + 512 * n:1024 * k + 512 * n + 512],
                                 start=(k == 0), stop=(k == KT - 1))
        for n in range(2):
            nc.vector.tensor_copy(ab_fm[:, 512 * n:512 * n + 512], ppab[n][:])

        # -------- batched decay prep for all chunks --------
        gta = dk.tile([128, 64], F32)
        bta = dk.tile([128, 64], F32)
        for ci in range(8):
            p_ab = pst(128, 16)
            nc.tensor.transpose(p_ab[:], ab_fm[:, 128 * ci:128 * ci + 128], idn[0:16, 0:16])
            nc.vector.tensor_copy(gta[:, 8 * ci:8 * ci + 8], p_ab[:, 0:8])
            nc.vector.tensor_copy(bta[:, 8 * ci:8 * ci + 8], p_ab[:, 8:16])
        nc.vector.tensor_add(gta[:], gta[:], dtb_bc[:])
        nc.scalar.activation(gta[:], gta[:], AF.Exp)
        nc.vector.tensor_scalar_add(gta[:], gta[:], 1.0)
        nc.scalar.activation(gta[:], gta[:], AF.Ln)
        nc.vector.tensor_mul(gta[:], gta[:], nega_bc[:])        # gt_all [128,64]
        beta_all = dk.tile([128, 64], F32)
        nc.scalar.activation(beta_all[:], bta[:], AF.Sigmoid)
        nbeta_all = dk.tile([128, 64], F32)
        nc.vector.tensor_scalar_mul(nbeta_all[:], beta_all[:], -1.0)
        p_bc = pst(128, 64)
        nc.tensor.matmul(p_bc[:], cum[:], gta[:], start=True, stop=True)
        bcum_tok = dk.tile([128, 64], F32)
        nc.vector.tensor_copy(bcum_tok[:], p_bc[:])
        lam_all = dk.tile([128, 64], F32)
        nc.scalar.activation(lam_all[:], p_bc[:], AF.Exp)
        p_bf = pst(64, 128)
        nc.tensor.transpose(p_bf[:], bcum_tok[:], idn[:])
        b_fm = dk.tile([64, 128], F32)
        nc.vector.tensor_copy(b_fm[:], p_bf[:])
        nc.scalar.dma_start(bfm_scr[:], b_fm[:])
        wfm = dk.tile([64, 128], F32)
        nc.vector.tensor_scalar(wfm[:], b_fm[:], b_fm[:, 127:128], None, OP.subtract)
        nc.scalar.activation(wfm[:], wfm[:], AF.Exp, scale=-1.0)
        p_wt = pst(128, 64)
        nc.tensor.transpose(p_wt[:], wfm[:], idn[0:64, 0:64])
        w_tok = dk.tile([128, 64], F32)
        nc.vector.tensor_copy(w_tok[:], p_wt[:])
        ebc_all = dk.tile([64, 1], F32)
        nc.scalar.activation(ebc_all[:], b_fm[:, 127:128], AF.Exp)
        # EB[8ci+h, ci] = ebc_all[8ci+h]; ebcJ[j][p, ci] = per-(ci,j) state-decay col
        EB = dk.tile([64, 8], F32)
        nc.vector.tensor_scalar_mul(EB[:], chk[:], ebc_all[:, 0:1])
        ebcJ = []
        for j in range(4):
            p_ebj = pst(128, 8)
            nc.tensor.matmul(p_ebj[:], selj[:, 128 * j:128 * j + 128], EB[:],
                             start=True, stop=True)
            ej = dk.tile([128, 8], F32, tag=f"ebj{j}", name=f"ebj{j}")
            nc.vector.tensor_copy(ej[:], p_ebj[:])
            ebcJ.append(ej)
        _rel(pb)

        # ============ Phase C ============
        if "C" not in PHASES:
            raise _SkipRest()
        dput = _pool(name="dput", bufs=16)
        dpa = _pool(name="dpa", bufs=10)
        dpx = _pool(name="dpx", bufs=10)
        dpf = _pool(name="dpf", bufs=6)
        dp2 = _pool(name="dp2", bufs=2)
        spool = _pool(name="spool", bufs=2)

        S_cur = {}
        for j in range(4):
            S_cur[j] = spool.tile([128, DV], F32, tag=f"s{j}", name=f"s{j}")
            nc.vector.memset(S_cur[j][:], 0.0)

        for ci in range(NCH):
            cs = slice(128 * ci, 128 * ci + 128)
            # ---- prep all 8 heads: abar, xx, xt ----
            ABAR, XX, XT = {}, {}, {}
            for j in range(4):
                for hh in range(2):
                    h = 2 * j + hh
                    rh = 64 * hh
                    kts = ksb[rh:rh + 48, 1024 * j + 128 * ci:1024 * j + 128 * ci + 128]
                    qts = qsb[rh:rh + 48, 1024 * j + 128 * ci:1024 * j + 128 * ci + 128]
                    p_kk = pst(128, 128)
                    nc.tensor.matmul(p_kk[:], kts, kts, start=True, stop=True)
                    p_kq = pst(128, 128)
                    nc.tensor.matmul(p_kq[:], kts, qts, start=True, stop=True)
                    bc128 = dpf.tile([128, 128], F32, tag="bc", name="bc128")
                    nc.gpsimd.dma_start(
                        bc128[:],
                        bfm_scr[8 * ci + h:8 * ci + h + 1, :].to_broadcast((128, 128)))
                    dte = dpf.tile([128, 128], F32, tag="dte", name="dte")
                    nc.vector.tensor_scalar(dte[:], bc128[:],
                                            bcum_tok[:, 8 * ci + h:8 * ci + h + 1],
                                            None, OP.subtract)
                    nc.vector.tensor_mul(dte[:], dte[:], msi[:])
                    nc.vector.tensor_add(dte[:], dte[:], negl[:])
                    dincl = dput.tile([128, 128], BF16, tag="ut", name="dincl")
                    nc.scalar.activation(dincl[:], dte[:], AF.Exp)
                    abar = dpa.tile([128, 128], BF16, tag="abar", name="abar")
                    nc.vector.tensor_mul(abar[:], p_kq[:], dincl[:])
                    dstr = dput.tile([128, 128], BF16, tag="ut", name="dstr")
                    nc.vector.tensor_mul(dstr[:], dincl[:], mstb[:])
                    x0 = dput.tile([128, 128], BF16, tag="ut", name="x0")
                    nc.vector.tensor_mul(x0[:], p_kk[:], dstr[:])
                    xx = dpx.tile([128, 128], BF16, tag="xx", name="xx")
                    nc.vector.tensor_scalar_mul(xx[:], x0[:],
                                                nbeta_all[:, 8 * ci + h:8 * ci + h + 1])
                    p_x = pst(128, 128, BF16)
                    nc.tensor.transpose(p_x[:], xx[:], idnb[:])
                    xt = dpx.tile([128, 128], BF16, tag="xt", name="xt")
                    nc.vector.tensor_copy(xt[:], p_x[:])
                    ABAR[h], XX[h], XT[h] = abar, xx, xt

            ot_all = dp2.tile([128, VD_C], F32, tag="otall", name="ot_all")
            # ---- per-head UT inverse + state/output ----
            for j in range(4):
                S_bf = dp2.tile([128, DV], BF16, tag=f"sbf{j}", name="S_bf")
                nc.vector.tensor_copy(S_bf[:], S_cur[j][:])
                p_s = pst(128, DV)
                for hh in range(2):
                    h = 2 * j + hh
                    rh = 64 * hh
                    kts = ksb[rh:rh + 48, 1024 * j + 128 * ci:1024 * j + 128 * ci + 128]
                    qts = qsb[rh:rh + 48, 1024 * j + 128 * ci:1024 * j + 128 * ci + 128]
                    xx, xt = XX[h], XT[h]
                    pmat = dput.tile([128, 128], BF16, tag="ut", name="pmat")
                    nc.vector.tensor_add(pmat[:], xx[:], idnb[:])
                    for lvl in range(6):
                        last = lvl == 5
                        if not last:
                            p_sq = pst(128, 128)
                            nc.tensor.matmul(p_sq[:], xt[:], xx[:], start=True, stop=True)
                            x2 = dput.tile([128, 128], BF16, tag="ut", name="x2")
                            nc.scalar.copy(x2[:], p_sq[:])
                        p_sqt = pst(128, 128)
                        nc.tensor.matmul(p_sqt[:], xx[:], xt[:], start=True, stop=True)
                        xt2 = dput.tile([128, 128], BF16, tag="ut", name="xt2")
                        nc.scalar.copy(xt2[:], p_sqt[:])
                        p_pr = pst(128, 128)
                        nc.tensor.matmul(p_pr[:], xt2[:], pmat[:], start=True, stop=True)
                        pnew = dput.tile([128, 128], BF16, tag="ut", name="pnew")
                        nc.vector.tensor_add(pnew[:], pmat[:], p_pr[:])
                        pmat = pnew
                        if not last:
                            xx, xt = x2, xt2

                    p_ks = pst(128, DV)
                    nc.tensor.matmul(p_ks[:], kts, S_bf[rh:rh + 48, :], start=True, stop=True)
                    r_ = dp2.tile([128, DV], BF16, tag="rr", name="r_")
                    nc.vector.tensor_scalar_mul(r_[:], p_ks[:],
                                                lam_all[:, 8 * ci + h:8 * ci + h + 1])
                    nc.vector.tensor_sub(r_[:], vtok[:, VD_C * ci + 96 * h:VD_C * ci + 96 * h + 96],
                                         r_[:])
                    p_w = pst(128, DV)
                    nc.tensor.matmul(p_w[:], pmat[:], r_[:], start=True, stop=True)
                    u_ = dp2.tile([128, DV], BF16, tag="uu", name="u_")
                    nc.vector.tensor_scalar_mul(u_[:], p_w[:],
                                                beta_all[:, 8 * ci + h:8 * ci + h + 1])
                    p_oi = pst(128, DV)
                    nc.tensor.matmul(p_oi[:], ABAR[h][:], u_[:], start=True, stop=True)
                    p_qs = pst(128, DV)
                    nc.tensor.matmul(p_qs[:], qts, S_bf[rh:rh + 48, :], start=True, stop=True)
                    ots = ot_all[:, 96 * h:96 * h + 96]
                    nc.vector.tensor_scalar_mul(ots, p_qs[:],
                                                lam_all[:, 8 * ci + h:8 * ci + h + 1])
                    nc.vector.tensor_add(ots, ots, p_oi[:])
                    kw = dp2.tile([128, 48], BF16, tag="kw", name="kw")
                    nc.vector.tensor_scalar_mul(
                        kw[:], ktok[:, 512 * ci + 128 * j + rh:512 * ci + 128 * j + rh + 48],
                        w_tok[:, 8 * ci + h:8 * ci + h + 1])
                    nc.tensor.matmul(p_s[rh:rh + 48, :], kw[:], u_[:], start=True, stop=True)

                s_new = spool.tile([128, DV], F32, tag=f"s{j}", name="s_new")
                for rh2 in (0, 64):
                    nc.vector.tensor_scalar_mul(
                        s_new[rh2:rh2 + 48, :], S_cur[j][rh2:rh2 + 48, :],
                        ebcJ[j][rh2:rh2 + 48, ci:ci + 1])
                    nc.vector.tensor_add(
                        s_new[rh2:rh2 + 48, :], s_new[rh2:rh2 + 48, :], p_s[rh2:rh2 + 48, :])
                S_cur[j] = s_new

            # ---- batched gated rmsnorm + gate + transpose to osb ----
            osq = dp2.tile([128, VD_C], F32, tag="osq", name="osq")
            nc.vector.tensor_mul(osq[:], ot_all[:], ot_all[:])
            rcol8 = dp2.tile([128, 8], F32, tag="rc8", name="rcol8")
            for h in range(HP):
                nc.vector.tensor_reduce(rcol8[:, h:h + 1], osq[:, 96 * h:96 * h + 96],
                                        mybir.AxisListType.X, OP.add)
            nc.scalar.activation(rcol8[:], rcol8[:], AF.Sqrt, bias=epsg[:], scale=1.0 / DV)
            nc.vector.reciprocal_approx_fast(rcol8[:], rcol8[:])
            for h in range(HP):
                nc.vector.tensor_scalar_mul(ot_all[:, 96 * h:96 * h + 96],
                                            ot_all[:, 96 * h:96 * h + 96], rcol8[:, h:h + 1])
            ob = dp2.tile([128, VD_C], BF16, tag="ob", name="ob")
            nc.vector.tensor_mul(ob[:], ot_all[:], g_tok[:, VD_C * ci:VD_C * ci + VD_C])
            for b6 in range(6):
                p_ot = pst(128, 128, BF16)
                nc.tensor.transpose(p_ot[:], ob[:, 128 * b6:128 * b6 + 128], idnb[:])
                nc.scalar.copy(osb[:, 1024 * b6 + 128 * ci:1024 * b6 + 128 * ci + 128], p_ot[:])

        for p in (spool, dp2, dpf, dpx, dpa, dput):
            _rel(p)
        if DUMP == "o":
            dump_y([(b, osb[:, 1024 * b:1024 * b + 1024]) for b in range(6)]
                   + [(6 + b, g_tok[:, 1024 * b:1024 * b + 1024]) for b in range(6)]
                   + [(12 + b, ktok[:, 1024 * b:1024 * b + 1024]) for b in range(4)])
        if DUMP == "dk":
            dump_y([(0, gta[:]), (1, beta_all[:]), (2, bcum_tok[:]),
                    (3, lam_all[:]), (4, w_tok[:]), (5, b_fm[:]),
                    (6, ebc_all[:]), (7, ebcJ[0][:]), (8, ebcJ[3][:])])
        _rel(dk)
        _rel(bigq)

        # ============ Phase D: o_projT halves + overlapped AllReduce ============
        if "D" not in PHASES:
            raise _SkipRest()
        wod = _pool(name="wod", bufs=14)
        pd = _pool(name="pd", bufs=4)
        for half in range(2):
            t0 = 512 * half
            for db in range(16):
                wt = wod.tile([128, VD_C], BF16, tag="wo", name="wo_t", bufs=4)
                nc.scalar.dma_start(wt[:], wo_d[:, VD_C * db:VD_C * db + VD_C])
                pp = pst()
                for fb in range(6):
                    nc.tensor.matmul(pp[:], wt[:, 128 * fb:128 * fb + 128],
                                     osb[:, 1024 * fb + t0:1024 * fb + t0 + 512],
                                     start=(fb == 0), stop=(fb == 5))
                stg = pd.tile([128, 512], BF16, tag="s512b", name="stg")
                nc.scalar.copy(stg[:], pp[:])
                nc.sync.dma_start(oT_in[half][128 * db:128 * db + 128, :], stg[:])
            nc.gpsimd.collective_compute(
                "AllReduce", OP.add, ins=[oT_in[half][:]], outs=[oT_out[half][:]],
                replica_groups=groups)
        if DUMP in ("ar", "oin"):
            src = oT_out if DUMP == "ar" else oT_in
            dbg = _pool(name="dbg", bufs=4)
            for bi in range(16):
                st = dbg.tile([128, 1024], F32, tag="dbg", name="st")
                for half in range(2):
                    so = dbg.tile([128, 512], BF16, tag="dbg2", name="so")
                    nc.sync.dma_start(so[:], src[half][128 * bi:128 * bi + 128, :])
                    nc.vector.tensor_copy(st[:, 512 * half:512 * half + 512], so[:])
                nc.sync.dma_start(y_d[128 * bi:128 * bi + 128, :], st[:])
            _rel(dbg)
            raise _Dumped()

        # ============ Phase E ============
        if "E" not in PHASES:
            raise _SkipRest()
        seq = _pool(name="seq", bufs=3)
        ffT = hT
        p_s2 = [pst(1, 512) for _ in range(2)]
        for half in range(2):
            hs = slice(512 * half, 512 * half + 512)
            for k in range(KT):
                xe = seq.tile([128, 512], F32, tag="xe")
                nc.sync.dma_start(xe[:], xT_d[128 * k:128 * k + 128, hs])
                oe = seq.tile([128, 512], BF16, tag="oe")
                nc.gpsimd.dma_start(oe[:], oT_out[half][128 * k:128 * k + 128, :])
                h2 = seq.tile([128, 512], F32, tag="h2T")
                nc.vector.tensor_add(h2[:], xe[:], oe[:])
                nc.scalar.dma_start(h2T_scr[128 * k:128 * k + 128, hs], h2[:])
                sqe = seq.tile([128, 512], BF16, tag="sqe")
                nc.vector.tensor_mul(sqe[:], h2[:], h2[:])
                nc.tensor.matmul(p_s2[half][:], ones1[:], sqe[:],
                                 start=(k == 0), stop=(k == KT - 1))
        srowE = seq.tile([1, 1024], F32, tag="srowE", bufs=1)
        for n in range(2):
            nc.scalar.activation(srowE[:, 512 * n:512 * n + 512], p_s2[n][:],
                                 AF.Sqrt, bias=eps1[0:1, :], scale=1.0 / D)
        nc.vector.reciprocal_approx_fast(srowE[:], srowE[:])
        sbcE = seq.tile([128, 1024], F32, tag="sbcE", bufs=1)
        nc.gpsimd.partition_broadcast(sbcE[:], srowE[:])
        for k in range(KT):
            h2r = seq.tile([128, 1024], F32, tag="h2r", bufs=4)
            nc.sync.dma_start(h2r[:], h2T_scr[128 * k:128 * k + 128, :])
            nc.vector.tensor_mul(ffT[:, 1024 * k:1024 * k + 1024], h2r[:], sbcE[:])
        if DUMP == "ffT":
            dump_y([(k, ffT[:, 1024 * k:1024 * k + 1024]) for k in range(KT)])

        mida = pg.tile([128, 6 * 1024], BF16, tag="gtok")
        pmid = _pool(name="pmid", bufs=1)
        midb = pmid.tile([128, 5 * 1024], BF16, tag="midb")

        def mid_ap(m, off, ln):
            if m < 6:
                return mida[:, 1024 * m + off:1024 * m + off + ln]
            return midb[:, 1024 * (m - 6) + off:1024 * (m - 6) + off + ln]

        wp13 = _pool(name="wp13", bufs=3)
        for m in range(11):
            pu1 = [pst() for _ in range(2)]
            pu3 = [pst() for _ in range(2)]
            wt13 = wp13.tile([128, 4096], BF16, tag="w13", name="wt13")
            nc.sync.dma_start(wt13[:], w13_d[:, 4096 * m:4096 * m + 4096])
            for k in range(KT):
                for n in range(2):
                    rhs = ffT[:, 1024 * k + 512 * n:1024 * k + 512 * n + 512]
                    nc.tensor.matmul(pu1[n][:], wt13[:, 256 * k:256 * k + 128], rhs,
                                     start=(k == 0), stop=(k == KT - 1))
                    nc.tensor.matmul(pu3[n][:], wt13[:, 256 * k + 128:256 * k + 256], rhs,
                                     start=(k == 0), stop=(k == KT - 1))
            for n in range(2):
                u1s = pd.tile([128, 512], F32, tag="s512", name="u1s")
                nc.scalar.activation(u1s[:], pu1[n][:], AF.Silu)
                nc.vector.tensor_mul(mid_ap(m, 512 * n, 512), u1s[:], pu3[n][:])

        if DUMP == "mid":
            dump_y([(m, mid_ap(m, 0, 1024)) for m in range(11)])
        wp2 = _pool(name="wp2", bufs=3)
        for db in range(16):
            wt2 = wp2.tile([128, INT_C], BF16, tag="w2", name="w2_t")
            nc.sync.dma_start(wt2[:], w2_d[:, INT_C * db:INT_C * db + INT_C])
            for half in range(2):
                pps = pst()
                for m in range(11):
                    nc.tensor.matmul(pps[:], wt2[:, 128 * m:128 * m + 128],
                                     mid_ap(m, 512 * half, 512),
                                     start=(m == 0), stop=(m == 10))
                h2t = pd.tile([128, 512], F32, tag="s512", name="h2t")
                nc.sync.dma_start(h2t[:], h2T_scr[128 * db:128 * db + 128,
                                                  512 * half:512 * half + 512])
                yst = pd.tile([128, 512], F32, tag="s512", name="yst")
                nc.vector.tensor_scalar_mul(yst[:], h2t[:], 0.25)
                nc.vector.tensor_add(yst[:], yst[:], pps[:])
                nc.sync.dma_start(y_d[128 * db:128 * db + 128, 512 * half:512 * half + 512],
                                  yst[:])

        for p in (wp2, wp13, pmid, seq, pd, wod, dram, wp, pg, big, ps, cpool):
            _rel(p)
      except _SkipRest:
        zst = _pool(name="zst", bufs=1)
        zt = zst.tile([128, 512], F32)
        nc.vector.memset(zt[:], 0.0)
        for i in range(16):
            for dh in range(2):
                nc.sync.dma_start(y_d[128 * i:128 * i + 128, 512 * dh:512 * dh + 512], zt[:])
        for p in reversed(live_pools):
            p.release()
      except _Dumped:
        for p in reversed(live_pools):
            p.release()

    nc.compile()
    return nc


def _pack_wo(wo):
    # [768, 2048] -> [128, 16*768]: col = 768*db + 128*fb + c
    return np.ascontiguousarray(
        wo.reshape(6, 128, 16, 128).transpose(1, 2, 0, 3).reshape(128, 16 * 768))


def _pack_w13(w1, w3):
    # [2048, 1408] x2 -> [128, 11*4096]: col = 4096*m + 256*k + 128*which + c
    a = w1.reshape(16, 128, 11, 128).transpose(1, 2, 0, 3)   # [128, 11, 16, 128]
    b = w3.reshape(16, 128, 11, 128).transpose(1, 2, 0, 3)
    return np.ascontiguousarray(
        np.stack([a, b], axis=3).reshape(128, 11 * 4096))


def _pack_w2(w2):
    # [1408, 2048] -> [128, 16*1408]: col = 1408*db + 128*m + c
    return np.ascontiguousarray(
        w2.reshape(11, 128, 16, 128).transpose(1, 2, 0, 3).reshape(128, 16 * 1408))


def _shard(inputs):
    f32 = np.float32
    bf = ml_dtypes.bfloat16
    rms1 = np.asarray(inputs["rms1_w"], f32)
    rms2 = np.asarray(inputs["rms2_w"], f32)
    gn = np.asarray(inputs["gnorm_w"], f32)
    in_maps = []
    for c in range(8):
        g, m = c // 4, c % 4
        qs = slice(384 * m, 384 * m + 384)
        vs = slice(768 * m, 768 * m + 768)
        hs = slice(8 * m, 8 * m + 8)
        isl = slice(1408 * m, 1408 * m + 1408)

        def padqk(w):
            wp_ = np.zeros((D, QKP), f32)
            for h in range(8):
                wp_[:, 64 * h:64 * h + 48] = w[:, 48 * h:48 * h + 48]
            return wp_

        def padcw(w):
            cp = np.zeros((QKP, 4), f32)
            for h in range(8):
                cp[64 * h:64 * h + 48] = w[48 * h:48 * h + 48]
            return cp

        def padv(w):
            colpad = w.shape[0] == D
            out = np.zeros((D, VP) if colpad else (VP, w.shape[1]), f32)
            for h in range(8):
                if colpad:
                    out[:, 128 * h:128 * h + 96] = w[:, 96 * h:96 * h + 96]
                else:
                    out[128 * h:128 * h + 96] = w[96 * h:96 * h + 96]
            return out

        dtb8 = np.asarray(inputs["dt_bias"], f32)[hs]
        nega8 = -np.exp(np.asarray(inputs["A_log"], f32)[hs])
        in_maps.append(dict(
            xT=np.ascontiguousarray(np.asarray(inputs["x"], f32)[g].T),
            wq=padqk(np.asarray(inputs["Wq"], f32)[:, qs] * rms1[:, None]).astype(bf),
            wk=padqk(np.asarray(inputs["Wk"], f32)[:, qs] * rms1[:, None]).astype(bf),
            wv=padv(np.asarray(inputs["Wv"], f32)[:, vs] * rms1[:, None]).astype(bf),
            wg=np.ascontiguousarray(
                np.asarray(inputs["Wg"], f32)[:, vs] * rms1[:, None]).astype(bf),
            wab=np.ascontiguousarray(np.concatenate(
                [np.asarray(inputs["Wa"], f32)[:, hs],
                 np.asarray(inputs["Wb"], f32)[:, hs]], 1) * rms1[:, None]).astype(bf),
            cq=padcw(np.asarray(inputs["conv_q_w"], f32)[qs]),
            ck=padcw(np.asarray(inputs["conv_k_w"], f32)[qs]),
            cv=padv(np.asarray(inputs["conv_v_w"], f32)[vs]),
            dtb=np.tile(dtb8, 8).reshape(1, 64).copy(),
            nega=np.tile(nega8, 8).reshape(1, 64).copy(),
            wo=_pack_wo(np.asarray(inputs["Wo"], f32)[vs] * np.tile(gn, 8)[:, None]).astype(bf),
            w13=_pack_w13(np.asarray(inputs["W1"], f32)[:, isl] * rms2[:, None],
                          np.asarray(inputs["W3"], f32)[:, isl] * rms2[:, None]).astype(bf),
            w2=_pack_w2(np.asarray(inputs["W2"], f32)[isl]).astype(bf),
        ))
    return in_maps


def kernel(**inputs):
    if "nc" not in _cache:
        _cache["nc"] = _build(8)
    res = run_bass_kernel_spmd(_cache["nc"], _shard(inputs), list(range(8)))
    out = np.zeros((B, T, D), np.float32)
    for g in range(2):
        yT = sum(res.results[4 * g + m]["y"] for m in range(4))
        out[g] = yT.T
    return out


# revision 42
# speedup vs baseline: 1.7557x; 1.3549x over previous
"""GatedDeltaNet block kernel for 8 Trainium2 cores (Bass/Tile), bf16 rework.

Sharding: DP2 (batch) x TP4 (heads / MLP-inter). Core c: group g=c//4 runs
batch g; member m=c%4 owns heads [8m,8m+8), q/k cols [384m,..), v/g cols
[768m,..), INTER [1408m,..). Two half-token AllReduces per 4-core group after
o_proj (overlapped with o_proj compute); final down-proj partials summed on
the host.

Everything runs feature-major (host passes x transposed, takes y transposed)
so there are no PE transposes outside the delta-rule inner loop. All big
GEMM operands are bf16 (host-cast weights); psum accumulation, the delta-rule
state, decay/beta math and norms stay fp32.

Per-core dataflow:
  A: xT [D,T] -> rmsnorm via matmul-accumulated column sumsq -> hT bf16 (SBUF)
  B: bf16 projections off hT; q/k feature-major (heads padded to 64 rows)
     -> conv+silu+l2norm -> SBUF (+ token-major copies of k, v via PE
     transposes); gate token-major; a/b -> batched decay prep for all chunks
  C: chunked gated delta rule (C=128, UT transform via log-doubling inverse,
     bf16 matmuls / fp32 state); writes normed+gated o feature-major to SBUF
  D: o_projT in two token halves, each followed by its AllReduce (overlapped)
  E: h2T = xT + oT; rmsnorm -> ffT bf16 (reuses hT); MLP bf16; yT partials
"""
import sys
sys.path.insert(0, '/opt/trn_rl_repo')
import numpy as np
import ml_dtypes

import concourse.bass as bass
import concourse.bacc as bacc
import concourse.mybir as mybir
import concourse.tile as tile
from concourse.bass_isa import ReduceOp
from concourse.bass_utils import run_bass_kernel_spmd

F32 = mybir.dt.float32
BF16 = mybir.dt.bfloat16
AF = mybir.ActivationFunctionType
OP = mybir.AluOpType

B, T, D = 2, 1024, 2048
H, DK, DV = 32, 48, 96
HP = 8
QKP = 512
VD_C = 768
VP = 1024
INT_C = 1408
C = 128
NCHUNK = T // C
KT = D // 128
NTOK = T // 128

_cache = {}
import os
PHASES = os.environ.get("DN_PHASES", "ABCDE")
NCH = int(os.environ.get("DN_NCHUNK", str(T // C)))
DUMP = os.environ.get("DN_DUMP", "")


class _SkipRest(Exception):
    pass


class _Dumped(Exception):
    pass


def _build(n_cores=8):
    groups = [[0, 1, 2, 3], [4, 5, 6, 7]] if n_cores == 8 else [[0]]
    nc = bacc.Bacc("TRN2", target_bir_lowering=False, debug=False, num_devices=n_cores)

    xT_d = nc.dram_tensor("xT", [D, T], F32, kind="ExternalInput")
    wq_d = nc.dram_tensor("wq", [D, QKP], BF16, kind="ExternalInput")
    wk_d = nc.dram_tensor("wk", [D, QKP], BF16, kind="ExternalInput")
    wv_d = nc.dram_tensor("wv", [D, VP], BF16, kind="ExternalInput")
    wg_d = nc.dram_tensor("wg", [D, VD_C], BF16, kind="ExternalInput")
    wab_d = nc.dram_tensor("wab", [D, 16], BF16, kind="ExternalInput")
    cq_d = nc.dram_tensor("cq", [QKP, 4], F32, kind="ExternalInput")
    ck_d = nc.dram_tensor("ck", [QKP, 4], F32, kind="ExternalInput")
    cv_d = nc.dram_tensor("cv", [VP, 4], F32, kind="ExternalInput")
    dtb_d = nc.dram_tensor("dtb", [1, 64], F32, kind="ExternalInput")
    nega_d = nc.dram_tensor("nega", [1, 64], F32, kind="ExternalInput")
    wo_d = nc.dram_tensor("wo", [128, 16 * VD_C], BF16, kind="ExternalInput")
    w13_d = nc.dram_tensor("w13", [128, 11 * 4096], BF16, kind="ExternalInput")
    w2_d = nc.dram_tensor("w2", [128, 16 * INT_C], BF16, kind="ExternalInput")
    y_d = nc.dram_tensor("y", [D, T], F32, kind="ExternalOutput")

    ones = np.ones((128, 128), np.float32)
    idn_c = nc.inline_tensor(np.eye(128, dtype=np.float32), "idn_c")
    idnb_c = nc.inline_tensor(np.eye(128, dtype=ml_dtypes.bfloat16), "idnb_c")
    cum_c = nc.inline_tensor(np.triu(ones).copy(), "cum_c")
    mst_c = nc.inline_tensor(np.triu(ones, 1).astype(ml_dtypes.bfloat16), "mst_c")
    msi_c = nc.inline_tensor(np.triu(ones).copy(), "msi_c")
    negl_c = nc.inline_tensor((np.tril(ones, -1) * -1e30).copy(), "negl_c")
    # SELJ[r, 128j+p] = 1 iff (r%8==2j and p<48) or (r%8==2j+1 and 64<=p<112)
    selj_np = np.zeros((64, 512), np.float32)
    for r in range(64):
        for j in range(4):
            if r % 8 == 2 * j:
                selj_np[r, 128 * j:128 * j + 48] = 1.0
            if r % 8 == 2 * j + 1:
                selj_np[r, 128 * j + 64:128 * j + 112] = 1.0
    selj_c = nc.inline_tensor(selj_np, "selj_c")
    # CHK[8ci+h, ci] = 1
    chk_np = np.zeros((64, 8), np.float32)
    for ci in range(8):
        chk_np[8 * ci:8 * ci + 8, ci] = 1.0
    chk_c = nc.inline_tensor(chk_np, "chk_c")
    on48_np = np.zeros((128, 2), ml_dtypes.bfloat16)
    on48_np[0:48, 0] = 1.0
    on48_np[64:112, 1] = 1.0
    on48_c = nc.inline_tensor(on48_np, "on48_c")
    ones1_np = np.ones((128, 1), ml_dtypes.bfloat16)
    ones1_c = nc.inline_tensor(ones1_np, "ones1_c")

    with tile.TileContext(nc) as tc:
      live_pools = []

      def _pool(**kw):
          p = tc.alloc_tile_pool(**kw)
          live_pools.append(p)
          return p

      def _rel(p):
          p.release()
          live_pools.remove(p)

      try:
        cpool = _pool(name="consts", bufs=1)
        ps = _pool(name="ps", bufs=8, space="PSUM")

        def pst(p=128, f=512, dt=F32):
            return ps.tile([p, f], dt, tag="ps", name="pst")

        def dump_y(items):
            # items: list of (y_block_index, ap [p, <=1024]) — copy (cast) to y
            dbg = _pool(name="dbg", bufs=4)
            for bi, ap in items:
                p, n = ap.shape[0], ap.shape[1]
                st = dbg.tile([128, 1024], F32, tag="dbg", name="st")
                nc.vector.tensor_copy(st[0:p, 0:n], ap)
                nc.sync.dma_start(y_d[128 * bi:128 * bi + p, 0:n], st[0:p, 0:n])
            _rel(dbg)
            raise _Dumped()

        idn = cpool.tile([128, 128], F32)
        idnb = cpool.tile([128, 128], BF16)
        cum = cpool.tile([128, 128], F32)
        mstb = cpool.tile([128, 128], BF16)
        msi = cpool.tile([128, 128], F32)
        negl = cpool.tile([128, 128], F32)
        selj = cpool.tile([64, 512], F32)
        chk = cpool.tile([64, 8], F32)
        on48 = cpool.tile([128, 2], BF16)
        ones1 = cpool.tile([128, 1], BF16)
        for t_, s_ in [(idn, idn_c), (idnb, idnb_c), (cum, cum_c), (mstb, mst_c),
                       (msi, msi_c), (negl, negl_c), (selj, selj_c), (chk, chk_c),
                       (on48, on48_c), (ones1, ones1_c)]:
            nc.sync.dma_start(t_[:], s_[:])
        eps1 = cpool.tile([128, 1], F32)
        nc.vector.memset(eps1[:], 1e-5)
        epsq = cpool.tile([128, 1], F32)
        nc.vector.memset(epsq[:], 48e-6)
        epsk = cpool.tile([128, 1], F32)
        nc.vector.memset(epsk[:], 1e-6)
        epsg = cpool.tile([128, 1], F32)
        nc.vector.memset(epsg[:], 1e-5)
        dtb_r = cpool.tile([1, 64], F32)
        nega_r = cpool.tile([1, 64], F32)
        nc.sync.dma_start(dtb_r[:], dtb_d[:])
        nc.sync.dma_start(nega_r[:], nega_d[:])
        dtb_bc = cpool.tile([128, 64], F32)
        nega_bc = cpool.tile([128, 64], F32)
        nc.gpsimd.partition_broadcast(dtb_bc[:], dtb_r[:])
        nc.gpsimd.partition_broadcast(nega_bc[:], nega_r[:])
        cqw = cpool.tile([128, 16], F32)
        ckw = cpool.tile([128, 16], F32)
        cvw = cpool.tile([128, 32], F32)
        for j in range(4):
            nc.sync.dma_start(cqw[:, 4 * j:4 * j + 4], cq_d[128 * j:128 * j + 128, :])
            nc.sync.dma_start(ckw[:, 4 * j:4 * j + 4], ck_d[128 * j:128 * j + 128, :])
        for j in range(8):
            nc.sync.dma_start(cvw[:, 4 * j:4 * j + 4], cv_d[128 * j:128 * j + 128, :])
        ab_fm = cpool.tile([16, 1024], F32)

        big = _pool(name="big", bufs=1)
        hT = big.tile([128, KT * 1024], BF16)       # also ffT in phase E
        osb = big.tile([128, 6 * 1024], BF16)       # feature-major o: [feat%128, 1024*(f//128)+tok]
        pg = _pool(name="pg", bufs=1)
        g_tok = pg.tile([128, NTOK * VD_C], BF16, tag="gtok")

        wp = _pool(name="wp", bufs=4)
        dram = _pool(name="dram", bufs=1, space="DRAM")
        oT_in = [dram.tile([D, 512], BF16, name=f"oT_in{i}") for i in range(2)]
        oT_out = [dram.tile([D, 512], BF16, name=f"oT_out{i}") for i in range(2)]
        h2T_scr = dram.tile([D, T], F32)
        bfm_scr = dram.tile([64, 128], F32)

        bigq = _pool(name="bigq", bufs=1)
        qsb = bigq.tile([128, 4 * 1024], BF16)
        ksb = bigq.tile([128, 4 * 1024], BF16)
        ktok = bigq.tile([128, 8 * 512], BF16)      # token-major k: [tok, 512ci+128j]
        vtok = bigq.tile([128, 8 * VD_C], BF16)     # token-major v: [tok, 768ci+96h]
        nc.vector.memset(qsb[:], 0.0)
        nc.vector.memset(ksb[:], 0.0)

        # ============ Phase A: hT = rmsnorm(x)^T in bf16 ============
        stA = _pool(name="stA", bufs=16)
        sqp = _pool(name="sqp", bufs=3)
        p_ss = [pst(1, 512) for _ in range(2)]
        xts = []
        for k in range(KT):
            xa = stA.tile([128, 1024], F32, tag="xT")
            nc.sync.dma_start(xa[:], xT_d[128 * k:128 * k + 128, :])
            xts.append(xa)
            sq = sqp.tile([128, 1024], BF16, tag="sq")
            nc.vector.tensor_mul(sq[:], xa[:], xa[:])
            for n in range(2):
                nc.tensor.matmul(p_ss[n][:], ones1[:], sq[:, 512 * n:512 * n + 512],
                                 start=(k == 0), stop=(k == KT - 1))
        srowA = sqp.tile([1, 1024], F32, tag="srowA", bufs=1)
        for n in range(2):
            nc.scalar.activation(srowA[:, 512 * n:512 * n + 512], p_ss[n][:],
                                 AF.Sqrt, bias=eps1[0:1, :], scale=1.0 / D)
        nc.vector.reciprocal_approx_fast(srowA[:], srowA[:])
        sbcA = sqp.tile([128, 1024], F32, tag="sbcA", bufs=1)
        nc.gpsimd.partition_broadcast(sbcA[:], srowA[:])
        for k in range(KT):
            nc.vector.tensor_mul(hT[:, 1024 * k:1024 * k + 1024], xts[k][:], sbcA[:])
        _rel(sqp)
        _rel(stA)
        if DUMP == "hT":
            dump_y([(k, hT[:, 1024 * k:1024 * k + 1024]) for k in range(KT)])

        # ============ Phase B ============
        if "B" not in PHASES:
            raise _SkipRest()
        dk = _pool(name="dk", bufs=1)
        pb = _pool(name="pb", bufs=6)

        def conv_acc(pre, cw, j):
            acc = pb.tile([128, 1024], F32, tag="s1k")
            nc.scalar.activation(acc[:], pre[:], AF.Copy, scale=cw[:, 4 * j + 3:4 * j + 4])
            for s in (1, 2, 3):
                tmp = pb.tile([128, 1024], F32, tag="s1k")
                nc.scalar.activation(tmp[:, 0:1024 - s], pre[:, 0:1024 - s],
                                     AF.Copy, scale=cw[:, 4 * j + 3 - s:4 * j + 4 - s])
                nc.vector.tensor_add(acc[:, s:1024], acc[:, s:1024], tmp[:, 0:1024 - s])
            return acc

        def qkv_pass(w_dram, cw, eps_col, mult, kind, jbase, wcol0):
            pps = [[pst() for n in range(2)] for j in range(4)]
            for k in range(KT):
                wt = wp.tile([128, 512], BF16, tag="wwide")
                nc.sync.dma_start(wt[:], w_dram[128 * k:128 * k + 128, wcol0:wcol0 + 512])
                for j in range(4):
                    for n in range(2):
                        nc.tensor.matmul(
                            pps[j][n][:], wt[:, 128 * j:128 * j + 128],
                            hT[:, 1024 * k + 512 * n:1024 * k + 512 * n + 512],
                            start=(k == 0), stop=(k == KT - 1))
            for j in range(4):
                jj = jbase + j
                pre = pb.tile([128, 1024], F32, tag="s1k")
                for n in range(2):
                    nc.vector.tensor_copy(pre[:, 512 * n:512 * n + 512], pps[j][n][:])
                acc = conv_acc(pre, cw, jj)
                if kind == "v":
                    vb = pb.tile([128, 1024], BF16, tag="vb16", bufs=2)
                    nc.scalar.activation(vb[:], acc[:], AF.Silu)
                    for ci in range(8):
                        pv = pst(128, 96, BF16)
                        nc.tensor.transpose(pv[:], vb[0:96, 128 * ci:128 * ci + 128],
                                            idnb[0:96, 0:96])
                        nc.scalar.copy(
                            vtok[:, VD_C * ci + 96 * jj:VD_C * ci + 96 * jj + 96], pv[:])
                else:
                    blk = pb.tile([128, 1024], F32, tag="s1k")
                    nc.scalar.activation(blk[:], acc[:], AF.Silu)
                    sq = pb.tile([128, 1024], BF16, tag="sqb", bufs=2)
                    nc.vector.tensor_mul(sq[:], blk[:], blk[:])
                    dst = qsb if kind == "q" else ksb
                    for hh, rh in ((0, 0), (1, 64)):
                        srow = pb.tile([1, 1024], F32, tag="srow", bufs=2)
                        for n2 in range(2):
                            p_ssq = pst(1, 512)
                            nc.tensor.matmul(
                                p_ssq[:], on48[:, hh:hh + 1], sq[:, 512 * n2:512 * n2 + 512],
                                start=True, stop=True)
                            nc.scalar.activation(srow[:, 512 * n2:512 * n2 + 512], p_ssq[:],
                                                 AF.Sqrt, bias=eps_col[0:1, :], scale=mult)
                        nc.vector.reciprocal_approx_fast(srow[:], srow[:])
                        sbc = pb.tile([128, 1024], F32, tag="sbc", bufs=2)
                        nc.gpsimd.partition_broadcast(sbc[:], srow[:])
                        nc.vector.tensor_mul(dst[rh:rh + 48, 1024 * jj:1024 * jj + 1024],
                                             blk[rh:rh + 48, :], sbc[rh:rh + 48, :])
                    if kind == "k":
                        for ci in range(8):
                            pk = pst(128, 128, BF16)
                            nc.tensor.transpose(
                                pk[:], ksb[:, 1024 * jj + 128 * ci:1024 * jj + 128 * ci + 128],
                                idnb[:])
                            nc.scalar.copy(
                                ktok[:, 512 * ci + 128 * jj:512 * ci + 128 * jj + 128], pk[:])

        qkv_pass(wq_d, cqw, epsq, 48.0, "q", 0, 0)
        qkv_pass(wk_d, ckw, epsk, 1.0, "k", 0, 0)
        qkv_pass(wv_d, cvw, None, None, "v", 0, 0)
        qkv_pass(wv_d, cvw, None, None, "v", 4, 512)
        if DUMP == "qkv":
            dump_y([(j, qsb[:, 1024 * j:1024 * j + 1024]) for j in range(4)]
                   + [(4 + j, ksb[:, 1024 * j:1024 * j + 1024]) for j in range(4)]
                   + [(8 + b, vtok[:, 1024 * b:1024 * b + 1024]) for b in range(6)]
                   + [(14 + b, ktok[:, 1024 * b:1024 * b + 1024]) for b in range(2)])

        # gate token-major
        for n in range(2):
            pgs = [pst(128, 384) for _ in range(NTOK)]
            for k in range(KT):
                wt = wp.tile([128, 384], BF16, tag="wg384")
                nc.sync.dma_start(wt[:], wg_d[128 * k:128 * k + 128, 384 * n:384 * n + 384])
                for i in range(NTOK):
                    nc.tensor.matmul(
                        pgs[i][:], hT[:, 1024 * k + 128 * i:1024 * k + 128 * i + 128], wt[:],
                        start=(k == 0), stop=(k == KT - 1))
            for i in range(NTOK):
                nc.scalar.activation(
                    g_tok[:, VD_C * i + 384 * n:VD_C * i + 384 * n + 384], pgs[i][:], AF.Silu)

        # a/b projections, feature-major [16, 1024]
        ppab = [pst(16, 512) for _ in range(2)]
        for k in range(KT):
            wt = wp.tile([128, 16], BF16, tag="wab")
            nc.sync.dma_start(wt[:], wab_d[128 * k:128 * k + 128, :])
            for n in range(2):
                nc.tensor.matmul(ppab[n][:], wt[:], hT[:, 1024 * k + 512 * n:1024 * k + 512 * n + 512],
                                 start=(k == 0), stop=(k == KT - 1))
        for n in range(2):
            nc.vector.tensor_copy(ab_fm[:, 512 * n:512 * n + 512], ppab[n][:])

        # -------- batched decay prep for all chunks --------
        gta = dk.tile([128, 64], F32)
        bta = dk.tile([128, 64], F32)
        for ci in range(8):
            p_ab = pst(128, 16)
            nc.tensor.transpose(p_ab[:], ab_fm[:, 128 * ci:128 * ci + 128], idn[0:16, 0:16])
            nc.vector.tensor_copy(gta[:, 8 * ci:8 * ci + 8], p_ab[:, 0:8])
            nc.vector.tensor_copy(bta[:, 8 * ci:8 * ci + 8], p_ab[:, 8:16])
        nc.vector.tensor_add(gta[:], gta[:], dtb_bc[:])
        nc.scalar.activation(gta[:], gta[:], AF.Exp)
        nc.vector.tensor_scalar_add(gta[:], gta[:], 1.0)
        nc.scalar.activation(gta[:], gta[:], AF.Ln)
        nc.vector.tensor_mul(gta[:], gta[:], nega_bc[:])        # gt_all [128,64]
        beta_all = dk.tile([128, 64], F32)
        nc.scalar.activation(beta_all[:], bta[:], AF.Sigmoid)
        nbeta_all = dk.tile([128, 64], F32)
        nc.vector.tensor_scalar_mul(nbeta_all[:], beta_all[:], -1.0)
        p_bc = pst(128, 64)
        nc.tensor.matmul(p_bc[:], cum[:], gta[:], start=True, stop=True)
        bcum_tok = dk.tile([128, 64], F32)
        nc.vector.tensor_copy(bcum_tok[:], p_bc[:])
        lam_all = dk.tile([128, 64], F32)
        nc.scalar.activation(lam_all[:], p_bc[:], AF.Exp)
        p_bf = pst(64, 128)
        nc.tensor.transpose(p_bf[:], bcum_tok[:], idn[:])
        b_fm = dk.tile([64, 128], F32)
        nc.vector.tensor_copy(b_fm[:], p_bf[:])
        nc.scalar.dma_start(bfm_scr[:], b_fm[:])
        wfm = dk.tile([64, 128], F32)
        nc.vector.tensor_scalar(wfm[:], b_fm[:], b_fm[:, 127:128], None, OP.subtract)
        nc.scalar.activation(wfm[:], wfm[:], AF.Exp, scale=-1.0)
        p_wt = pst(128, 64)
        nc.tensor.transpose(p_wt[:], wfm[:], idn[0:64, 0:64])
        w_tok = dk.tile([128, 64], F32)
        nc.vector.tensor_copy(w_tok[:], p_wt[:])
        ebc_all = dk.tile([64, 1], F32)
        nc.scalar.activation(ebc_all[:], b_fm[:, 127:128], AF.Exp)
        # EB[8ci+h, ci] = ebc_all[8ci+h]; ebcJ[j][p, ci] = per-(ci,j) state-decay col
        EB = dk.tile([64, 8], F32)
        nc.vector.tensor_scalar_mul(EB[:], chk[:], ebc_all[:, 0:1])
        ebcJ = []
        for j in range(4):
            p_ebj = pst(128, 8)
            nc.tensor.matmul(p_ebj[:], selj[:, 128 * j:128 * j + 128], EB[:],
                             start=True, stop=True)
            ej = dk.tile([128, 8], F32, tag=f"ebj{j}", name=f"ebj{j}")
            nc.vector.tensor_copy(ej[:], p_ebj[:])
            ebcJ.append(ej)
        _rel(pb)

        # ============ Phase C ============
        if "C" not in PHASES:
            raise _SkipRest()
        wod = _pool(name="wod", bufs=16)
        pd = _pool(name="pd", bufs=8)
        dput = _pool(name="dput", bufs=48)
        dpa = _pool(name="dpa", bufs=10)
        dpx = _pool(name="dpx", bufs=10)
        dpf = _pool(name="dpf", bufs=6)
        dp2 = _pool(name="dp2", bufs=2)
        spool = _pool(name="spool", bufs=2)

        def emit_D(half):
            t0 = 512 * half
            for db in range(16):
                wt = wod.tile([128, VD_C], BF16, tag="wo", name="wo_t", bufs=16)
                nc.scalar.dma_start(wt[:], wo_d[:, VD_C * db:VD_C * db + VD_C])
                pp = pst()
                for fb in range(6):
                    nc.tensor.matmul(pp[:], wt[:, 128 * fb:128 * fb + 128],
                                     osb[:, 1024 * fb + t0:1024 * fb + t0 + 512],
                                     start=(fb == 0), stop=(fb == 5))
                stg = pd.tile([128, 512], BF16, tag="s512b", name="stg", bufs=8)
                nc.scalar.copy(stg[:], pp[:])
                nc.sync.dma_start(oT_in[half][128 * db:128 * db + 128, :], stg[:])
            nc.gpsimd.collective_compute(
                "AllReduce", OP.add, ins=[oT_in[half][:]], outs=[oT_out[half][:]],
                replica_groups=groups)

        S_cur = {}
        for j in range(4):
            S_cur[j] = spool.tile([128, DV], F32, tag=f"s{j}", name=f"s{j}")
            nc.vector.memset(S_cur[j][:], 0.0)

        for ci in range(NCH):
            # ---- prep all 8 heads: abar, xx, xt ----
            ABAR, XX, XT = {}, {}, {}
            for j in range(4):
                for hh in range(2):
                    h = 2 * j + hh
                    rh = 64 * hh
                    kts = ksb[rh:rh + 48, 1024 * j + 128 * ci:1024 * j + 128 * ci + 128]
                    qts = qsb[rh:rh + 48, 1024 * j + 128 * ci:1024 * j + 128 * ci + 128]
                    p_kk = pst(128, 128)
                    nc.tensor.matmul(p_kk[:], kts, kts, start=True, stop=True)
                    p_kq = pst(128, 128)
                    nc.tensor.matmul(p_kq[:], kts, qts, start=True, stop=True)
                    bc128 = dpf.tile([128, 128], F32, tag="bc", name="bc128")
                    nc.gpsimd.dma_start(
                        bc128[:],
                        bfm_scr[8 * ci + h:8 * ci + h + 1, :].to_broadcast((128, 128)))
                    dte = dpf.tile([128, 128], F32, tag="dte", name="dte")
                    nc.vector.tensor_scalar(dte[:], bc128[:],
                                            bcum_tok[:, 8 * ci + h:8 * ci + h + 1],
                                            None, OP.subtract)
                    nc.vector.tensor_mul(dte[:], dte[:], msi[:])
                    nc.vector.tensor_add(dte[:], dte[:], negl[:])
                    dincl = dput.tile([128, 128], BF16, tag="ut", name="dincl")
                    nc.scalar.activation(dincl[:], dte[:], AF.Exp)
                    abar = dpa.tile([128, 128], BF16, tag="abar", name="abar")
                    nc.vector.tensor_mul(abar[:], p_kq[:], dincl[:])
                    dstr = dput.tile([128, 128], BF16, tag="ut", name="dstr")
                    nc.vector.tensor_mul(dstr[:], dincl[:], mstb[:])
                    x0 = dput.tile([128, 128], BF16, tag="ut", name="x0")
                    nc.vector.tensor_mul(x0[:], p_kk[:], dstr[:])
                    xx = dpx.tile([128, 128], BF16, tag="xx", name="xx")
                    nc.vector.tensor_scalar_mul(xx[:], x0[:],
                                                nbeta_all[:, 8 * ci + h:8 * ci + h + 1])
                    p_x = pst(128, 128, BF16)
                    nc.tensor.transpose(p_x[:], xx[:], idnb[:])
                    xt = dpx.tile([128, 128], BF16, tag="xt", name="xt")
                    nc.vector.tensor_copy(xt[:], p_x[:])
                    ABAR[h], XX[h], XT[h] = abar, xx, xt

            # ---- UT inverse, level-major across all 8 heads ----
            PM = {}
            for h in range(HP):
                pmat = dput.tile([128, 128], BF16, tag="ut", name="pmat")
                nc.vector.tensor_add(pmat[:], XX[h][:], idnb[:])
                PM[h] = pmat
            cur = {h: (XX[h], XT[h]) for h in range(HP)}
            for lvl in range(6):
                last = lvl == 5
                nxt = {}
                for h in range(HP):
                    xx, xt = cur[h]
                    x2 = None
                    if not last:
                        p_sq = pst(128, 128)
                        nc.tensor.matmul(p_sq[:], xt[:], xx[:], start=True, stop=True)
                        x2 = dput.tile([128, 128], BF16, tag="ut", name="x2")
                        nc.scalar.copy(x2[:], p_sq[:])
                    p_sqt = pst(128, 128)
                    nc.tensor.matmul(p_sqt[:], xx[:], xt[:], start=True, stop=True)
                    xt2 = dput.tile([128, 128], BF16, tag="ut", name="xt2")
                    nc.scalar.copy(xt2[:], p_sqt[:])
                    nxt[h] = (x2, xt2)
                for h in range(HP):
                    p_pr = pst(128, 128)
                    nc.tensor.matmul(p_pr[:], nxt[h][1][:], PM[h][:], start=True, stop=True)
                    pnew = dput.tile([128, 128], BF16, tag="ut", name="pnew")
                    nc.vector.tensor_add(pnew[:], PM[h][:], p_pr[:])
                    PM[h] = pnew
                if not last:
                    cur = nxt

            # ---- state/output, step-major in 4-head waves ----
            ot_all = dp2.tile([128, VD_C], F32, tag="otall", name="ot_all")
            for jp in (0, 2):
                heads = [(j, hh) for j in (jp, jp + 1) for hh in (0, 1)]
                SB, PS, KW = {}, {}, {}
                for j in (jp, jp + 1):
                    S_bf = dp2.tile([128, DV], BF16, tag=f"sbf{j}", name="S_bf")
                    nc.vector.tensor_copy(S_bf[:], S_cur[j][:])
                    SB[j] = S_bf
                    PS[j] = pst(128, DV)
                for j, hh in heads:
                    h = 2 * j + hh
                    rh = 64 * hh
                    kw = dp2.tile([128, 48], BF16, tag="kw", name="kw", bufs=6)
                    nc.vector.tensor_scalar_mul(
                        kw[:], ktok[:, 512 * ci + 128 * j + rh:512 * ci + 128 * j + rh + 48],
                        w_tok[:, 8 * ci + h:8 * ci + h + 1])
                    KW[h] = kw
                RR, UU = {}, {}
                for j, hh in heads:
                    h = 2 * j + hh
                    rh = 64 * hh
                    kts = ksb[rh:rh + 48, 1024 * j + 128 * ci:1024 * j + 128 * ci + 128]
                    p_ks = pst(128, DV)
                    nc.tensor.matmul(p_ks[:], kts, SB[j][rh:rh + 48, :], start=True, stop=True)
                    r_ = dp2.tile([128, DV], BF16, tag="rr", name="r_", bufs=5)
                    nc.vector.tensor_scalar_mul(r_[:], p_ks[:],
                                                lam_all[:, 8 * ci + h:8 * ci + h + 1])
                    nc.vector.tensor_sub(
                        r_[:], vtok[:, VD_C * ci + 96 * h:VD_C * ci + 96 * h + 96], r_[:])
                    RR[h] = r_
                for j, hh in heads:
                    h = 2 * j + hh
                    p_w = pst(128, DV)
                    nc.tensor.matmul(p_w[:], PM[h][:], RR[h][:], start=True, stop=True)
                    u_ = dp2.tile([128, DV], BF16, tag="uu", name="u_", bufs=5)
                    nc.vector.tensor_scalar_mul(u_[:], p_w[:],
                                                beta_all[:, 8 * ci + h:8 * ci + h + 1])
                    UU[h] = u_
                for j, hh in heads:
                    h = 2 * j + hh
                    rh = 64 * hh
                    qts = qsb[rh:rh + 48, 1024 * j + 128 * ci:1024 * j + 128 * ci + 128]
                    p_oi = pst(128, DV)
                    nc.tensor.matmul(p_oi[:], ABAR[h][:], UU[h][:], start=True, stop=True)
                    p_qs = pst(128, DV)
                    nc.tensor.matmul(p_qs[:], qts, SB[j][rh:rh + 48, :], start=True, stop=True)
                    ots = ot_all[:, 96 * h:96 * h + 96]
                    nc.vector.tensor_scalar_mul(ots, p_qs[:],
                                                lam_all[:, 8 * ci + h:8 * ci + h + 1])
                    nc.vector.tensor_add(ots, ots, p_oi[:])
                    nc.tensor.matmul(PS[j][rh:rh + 48, :], KW[h][:], UU[h][:],
                                     start=True, stop=True)
                for j in (jp, jp + 1):
                    s_new = spool.tile([128, DV], F32, tag=f"s{j}", name="s_new")
                    for rh2 in (0, 64):
                        nc.vector.tensor_scalar_mul(
                            s_new[rh2:rh2 + 48, :], S_cur[j][rh2:rh2 + 48, :],
                            ebcJ[j][rh2:rh2 + 48, ci:ci + 1])
                        nc.vector.tensor_add(
                            s_new[rh2:rh2 + 48, :], s_new[rh2:rh2 + 48, :],
                            PS[j][rh2:rh2 + 48, :])
                    S_cur[j] = s_new

            # ---- batched gated rmsnorm + gate + transpose to osb ----
            osq = dp2.tile([128, VD_C], F32, tag="osq", name="osq")
            nc.vector.tensor_mul(osq[:], ot_all[:], ot_all[:])
            rcol8 = dp2.tile([128, 8], F32, tag="rc8", name="rcol8")
            for h in range(HP):
                nc.vector.tensor_reduce(rcol8[:, h:h + 1], osq[:, 96 * h:96 * h + 96],
                                        mybir.AxisListType.X, OP.add)
            nc.scalar.activation(rcol8[:], rcol8[:], AF.Sqrt, bias=epsg[:], scale=1.0 / DV)
            nc.vector.reciprocal_approx_fast(rcol8[:], rcol8[:])
            for h in range(HP):
                nc.vector.tensor_scalar_mul(ot_all[:, 96 * h:96 * h + 96],
                                            ot_all[:, 96 * h:96 * h + 96], rcol8[:, h:h + 1])
            ob = dp2.tile([128, VD_C], BF16, tag="ob", name="ob")
            nc.vector.tensor_mul(ob[:], ot_all[:], g_tok[:, VD_C * ci:VD_C * ci + VD_C])
            for b6 in range(6):
                p_ot = pst(128, 128, BF16)
                nc.tensor.transpose(p_ot[:], ob[:, 128 * b6:128 * b6 + 128], idnb[:])
                nc.scalar.copy(osb[:, 1024 * b6 + 128 * ci:1024 * b6 + 128 * ci + 128], p_ot[:])

            # ---- o_projT half + AllReduce as soon as its tokens exist ----
            if ci == 3:
                emit_D(0)
            if ci == 7:
                emit_D(1)

        for p in (spool, dp2, dpf, dpx, dpa, dput, pd, wod):
            _rel(p)
        if DUMP == "o":
            dump_y([(b, osb[:, 1024 * b:1024 * b + 1024]) for b in range(6)]
                   + [(6 + b, g_tok[:, 1024 * b:1024 * b + 1024]) for b in range(6)]
                   + [(12 + b, ktok[:, 1024 * b:1024 * b + 1024]) for b in range(4)])
        if DUMP == "dk":
            dump_y([(0, gta[:]), (1, beta_all[:]), (2, bcum_tok[:]),
                    (3, lam_all[:]), (4, w_tok[:]), (5, b_fm[:]),
                    (6, ebc_all[:]), (7, ebcJ[0][:]), (8, ebcJ[3][:])])
        _rel(dk)
        _rel(bigq)

        if "D" not in PHASES:
            raise _SkipRest()
        if DUMP in ("ar", "oin"):
            src = oT_out if DUMP == "ar" else oT_in
            dbg = _pool(name="dbg", bufs=4)
            for bi in range(16):
                st = dbg.tile([128, 1024], F32, tag="dbg", name="st")
                for half in range(2):
                    so = dbg.tile([128, 512], BF16, tag="dbg2", name="so")
                    nc.sync.dma_start(so[:], src[half][128 * bi:128 * bi + 128, :])
                    nc.vector.tensor_copy(st[:, 512 * half:512 * half + 512], so[:])
                nc.sync.dma_start(y_d[128 * bi:128 * bi + 128, :], st[:])
            _rel(dbg)
            raise _Dumped()

        # ============ Phase E ============
        if "E" not in PHASES:
            raise _SkipRest()
        seq = _pool(name="seq", bufs=3)
        pe = _pool(name="pe", bufs=4)
        ffT = hT
        p_s2 = [pst(1, 512) for _ in range(2)]
        for half in range(2):
            hs = slice(512 * half, 512 * half + 512)
            for k in range(KT):
                xe = seq.tile([128, 512], F32, tag="xe")
                nc.sync.dma_start(xe[:], xT_d[128 * k:128 * k + 128, hs])
                oe = seq.tile([128, 512], BF16, tag="oe")
                nc.gpsimd.dma_start(oe[:], oT_out[half][128 * k:128 * k + 128, :])
                h2 = seq.tile([128, 512], F32, tag="h2T")
                nc.vector.tensor_add(h2[:], xe[:], oe[:])
                nc.scalar.dma_start(h2T_scr[128 * k:128 * k + 128, hs], h2[:])
                sqe = seq.tile([128, 512], BF16, tag="sqe")
                nc.vector.tensor_mul(sqe[:], h2[:], h2[:])
                nc.tensor.matmul(p_s2[half][:], ones1[:], sqe[:],
                                 start=(k == 0), stop=(k == KT - 1))
        srowE = seq.tile([1, 1024], F32, tag="srowE", bufs=1)
        for n in range(2):
            nc.scalar.activation(srowE[:, 512 * n:512 * n + 512], p_s2[n][:],
                                 AF.Sqrt, bias=eps1[0:1, :], scale=1.0 / D)
        nc.vector.reciprocal_approx_fast(srowE[:], srowE[:])
        sbcE = seq.tile([128, 1024], F32, tag="sbcE", bufs=1)
        nc.gpsimd.partition_broadcast(sbcE[:], srowE[:])
        for k in range(KT):
            h2r = seq.tile([128, 1024], F32, tag="h2r", bufs=4)
            nc.sync.dma_start(h2r[:], h2T_scr[128 * k:128 * k + 128, :])
            nc.vector.tensor_mul(ffT[:, 1024 * k:1024 * k + 1024], h2r[:], sbcE[:])
        if DUMP == "ffT":
            dump_y([(k, ffT[:, 1024 * k:1024 * k + 1024]) for k in range(KT)])

        mida = pg.tile([128, 6 * 1024], BF16, tag="gtok")
        pmid = _pool(name="pmid", bufs=1)
        midb = pmid.tile([128, 5 * 1024], BF16, tag="midb")

        def mid_ap(m, off, ln):
            if m < 6:
                return mida[:, 1024 * m + off:1024 * m + off + ln]
            return midb[:, 1024 * (m - 6) + off:1024 * (m - 6) + off + ln]

        wp13 = _pool(name="wp13", bufs=3)
        for m in range(11):
            pu1 = [pst() for _ in range(2)]
            pu3 = [pst() for _ in range(2)]
            wt13 = wp13.tile([128, 4096], BF16, tag="w13", name="wt13")
            nc.sync.dma_start(wt13[:], w13_d[:, 4096 * m:4096 * m + 4096])
            for k in range(KT):
                for n in range(2):
                    rhs = ffT[:, 1024 * k + 512 * n:1024 * k + 512 * n + 512]
                    nc.tensor.matmul(pu1[n][:], wt13[:, 256 * k:256 * k + 128], rhs,
                                     start=(k == 0), stop=(k == KT - 1))
                    nc.tensor.matmul(pu3[n][:], wt13[:, 256 * k + 128:256 * k + 256], rhs,
                                     start=(k == 0), stop=(k == KT - 1))
            for n in range(2):
                u1s = pe.tile([128, 512], F32, tag="s512", name="u1s")
                nc.scalar.activation(u1s[:], pu1[n][:], AF.Silu)
                nc.vector.tensor_mul(mid_ap(m, 512 * n, 512), u1s[:], pu3[n][:])

        if DUMP == "mid":
            dump_y([(m, mid_ap(m, 0, 1024)) for m in range(11)])
        wp2 = _pool(name="wp2", bufs=3)
        for db in range(16):
            wt2 = wp2.tile([128, INT_C], BF16, tag="w2", name="w2_t")
            nc.sync.dma_start(wt2[:], w2_d[:, INT_C * db:INT_C * db + INT_C])
            for half in range(2):
                pps = pst()
                for m in range(11):
                    nc.tensor.matmul(pps[:], wt2[:, 128 * m:128 * m + 128],
                                     mid_ap(m, 512 * half, 512),
                                     start=(m == 0), stop=(m == 10))
                h2t = pe.tile([128, 512], F32, tag="s512", name="h2t")
                nc.sync.dma_start(h2t[:], h2T_scr[128 * db:128 * db + 128,
                                                  512 * half:512 * half + 512])
                yst = pe.tile([128, 512], F32, tag="s512", name="yst")
                nc.vector.tensor_scalar_mul(yst[:], h2t[:], 0.25)
                nc.vector.tensor_add(yst[:], yst[:], pps[:])
                nc.sync.dma_start(y_d[128 * db:128 * db + 128, 512 * half:512 * half + 512],
                                  yst[:])

        for p in (wp2, wp13, pmid, pe, seq, dram, wp, pg, big, ps, cpool):
            _rel(p)
      except _SkipRest:
        zst = _pool(name="zst", bufs=1)
        zt = zst.tile([128, 512], F32)
        nc.vector.memset(zt[:], 0.0)
        for i in range(16):
            for dh in range(2):
                nc.sync.dma_start(y_d[128 * i:128 * i + 128, 512 * dh:512 * dh + 512], zt[:])
        for p in reversed(live_pools):
            p.release()
      except _Dumped:
        for p in reversed(live_pools):
            p.release()

    nc.compile()
    return nc


def _pack_wo(wo):
    # [768, 2048] -> [128, 16*768]: col = 768*db + 128*fb + c
    return np.ascontiguousarray(
        wo.reshape(6, 128, 16, 128).transpose(1, 2, 0, 3).reshape(128, 16 * 768))


def _pack_w13(w1, w3):
    # [2048, 1408] x2 -> [128, 11*4096]: col = 4096*m + 256*k + 128*which + c
    a = w1.reshape(16, 128, 11, 128).transpose(1, 2, 0, 3)   # [128, 11, 16, 128]
    b = w3.reshape(16, 128, 11, 128).transpose(1, 2, 0, 3)
    return np.ascontiguousarray(
        np.stack([a, b], axis=3).reshape(128, 11 * 4096))


def _pack_w2(w2):
    # [1408, 2048] -> [128, 16*1408]: col = 1408*db + 128*m + c
    return np.ascontiguousarray(
        w2.reshape(11, 128, 16, 128).transpose(1, 2, 0, 3).reshape(128, 16 * 1408))


def _shard(inputs):
    f32 = np.float32
    bf = ml_dtypes.bfloat16
    rms1 = np.asarray(inputs["rms1_w"], f32)
    rms2 = np.asarray(inputs["rms2_w"], f32)
    gn = np.asarray(inputs["gnorm_w"], f32)
    in_maps = []
    for c in range(8):
        g, m = c // 4, c % 4
        qs = slice(384 * m, 384 * m + 384)
        vs = slice(768 * m, 768 * m + 768)
        hs = slice(8 * m, 8 * m + 8)
        isl = slice(1408 * m, 1408 * m + 1408)

        def padqk(w):
            wp_ = np.zeros((D, QKP), f32)
            for h in range(8):
                wp_[:, 64 * h:64 * h + 48] = w[:, 48 * h:48 * h + 48]
            return wp_

        def padcw(w):
            cp = np.zeros((QKP, 4), f32)
            for h in range(8):
                cp[64 * h:64 * h + 48] = w[48 * h:48 * h + 48]
            return cp

        def padv(w):
            colpad = w.shape[0] == D
            out = np.zeros((D, VP) if colpad else (VP, w.shape[1]), f32)
            for h in range(8):
                if colpad:
                    out[:, 128 * h:128 * h + 96] = w[:, 96 * h:96 * h + 96]
                else:
                    out[128 * h:128 * h + 96] = w[96 * h:96 * h + 96]
            return out

        dtb8 = np.asarray(inputs["dt_bias"], f32)[hs]
        nega8 = -np.exp(np.asarray(inputs["A_log"], f32)[hs])
        in_maps.append(dict(
            xT=np.ascontiguousarray(np.asarray(inputs["x"], f32)[g].T),
            wq=padqk(np.asarray(inputs["Wq"], f32)[:, qs] * rms1[:, None]).astype(bf),
            wk=padqk(np.asarray(inputs["Wk"], f32)[:, qs] * rms1[:, None]).astype(bf),
            wv=padv(np.asarray(inputs["Wv"], f32)[:, vs] * rms1[:, None]).astype(bf),
            wg=np.ascontiguousarray(
                np.asarray(inputs["Wg"], f32)[:, vs] * rms1[:, None]).astype(bf),
            wab=np.ascontiguousarray(np.concatenate(
                [np.asarray(inputs["Wa"], f32)[:, hs],
                 np.asarray(inputs["Wb"], f32)[:, hs]], 1) * rms1[:, None]).astype(bf),
            cq=padcw(np.asarray(inputs["conv_q_w"], f32)[qs]),
            ck=padcw(np.asarray(inputs["conv_k_w"], f32)[qs]),
            cv=padv(np.asarray(inputs["conv_v_w"], f32)[vs]),
            dtb=np.tile(dtb8, 8).reshape(1, 64).copy(),
            nega=np.tile(nega8, 8).reshape(1, 64).copy(),
            wo=_pack_wo(np.asarray(inputs["Wo"], f32)[vs] * np.tile(gn, 8)[:, None]).astype(bf),
            w13=_pack_w13(np.asarray(inputs["W1"], f32)[:, isl] * rms2[:, None],
                          np.asarray(inputs["W3"], f32)[:, isl] * rms2[:, None]).astype(bf),
            w2=_pack_w2(np.asarray(inputs["W2"], f32)[isl]).astype(bf),
        ))
    return in_maps


def kernel(**inputs):
    if "nc" not in _cache:
        _cache["nc"] = _build(8)
    res = run_bass_kernel_spmd(_cache["nc"], _shard(inputs), list(range(8)))
    out = np.zeros((B, T, D), np.float32)
    for g in range(2):
        yT = sum(res.results[4 * g + m]["y"] for m in range(4))
        out[g] = yT.T
    return out


# revision 43
# speedup vs baseline: 1.8390x; 1.0474x over previous
"""GatedDeltaNet block kernel for 8 Trainium2 cores (Bass/Tile), bf16 rework.

Sharding: DP2 (batch) x TP4 (heads / MLP-inter). Core c: group g=c//4 runs
batch g; member m=c%4 owns heads [8m,8m+8), q/k cols [384m,..), v/g cols
[768m,..), INTER [1408m,..). Two half-token AllReduces per 4-core group after
o_proj (overlapped with o_proj compute); final down-proj partials summed on
the host.

Everything runs feature-major (host passes x transposed, takes y transposed)
so there are no PE transposes outside the delta-rule inner loop. All big
GEMM operands are bf16 (host-cast weights); psum accumulation, the delta-rule
state, decay/beta math and norms stay fp32.

Per-core dataflow:
  A: xT [D,T] -> rmsnorm via matmul-accumulated column sumsq -> hT bf16 (SBUF)
  B: bf16 projections off hT; q/k feature-major (heads padded to 64 rows)
     -> conv+silu+l2norm -> SBUF (+ token-major copies of k, v via PE
     transposes); gate token-major; a/b -> batched decay prep for all chunks
  C: chunked gated delta rule (C=128, UT transform via log-doubling inverse,
     bf16 matmuls / fp32 state); writes normed+gated o feature-major to SBUF
  D: o_projT in two token halves, each followed by its AllReduce (overlapped)
  E: h2T = xT + oT; rmsnorm -> ffT bf16 (reuses hT); MLP bf16; yT partials
"""
import sys
sys.path.insert(0, '/opt/trn_rl_repo')
import numpy as np
import ml_dtypes

import concourse.bass as bass
import concourse.bacc as bacc
import concourse.mybir as mybir
import concourse.tile as tile
from concourse.bass_isa import ReduceOp
from concourse.bass_utils import run_bass_kernel_spmd

F32 = mybir.dt.float32
BF16 = mybir.dt.bfloat16
AF = mybir.ActivationFunctionType
OP = mybir.AluOpType

B, T, D = 2, 1024, 2048
H, DK, DV = 32, 48, 96
HP = 8
QKP = 512
VD_C = 768
VP = 1024
INT_C = 1408
C = 128
NCHUNK = T // C
KT = D // 128
NTOK = T // 128

_cache = {}
import os
PHASES = os.environ.get("DN_PHASES", "ABCDE")
NCH = int(os.environ.get("DN_NCHUNK", str(T // C)))
DUMP = os.environ.get("DN_DUMP", "")


class _SkipRest(Exception):
    pass


class _Dumped(Exception):
    pass


def _build(n_cores=8):
    groups = [[0, 1, 2, 3], [4, 5, 6, 7]] if n_cores == 8 else [[0]]
    nc = bacc.Bacc("TRN2", target_bir_lowering=False, debug=False, num_devices=n_cores)

    xT_d = nc.dram_tensor("xT", [D, T], F32, kind="ExternalInput")
    wq_d = nc.dram_tensor("wq", [D, QKP], BF16, kind="ExternalInput")
    wk_d = nc.dram_tensor("wk", [D, QKP], BF16, kind="ExternalInput")
    wv_d = nc.dram_tensor("wv", [D, VP], BF16, kind="ExternalInput")
    wg_d = nc.dram_tensor("wg", [D, VD_C], BF16, kind="ExternalInput")
    wab_d = nc.dram_tensor("wab", [D, 16], BF16, kind="ExternalInput")
    cq_d = nc.dram_tensor("cq", [QKP, 4], F32, kind="ExternalInput")
    ck_d = nc.dram_tensor("ck", [QKP, 4], F32, kind="ExternalInput")
    cv_d = nc.dram_tensor("cv", [VP, 4], F32, kind="ExternalInput")
    dtb_d = nc.dram_tensor("dtb", [1, 64], F32, kind="ExternalInput")
    nega_d = nc.dram_tensor("nega", [1, 64], F32, kind="ExternalInput")
    wo_d = nc.dram_tensor("wo", [128, 16 * VD_C], BF16, kind="ExternalInput")
    w13_d = nc.dram_tensor("w13", [128, 11 * 4096], BF16, kind="ExternalInput")
    w2_d = nc.dram_tensor("w2", [128, 16 * INT_C], BF16, kind="ExternalInput")
    y_d = nc.dram_tensor("y", [D, T], F32, kind="ExternalOutput")

    ones = np.ones((128, 128), np.float32)
    idn_c = nc.inline_tensor(np.eye(128, dtype=np.float32), "idn_c")
    idnb_c = nc.inline_tensor(np.eye(128, dtype=ml_dtypes.bfloat16), "idnb_c")
    cum_c = nc.inline_tensor(np.triu(ones).copy(), "cum_c")
    mst_c = nc.inline_tensor(np.triu(ones, 1).astype(ml_dtypes.bfloat16), "mst_c")
    msi_c = nc.inline_tensor(np.triu(ones).copy(), "msi_c")
    negl_c = nc.inline_tensor((np.tril(ones, -1) * -1e30).copy(), "negl_c")
    # SELJ[r, 128j+p] = 1 iff (r%8==2j and p<48) or (r%8==2j+1 and 64<=p<112)
    selj_np = np.zeros((64, 512), np.float32)
    for r in range(64):
        for j in range(4):
            if r % 8 == 2 * j:
                selj_np[r, 128 * j:128 * j + 48] = 1.0
            if r % 8 == 2 * j + 1:
                selj_np[r, 128 * j + 64:128 * j + 112] = 1.0
    selj_c = nc.inline_tensor(selj_np, "selj_c")
    # CHK[8ci+h, ci] = 1
    chk_np = np.zeros((64, 8), np.float32)
    for ci in range(8):
        chk_np[8 * ci:8 * ci + 8, ci] = 1.0
    chk_c = nc.inline_tensor(chk_np, "chk_c")
    on48_np = np.zeros((128, 2), ml_dtypes.bfloat16)
    on48_np[0:48, 0] = 1.0
    on48_np[64:112, 1] = 1.0
    on48_c = nc.inline_tensor(on48_np, "on48_c")
    ones1_np = np.ones((128, 1), ml_dtypes.bfloat16)
    ones1_c = nc.inline_tensor(ones1_np, "ones1_c")

    with tile.TileContext(nc) as tc:
      live_pools = []

      def _pool(**kw):
          p = tc.alloc_tile_pool(**kw)
          live_pools.append(p)
          return p

      def _rel(p):
          p.release()
          live_pools.remove(p)

      try:
        cpool = _pool(name="consts", bufs=1)
        ps = _pool(name="ps", bufs=8, space="PSUM")

        def pst(p=128, f=512, dt=F32):
            return ps.tile([p, f], dt, tag="ps", name="pst")

        def dump_y(items):
            # items: list of (y_block_index, ap [p, <=1024]) — copy (cast) to y
            dbg = _pool(name="dbg", bufs=4)
            for bi, ap in items:
                p, n = ap.shape[0], ap.shape[1]
                st = dbg.tile([128, 1024], F32, tag="dbg", name="st")
                nc.vector.tensor_copy(st[0:p, 0:n], ap)
                nc.sync.dma_start(y_d[128 * bi:128 * bi + p, 0:n], st[0:p, 0:n])
            _rel(dbg)
            raise _Dumped()

        idn = cpool.tile([128, 128], F32)
        idnb = cpool.tile([128, 128], BF16)
        cum = cpool.tile([128, 128], F32)
        mstb = cpool.tile([128, 128], BF16)
        msi = cpool.tile([128, 128], F32)
        negl = cpool.tile([128, 128], F32)
        selj = cpool.tile([64, 512], F32)
        chk = cpool.tile([64, 8], F32)
        on48 = cpool.tile([128, 2], BF16)
        ones1 = cpool.tile([128, 1], BF16)
        for t_, s_ in [(idn, idn_c), (idnb, idnb_c), (cum, cum_c), (mstb, mst_c),
                       (msi, msi_c), (negl, negl_c), (selj, selj_c), (chk, chk_c),
                       (on48, on48_c), (ones1, ones1_c)]:
            nc.sync.dma_start(t_[:], s_[:])
        eps1 = cpool.tile([128, 1], F32)
        nc.vector.memset(eps1[:], 1e-5)
        epsq = cpool.tile([128, 1], F32)
        nc.vector.memset(epsq[:], 48e-6)
        epsk = cpool.tile([128, 1], F32)
        nc.vector.memset(epsk[:], 1e-6)
        epsg = cpool.tile([128, 1], F32)
        nc.vector.memset(epsg[:], 1e-5)
        dtb_r = cpool.tile([1, 64], F32)
        nega_r = cpool.tile([1, 64], F32)
        nc.sync.dma_start(dtb_r[:], dtb_d[:])
        nc.sync.dma_start(nega_r[:], nega_d[:])
        dtb_bc = cpool.tile([128, 64], F32)
        nega_bc = cpool.tile([128, 64], F32)
        nc.gpsimd.partition_broadcast(dtb_bc[:], dtb_r[:])
        nc.gpsimd.partition_broadcast(nega_bc[:], nega_r[:])
        cqw = cpool.tile([128, 16], F32)
        ckw = cpool.tile([128, 16], F32)
        cvw = cpool.tile([128, 32], F32)
        for j in range(4):
            nc.sync.dma_start(cqw[:, 4 * j:4 * j + 4], cq_d[128 * j:128 * j + 128, :])
            nc.sync.dma_start(ckw[:, 4 * j:4 * j + 4], ck_d[128 * j:128 * j + 128, :])
        for j in range(8):
            nc.sync.dma_start(cvw[:, 4 * j:4 * j + 4], cv_d[128 * j:128 * j + 128, :])
        ab_fm = cpool.tile([16, 1024], F32)

        big = _pool(name="big", bufs=1)
        hT = big.tile([128, KT * 1024], BF16)       # also ffT in phase E
        osb = big.tile([128, 6 * 1024], BF16)       # feature-major o: [feat%128, 1024*(f//128)+tok]
        pg = _pool(name="pg", bufs=1)
        g_tok = pg.tile([128, NTOK * VD_C], BF16, tag="gtok")

        wp = _pool(name="wp", bufs=4)
        dram = _pool(name="dram", bufs=1, space="DRAM")
        oT_in = [dram.tile([D, 512], BF16, name=f"oT_in{i}") for i in range(2)]
        oT_out = [dram.tile([D, 512], BF16, name=f"oT_out{i}") for i in range(2)]
        h2T_scr = dram.tile([D, T], F32)
        bfm_scr = dram.tile([64, 128], F32)

        bigq = _pool(name="bigq", bufs=1)
        qsb = bigq.tile([128, 4 * 1024], BF16)
        ksb = bigq.tile([128, 4 * 1024], BF16)
        ktok = bigq.tile([128, 8 * 512], BF16)      # token-major k: [tok, 512ci+128j]
        vtok = bigq.tile([128, 8 * VD_C], BF16)     # token-major v: [tok, 768ci+96h]
        nc.vector.memset(qsb[:], 0.0)
        nc.vector.memset(ksb[:], 0.0)

        # ============ Phase A: hT = rmsnorm(x)^T in bf16 ============
        stA = _pool(name="stA", bufs=16)
        sqp = _pool(name="sqp", bufs=3)
        p_ss = [pst(1, 512) for _ in range(2)]
        xts = []
        for k in range(KT):
            xa = stA.tile([128, 1024], F32, tag="xT")
            nc.sync.dma_start(xa[:], xT_d[128 * k:128 * k + 128, :])
            xts.append(xa)
            sq = sqp.tile([128, 1024], BF16, tag="sq")
            nc.vector.tensor_mul(sq[:], xa[:], xa[:])
            for n in range(2):
                nc.tensor.matmul(p_ss[n][:], ones1[:], sq[:, 512 * n:512 * n + 512],
                                 start=(k == 0), stop=(k == KT - 1))
        srowA = sqp.tile([1, 1024], F32, tag="srowA", bufs=1)
        for n in range(2):
            nc.scalar.activation(srowA[:, 512 * n:512 * n + 512], p_ss[n][:],
                                 AF.Sqrt, bias=eps1[0:1, :], scale=1.0 / D)
        nc.vector.reciprocal_approx_fast(srowA[:], srowA[:])
        sbcA = sqp.tile([128, 1024], F32, tag="sbcA", bufs=1)
        nc.gpsimd.partition_broadcast(sbcA[:], srowA[:])
        for k in range(KT):
            nc.vector.tensor_mul(hT[:, 1024 * k:1024 * k + 1024], xts[k][:], sbcA[:])
        _rel(sqp)
        _rel(stA)
        if DUMP == "hT":
            dump_y([(k, hT[:, 1024 * k:1024 * k + 1024]) for k in range(KT)])

        # ============ Phase B ============
        if "B" not in PHASES:
            raise _SkipRest()
        dk = _pool(name="dk", bufs=1)
        pb = _pool(name="pb", bufs=6)

        def conv_acc(pre, cw, j):
            acc = pb.tile([128, 1024], F32, tag="s1k")
            nc.scalar.activation(acc[:], pre[:], AF.Copy, scale=cw[:, 4 * j + 3:4 * j + 4])
            for s in (1, 2, 3):
                tmp = pb.tile([128, 1024], F32, tag="s1k")
                if s == 2:
                    nc.scalar.activation(tmp[:, 0:1024 - s], pre[:, 0:1024 - s],
                                         AF.Copy, scale=cw[:, 4 * j + 3 - s:4 * j + 4 - s])
                else:
                    nc.vector.tensor_scalar_mul(tmp[:, 0:1024 - s], pre[:, 0:1024 - s],
                                                cw[:, 4 * j + 3 - s:4 * j + 4 - s])
                nc.vector.tensor_add(acc[:, s:1024], acc[:, s:1024], tmp[:, 0:1024 - s])
            return acc

        def qkv_pass(w_dram, cw, eps_col, mult, kind, jbase, wcol0):
            pps = [[pst() for n in range(2)] for j in range(4)]
            for k in range(KT):
                wt = wp.tile([128, 512], BF16, tag="wwide")
                nc.sync.dma_start(wt[:], w_dram[128 * k:128 * k + 128, wcol0:wcol0 + 512])
                for j in range(4):
                    for n in range(2):
                        nc.tensor.matmul(
                            pps[j][n][:], wt[:, 128 * j:128 * j + 128],
                            hT[:, 1024 * k + 512 * n:1024 * k + 512 * n + 512],
                            start=(k == 0), stop=(k == KT - 1))
            for j in range(4):
                jj = jbase + j
                pre = pb.tile([128, 1024], F32, tag="s1k")
                for n in range(2):
                    nc.vector.tensor_copy(pre[:, 512 * n:512 * n + 512], pps[j][n][:])
                acc = conv_acc(pre, cw, jj)
                if kind == "v":
                    vb = pb.tile([128, 1024], BF16, tag="vb16", bufs=2)
                    nc.scalar.activation(vb[:], acc[:], AF.Silu)
                    for ci in range(8):
                        pv = pst(128, 96, BF16)
                        nc.tensor.transpose(pv[:], vb[0:96, 128 * ci:128 * ci + 128],
                                            idnb[0:96, 0:96])
                        nc.scalar.copy(
                            vtok[:, VD_C * ci + 96 * jj:VD_C * ci + 96 * jj + 96], pv[:])
                else:
                    blk = pb.tile([128, 1024], F32, tag="s1k")
                    nc.scalar.activation(blk[:], acc[:], AF.Silu)
                    sq = pb.tile([128, 1024], BF16, tag="sqb", bufs=2)
                    nc.vector.tensor_mul(sq[:], blk[:], blk[:])
                    dst = qsb if kind == "q" else ksb
                    for hh, rh in ((0, 0), (1, 64)):
                        srow = pb.tile([1, 1024], F32, tag="srow", bufs=2)
                        for n2 in range(2):
                            p_ssq = pst(1, 512)
                            nc.tensor.matmul(
                                p_ssq[:], on48[:, hh:hh + 1], sq[:, 512 * n2:512 * n2 + 512],
                                start=True, stop=True)
                            nc.scalar.activation(srow[:, 512 * n2:512 * n2 + 512], p_ssq[:],
                                                 AF.Sqrt, bias=eps_col[0:1, :], scale=mult)
                        nc.vector.reciprocal_approx_fast(srow[:], srow[:])
                        sbc = pb.tile([128, 1024], F32, tag="sbc", bufs=2)
                        nc.gpsimd.partition_broadcast(sbc[:], srow[:])
                        nc.vector.tensor_mul(dst[rh:rh + 48, 1024 * jj:1024 * jj + 1024],
                                             blk[rh:rh + 48, :], sbc[rh:rh + 48, :])
                    if kind == "k":
                        for ci in range(8):
                            pk = pst(128, 128, BF16)
                            nc.tensor.transpose(
                                pk[:], ksb[:, 1024 * jj + 128 * ci:1024 * jj + 128 * ci + 128],
                                idnb[:])
                            nc.scalar.copy(
                                ktok[:, 512 * ci + 128 * jj:512 * ci + 128 * jj + 128], pk[:])

        qkv_pass(wq_d, cqw, epsq, 48.0, "q", 0, 0)
        qkv_pass(wk_d, ckw, epsk, 1.0, "k", 0, 0)
        qkv_pass(wv_d, cvw, None, None, "v", 0, 0)
        qkv_pass(wv_d, cvw, None, None, "v", 4, 512)
        if DUMP == "qkv":
            dump_y([(j, qsb[:, 1024 * j:1024 * j + 1024]) for j in range(4)]
                   + [(4 + j, ksb[:, 1024 * j:1024 * j + 1024]) for j in range(4)]
                   + [(8 + b, vtok[:, 1024 * b:1024 * b + 1024]) for b in range(6)]
                   + [(14 + b, ktok[:, 1024 * b:1024 * b + 1024]) for b in range(2)])

        # gate token-major
        for n in range(2):
            pgs = [pst(128, 384) for _ in range(NTOK)]
            for k in range(KT):
                wt = wp.tile([128, 384], BF16, tag="wg384")
                nc.sync.dma_start(wt[:], wg_d[128 * k:128 * k + 128, 384 * n:384 * n + 384])
                for i in range(NTOK):
                    nc.tensor.matmul(
                        pgs[i][:], hT[:, 1024 * k + 128 * i:1024 * k + 128 * i + 128], wt[:],
                        start=(k == 0), stop=(k == KT - 1))
            for i in range(NTOK):
                nc.scalar.activation(
                    g_tok[:, VD_C * i + 384 * n:VD_C * i + 384 * n + 384], pgs[i][:], AF.Silu)

        # a/b projections, feature-major [16, 1024]
        ppab = [pst(16, 512) for _ in range(2)]
        for k in range(KT):
            wt = wp.tile([128, 16], BF16, tag="wab")
            nc.sync.dma_start(wt[:], wab_d[128 * k:128 * k + 128, :])
            for n in range(2):
                nc.tensor.matmul(ppab[n][:], wt[:], hT[:, 1024 * k + 512 * n:1024 * k + 512 * n + 512],
                                 start=(k == 0), stop=(k == KT - 1))
        for n in range(2):
            nc.vector.tensor_copy(ab_fm[:, 512 * n:512 * n + 512], ppab[n][:])

        # -------- batched decay prep for all chunks --------
        gta = dk.tile([128, 64], F32)
        bta = dk.tile([128, 64], F32)
        for ci in range(8):
            p_ab = pst(128, 16)
            nc.tensor.transpose(p_ab[:], ab_fm[:, 128 * ci:128 * ci + 128], idn[0:16, 0:16])
            nc.vector.tensor_copy(gta[:, 8 * ci:8 * ci + 8], p_ab[:, 0:8])
            nc.vector.tensor_copy(bta[:, 8 * ci:8 * ci + 8], p_ab[:, 8:16])
        nc.vector.tensor_add(gta[:], gta[:], dtb_bc[:])
        nc.scalar.activation(gta[:], gta[:], AF.Exp)
        nc.vector.tensor_scalar_add(gta[:], gta[:], 1.0)
        nc.scalar.activation(gta[:], gta[:], AF.Ln)
        nc.vector.tensor_mul(gta[:], gta[:], nega_bc[:])        # gt_all [128,64]
        beta_all = dk.tile([128, 64], F32)
        nc.scalar.activation(beta_all[:], bta[:], AF.Sigmoid)
        nbeta_all = dk.tile([128, 64], F32)
        nc.vector.tensor_scalar_mul(nbeta_all[:], beta_all[:], -1.0)
        p_bc = pst(128, 64)
        nc.tensor.matmul(p_bc[:], cum[:], gta[:], start=True, stop=True)
        bcum_tok = dk.tile([128, 64], F32)
        nc.vector.tensor_copy(bcum_tok[:], p_bc[:])
        lam_all = dk.tile([128, 64], F32)
        nc.scalar.activation(lam_all[:], p_bc[:], AF.Exp)
        p_bf = pst(64, 128)
        nc.tensor.transpose(p_bf[:], bcum_tok[:], idn[:])
        b_fm = dk.tile([64, 128], F32)
        nc.vector.tensor_copy(b_fm[:], p_bf[:])
        nc.scalar.dma_start(bfm_scr[:], b_fm[:])
        wfm = dk.tile([64, 128], F32)
        nc.vector.tensor_scalar(wfm[:], b_fm[:], b_fm[:, 127:128], None, OP.subtract)
        nc.scalar.activation(wfm[:], wfm[:], AF.Exp, scale=-1.0)
        p_wt = pst(128, 64)
        nc.tensor.transpose(p_wt[:], wfm[:], idn[0:64, 0:64])
        w_tok = dk.tile([128, 64], F32)
        nc.vector.tensor_copy(w_tok[:], p_wt[:])
        ebc_all = dk.tile([64, 1], F32)
        nc.scalar.activation(ebc_all[:], b_fm[:, 127:128], AF.Exp)
        # EB[8ci+h, ci] = ebc_all[8ci+h]; ebcJ[j][p, ci] = per-(ci,j) state-decay col
        EB = dk.tile([64, 8], F32)
        nc.vector.tensor_scalar_mul(EB[:], chk[:], ebc_all[:, 0:1])
        ebcJ = []
        for j in range(4):
            p_ebj = pst(128, 8)
            nc.tensor.matmul(p_ebj[:], selj[:, 128 * j:128 * j + 128], EB[:],
                             start=True, stop=True)
            ej = dk.tile([128, 8], F32, tag=f"ebj{j}", name=f"ebj{j}")
            nc.vector.tensor_copy(ej[:], p_ebj[:])
            ebcJ.append(ej)
        _rel(pb)

        # ============ Phase C ============
        if "C" not in PHASES:
            raise _SkipRest()
        wod = _pool(name="wod", bufs=16)
        pd = _pool(name="pd", bufs=8)
        dput = _pool(name="dput", bufs=48)
        dpa = _pool(name="dpa", bufs=10)
        dpx = _pool(name="dpx", bufs=10)
        dpf = _pool(name="dpf", bufs=6)
        dp2 = _pool(name="dp2", bufs=2)
        spool = _pool(name="spool", bufs=2)

        def emit_D(half):
            t0 = 512 * half
            for db in range(16):
                wt = wod.tile([128, VD_C], BF16, tag="wo", name="wo_t", bufs=16)
                nc.scalar.dma_start(wt[:], wo_d[:, VD_C * db:VD_C * db + VD_C])
                pp = pst()
                for fb in range(6):
                    nc.tensor.matmul(pp[:], wt[:, 128 * fb:128 * fb + 128],
                                     osb[:, 1024 * fb + t0:1024 * fb + t0 + 512],
                                     start=(fb == 0), stop=(fb == 5))
                stg = pd.tile([128, 512], BF16, tag="s512b", name="stg", bufs=8)
                nc.scalar.copy(stg[:], pp[:])
                nc.sync.dma_start(oT_in[half][128 * db:128 * db + 128, :], stg[:])
            nc.gpsimd.collective_compute(
                "AllReduce", OP.add, ins=[oT_in[half][:]], outs=[oT_out[half][:]],
                replica_groups=groups)

        S_cur = {}
        for j in range(4):
            S_cur[j] = spool.tile([128, DV], F32, tag=f"s{j}", name=f"s{j}")
            nc.vector.memset(S_cur[j][:], 0.0)

        for ci in range(NCH):
            # ---- prep all 8 heads: abar, xx, xt ----
            ABAR, XX, XT = {}, {}, {}
            for j in range(4):
                for hh in range(2):
                    h = 2 * j + hh
                    rh = 64 * hh
                    kts = ksb[rh:rh + 48, 1024 * j + 128 * ci:1024 * j + 128 * ci + 128]
                    qts = qsb[rh:rh + 48, 1024 * j + 128 * ci:1024 * j + 128 * ci + 128]
                    p_kk = pst(128, 128)
                    nc.tensor.matmul(p_kk[:], kts, kts, start=True, stop=True)
                    p_kq = pst(128, 128)
                    nc.tensor.matmul(p_kq[:], kts, qts, start=True, stop=True)
                    bc128 = dpf.tile([128, 128], F32, tag="bc", name="bc128")
                    nc.gpsimd.dma_start(
                        bc128[:],
                        bfm_scr[8 * ci + h:8 * ci + h + 1, :].to_broadcast((128, 128)))
                    dte = dpf.tile([128, 128], F32, tag="dte", name="dte")
                    nc.vector.tensor_scalar(dte[:], bc128[:],
                                            bcum_tok[:, 8 * ci + h:8 * ci + h + 1],
                                            None, OP.subtract)
                    nc.vector.tensor_add(dte[:], dte[:], negl[:])
                    dincl = dput.tile([128, 128], BF16, tag="ut", name="dincl")
                    nc.scalar.activation(dincl[:], dte[:], AF.Exp)
                    abar = dpa.tile([128, 128], BF16, tag="abar", name="abar")
                    nc.vector.tensor_mul(abar[:], p_kq[:], dincl[:])
                    dstr = dput.tile([128, 128], BF16, tag="ut", name="dstr")
                    nc.vector.tensor_mul(dstr[:], dincl[:], mstb[:])
                    x0 = dput.tile([128, 128], BF16, tag="ut", name="x0")
                    nc.vector.tensor_mul(x0[:], p_kk[:], dstr[:])
                    xx = dpx.tile([128, 128], BF16, tag="xx", name="xx")
                    nc.vector.tensor_scalar_mul(xx[:], x0[:],
                                                nbeta_all[:, 8 * ci + h:8 * ci + h + 1])
                    p_x = pst(128, 128, BF16)
                    nc.tensor.transpose(p_x[:], xx[:], idnb[:])
                    xt = dpx.tile([128, 128], BF16, tag="xt", name="xt")
                    nc.vector.tensor_copy(xt[:], p_x[:])
                    ABAR[h], XX[h], XT[h] = abar, xx, xt

            # ---- UT inverse, level-major across all 8 heads ----
            PM = {}
            for h in range(HP):
                pmat = dput.tile([128, 128], BF16, tag="ut", name="pmat")
                nc.vector.tensor_add(pmat[:], XX[h][:], idnb[:])
                PM[h] = pmat
            cur = {h: (XX[h], XT[h]) for h in range(HP)}
            for lvl in range(6):
                last = lvl == 5
                nxt = {}
                for h in range(HP):
                    xx, xt = cur[h]
                    x2 = None
                    if not last:
                        p_sq = pst(128, 128)
                        nc.tensor.matmul(p_sq[:], xt[:], xx[:], start=True, stop=True)
                        x2 = dput.tile([128, 128], BF16, tag="ut", name="x2")
                        nc.scalar.copy(x2[:], p_sq[:])
                    p_sqt = pst(128, 128)
                    nc.tensor.matmul(p_sqt[:], xx[:], xt[:], start=True, stop=True)
                    xt2 = dput.tile([128, 128], BF16, tag="ut", name="xt2")
                    if h % 2 == 0:
                        nc.scalar.copy(xt2[:], p_sqt[:])
                    else:
                        nc.vector.tensor_copy(xt2[:], p_sqt[:])
                    nxt[h] = (x2, xt2)
                for h in range(HP):
                    p_pr = pst(128, 128)
                    nc.tensor.matmul(p_pr[:], nxt[h][1][:], PM[h][:], start=True, stop=True)
                    pnew = dput.tile([128, 128], BF16, tag="ut", name="pnew")
                    if h % 2 == 0:
                        nc.vector.tensor_add(pnew[:], PM[h][:], p_pr[:])
                    else:
                        nc.vector.tensor_add(pnew[:], p_pr[:], PM[h][:])
                    PM[h] = pnew
                if not last:
                    cur = nxt

            # ---- state/output, step-major in 4-head waves ----
            ot_all = dp2.tile([128, VD_C], F32, tag="otall", name="ot_all")
            for jp in (0, 2):
                heads = [(j, hh) for j in (jp, jp + 1) for hh in (0, 1)]
                SB, PS, KW = {}, {}, {}
                for j in (jp, jp + 1):
                    S_bf = dp2.tile([128, DV], BF16, tag=f"sbf{j}", name="S_bf")
                    nc.vector.tensor_copy(S_bf[:], S_cur[j][:])
                    SB[j] = S_bf
                    PS[j] = pst(128, DV)
                for j, hh in heads:
                    h = 2 * j + hh
                    rh = 64 * hh
                    kw = dp2.tile([128, 48], BF16, tag="kw", name="kw", bufs=6)
                    nc.vector.tensor_scalar_mul(
                        kw[:], ktok[:, 512 * ci + 128 * j + rh:512 * ci + 128 * j + rh + 48],
                        w_tok[:, 8 * ci + h:8 * ci + h + 1])
                    KW[h] = kw
                RR, UU = {}, {}
                for j, hh in heads:
                    h = 2 * j + hh
                    rh = 64 * hh
                    kts = ksb[rh:rh + 48, 1024 * j + 128 * ci:1024 * j + 128 * ci + 128]
                    p_ks = pst(128, DV)
                    nc.tensor.matmul(p_ks[:], kts, SB[j][rh:rh + 48, :], start=True, stop=True)
                    r_ = dp2.tile([128, DV], BF16, tag="rr", name="r_", bufs=5)
                    nc.vector.tensor_scalar_mul(r_[:], p_ks[:],
                                                lam_all[:, 8 * ci + h:8 * ci + h + 1])
                    nc.vector.tensor_sub(
                        r_[:], vtok[:, VD_C * ci + 96 * h:VD_C * ci + 96 * h + 96], r_[:])
                    RR[h] = r_
                for j, hh in heads:
                    h = 2 * j + hh
                    p_w = pst(128, DV)
                    nc.tensor.matmul(p_w[:], PM[h][:], RR[h][:], start=True, stop=True)
                    u_ = dp2.tile([128, DV], BF16, tag="uu", name="u_", bufs=5)
                    nc.vector.tensor_scalar_mul(u_[:], p_w[:],
                                                beta_all[:, 8 * ci + h:8 * ci + h + 1])
                    UU[h] = u_
                for j, hh in heads:
                    h = 2 * j + hh
                    rh = 64 * hh
                    qts = qsb[rh:rh + 48, 1024 * j + 128 * ci:1024 * j + 128 * ci + 128]
                    p_oi = pst(128, DV)
                    nc.tensor.matmul(p_oi[:], ABAR[h][:], UU[h][:], start=True, stop=True)
                    p_qs = pst(128, DV)
                    nc.tensor.matmul(p_qs[:], qts, SB[j][rh:rh + 48, :], start=True, stop=True)
                    ots = ot_all[:, 96 * h:96 * h + 96]
                    nc.vector.tensor_scalar_mul(ots, p_qs[:],
                                                lam_all[:, 8 * ci + h:8 * ci + h + 1])
                    nc.vector.tensor_add(ots, ots, p_oi[:])
                    nc.tensor.matmul(PS[j][rh:rh + 48, :], KW[h][:], UU[h][:],
                                     start=True, stop=True)
                for j in (jp, jp + 1):
                    s_new = spool.tile([128, DV], F32, tag=f"s{j}", name="s_new")
                    for rh2 in (0, 64):
                        nc.vector.tensor_scalar_mul(
                            s_new[rh2:rh2 + 48, :], S_cur[j][rh2:rh2 + 48, :],
                            ebcJ[j][rh2:rh2 + 48, ci:ci + 1])
                        nc.vector.tensor_add(
                            s_new[rh2:rh2 + 48, :], s_new[rh2:rh2 + 48, :],
                            PS[j][rh2:rh2 + 48, :])
                    S_cur[j] = s_new

            # ---- batched gated rmsnorm + gate + transpose to osb ----
            osq = dp2.tile([128, VD_C], F32, tag="osq", name="osq")
            nc.vector.tensor_mul(osq[:], ot_all[:], ot_all[:])
            rcol8 = dp2.tile([128, 8], F32, tag="rc8", name="rcol8")
            for h in range(HP):
                nc.vector.tensor_reduce(rcol8[:, h:h + 1], osq[:, 96 * h:96 * h + 96],
                                        mybir.AxisListType.X, OP.add)
            nc.scalar.activation(rcol8[:], rcol8[:], AF.Sqrt, bias=epsg[:], scale=1.0 / DV)
            nc.vector.reciprocal_approx_fast(rcol8[:], rcol8[:])
            for h in range(HP):
                nc.vector.tensor_scalar_mul(ot_all[:, 96 * h:96 * h + 96],
                                            ot_all[:, 96 * h:96 * h + 96], rcol8[:, h:h + 1])
            ob = dp2.tile([128, VD_C], BF16, tag="ob", name="ob")
            nc.vector.tensor_mul(ob[:], ot_all[:], g_tok[:, VD_C * ci:VD_C * ci + VD_C])
            for b6 in range(6):
                p_ot = pst(128, 128, BF16)
                nc.tensor.transpose(p_ot[:], ob[:, 128 * b6:128 * b6 + 128], idnb[:])
                nc.scalar.copy(osb[:, 1024 * b6 + 128 * ci:1024 * b6 + 128 * ci + 128], p_ot[:])

            # ---- o_projT half + AllReduce as soon as its tokens exist ----
            if ci == 3:
                emit_D(0)
            if ci == 7:
                emit_D(1)

        for p in (spool, dp2, dpf, dpx, dpa, dput, pd, wod):
            _rel(p)
        if DUMP == "o":
            dump_y([(b, osb[:, 1024 * b:1024 * b + 1024]) for b in range(6)]
                   + [(6 + b, g_tok[:, 1024 * b:1024 * b + 1024]) for b in range(6)]
                   + [(12 + b, ktok[:, 1024 * b:1024 * b + 1024]) for b in range(4)])
        if DUMP == "dk":
            dump_y([(0, gta[:]), (1, beta_all[:]), (2, bcum_tok[:]),
                    (3, lam_all[:]), (4, w_tok[:]), (5, b_fm[:]),
                    (6, ebc_all[:]), (7, ebcJ[0][:]), (8, ebcJ[3][:])])
        _rel(dk)
        _rel(bigq)

        if "D" not in PHASES:
            raise _SkipRest()
        if DUMP in ("ar", "oin"):
            src = oT_out if DUMP == "ar" else oT_in
            dbg = _pool(name="dbg", bufs=4)
            for bi in range(16):
                st = dbg.tile([128, 1024], F32, tag="dbg", name="st")
                for half in range(2):
                    so = dbg.tile([128, 512], BF16, tag="dbg2", name="so")
                    nc.sync.dma_start(so[:], src[half][128 * bi:128 * bi + 128, :])
                    nc.vector.tensor_copy(st[:, 512 * half:512 * half + 512], so[:])
                nc.sync.dma_start(y_d[128 * bi:128 * bi + 128, :], st[:])
            _rel(dbg)
            raise _Dumped()

        # ============ Phase E ============
        if "E" not in PHASES:
            raise _SkipRest()
        seq = _pool(name="seq", bufs=3)
        pe = _pool(name="pe", bufs=4)
        ffT = hT
        p_s2 = [pst(1, 512) for _ in range(2)]
        for half in range(2):
            hs = slice(512 * half, 512 * half + 512)
            for k in range(KT):
                xe = seq.tile([128, 512], F32, tag="xe")
                nc.sync.dma_start(xe[:], xT_d[128 * k:128 * k + 128, hs])
                oe = seq.tile([128, 512], BF16, tag="oe")
                nc.gpsimd.dma_start(oe[:], oT_out[half][128 * k:128 * k + 128, :])
                h2 = seq.tile([128, 512], F32, tag="h2T")
                nc.vector.tensor_add(h2[:], xe[:], oe[:])
                nc.scalar.dma_start(h2T_scr[128 * k:128 * k + 128, hs], h2[:])
                sqe = seq.tile([128, 512], BF16, tag="sqe")
                nc.vector.tensor_mul(sqe[:], h2[:], h2[:])
                nc.tensor.matmul(p_s2[half][:], ones1[:], sqe[:],
                                 start=(k == 0), stop=(k == KT - 1))
        srowE = seq.tile([1, 1024], F32, tag="srowE", bufs=1)
        for n in range(2):
            nc.scalar.activation(srowE[:, 512 * n:512 * n + 512], p_s2[n][:],
                                 AF.Sqrt, bias=eps1[0:1, :], scale=1.0 / D)
        nc.vector.reciprocal_approx_fast(srowE[:], srowE[:])
        sbcE = seq.tile([128, 1024], F32, tag="sbcE", bufs=1)
        nc.gpsimd.partition_broadcast(sbcE[:], srowE[:])
        for k in range(KT):
            h2r = seq.tile([128, 1024], F32, tag="h2r", bufs=4)
            nc.sync.dma_start(h2r[:], h2T_scr[128 * k:128 * k + 128, :])
            nc.vector.tensor_mul(ffT[:, 1024 * k:1024 * k + 1024], h2r[:], sbcE[:])
        if DUMP == "ffT":
            dump_y([(k, ffT[:, 1024 * k:1024 * k + 1024]) for k in range(KT)])

        mida = pg.tile([128, 6 * 1024], BF16, tag="gtok")
        pmid = _pool(name="pmid", bufs=1)
        midb = pmid.tile([128, 5 * 1024], BF16, tag="midb")

        def mid_ap(m, off, ln):
            if m < 6:
                return mida[:, 1024 * m + off:1024 * m + off + ln]
            return midb[:, 1024 * (m - 6) + off:1024 * (m - 6) + off + ln]

        wp13 = _pool(name="wp13", bufs=3)
        for m in range(11):
            pu1 = [pst() for _ in range(2)]
            pu3 = [pst() for _ in range(2)]
            wt13 = wp13.tile([128, 4096], BF16, tag="w13", name="wt13")
            nc.sync.dma_start(wt13[:], w13_d[:, 4096 * m:4096 * m + 4096])
            for k in range(KT):
                for n in range(2):
                    rhs = ffT[:, 1024 * k + 512 * n:1024 * k + 512 * n + 512]
                    nc.tensor.matmul(pu1[n][:], wt13[:, 256 * k:256 * k + 128], rhs,
                                     start=(k == 0), stop=(k == KT - 1))
                    nc.tensor.matmul(pu3[n][:], wt13[:, 256 * k + 128:256 * k + 256], rhs,
                                     start=(k == 0), stop=(k == KT - 1))
            for n in range(2):
                u1s = pe.tile([128, 512], F32, tag="s512", name="u1s")
                nc.scalar.activation(u1s[:], pu1[n][:], AF.Silu)
                nc.vector.tensor_mul(mid_ap(m, 512 * n, 512), u1s[:], pu3[n][:])

        if DUMP == "mid":
            dump_y([(m, mid_ap(m, 0, 1024)) for m in range(11)])
        wp2 = _pool(name="wp2", bufs=3)
        for db in range(16):
            wt2 = wp2.tile([128, INT_C], BF16, tag="w2", name="w2_t")
            nc.sync.dma_start(wt2[:], w2_d[:, INT_C * db:INT_C * db + INT_C])
            for half in range(2):
                pps = pst()
                for m in range(11):
                    nc.tensor.matmul(pps[:], wt2[:, 128 * m:128 * m + 128],
                                     mid_ap(m, 512 * half, 512),
                                     start=(m == 0), stop=(m == 10))
                h2t = pe.tile([128, 512], F32, tag="s512", name="h2t")
                nc.sync.dma_start(h2t[:], h2T_scr[128 * db:128 * db + 128,
                                                  512 * half:512 * half + 512])
                yst = pe.tile([128, 512], F32, tag="s512", name="yst")
                nc.vector.tensor_scalar_mul(yst[:], h2t[:], 0.25)
                nc.vector.tensor_add(yst[:], yst[:], pps[:])
                nc.sync.dma_start(y_d[128 * db:128 * db + 128, 512 * half:512 * half + 512],
                                  yst[:])

        for p in (wp2, wp13, pmid, pe, seq, dram, wp, pg, big, ps, cpool):
            _rel(p)
      except _SkipRest:
        zst = _pool(name="zst", bufs=1)
        zt = zst.tile([128, 512], F32)
        nc.vector.memset(zt[:], 0.0)
        for i in range(16):
            for dh in range(2):
                nc.sync.dma_start(y_d[128 * i:128 * i + 128, 512 * dh:512 * dh + 512], zt[:])
        for p in reversed(live_pools):
            p.release()
      except _Dumped:
        for p in reversed(live_pools):
            p.release()

    nc.compile()
    return nc


def _pack_wo(wo):
    # [768, 2048] -> [128, 16*768]: col = 768*db + 128*fb + c
    return np.ascontiguousarray(
        wo.reshape(6, 128, 16, 128).transpose(1, 2, 0, 3).reshape(128, 16 * 768))


def _pack_w13(w1, w3):
    # [2048, 1408] x2 -> [128, 11*4096]: col = 4096*m + 256*k + 128*which + c
    a = w1.reshape(16, 128, 11, 128).transpose(1, 2, 0, 3)   # [128, 11, 16, 128]
    b = w3.reshape(16, 128, 11, 128).transpose(1, 2, 0, 3)
    return np.ascontiguousarray(
        np.stack([a, b], axis=3).reshape(128, 11 * 4096))


def _pack_w2(w2):
    # [1408, 2048] -> [128, 16*1408]: col = 1408*db + 128*m + c
    return np.ascontiguousarray(
        w2.reshape(11, 128, 16, 128).transpose(1, 2, 0, 3).reshape(128, 16 * 1408))


def _shard(inputs):
    f32 = np.float32
    bf = ml_dtypes.bfloat16
    rms1 = np.asarray(inputs["rms1_w"], f32)
    rms2 = np.asarray(inputs["rms2_w"], f32)
    gn = np.asarray(inputs["gnorm_w"], f32)
    in_maps = []
    for c in range(8):
        g, m = c // 4, c % 4
        qs = slice(384 * m, 384 * m + 384)
        vs = slice(768 * m, 768 * m + 768)
        hs = slice(8 * m, 8 * m + 8)
        isl = slice(1408 * m, 1408 * m + 1408)

        def padqk(w):
            wp_ = np.zeros((D, QKP), f32)
            for h in range(8):
                wp_[:, 64 * h:64 * h + 48] = w[:, 48 * h:48 * h + 48]
            return wp_

        def padcw(w):
            cp = np.zeros((QKP, 4), f32)
            for h in range(8):
                cp[64 * h:64 * h + 48] = w[48 * h:48 * h + 48]
            return cp

        def padv(w):
            colpad = w.shape[0] == D
            out = np.zeros((D, VP) if colpad else (VP, w.shape[1]), f32)
            for h in range(8):
                if colpad:
                    out[:, 128 * h:128 * h + 96] = w[:, 96 * h:96 * h + 96]
                else:
                    out[128 * h:128 * h + 96] = w[96 * h:96 * h + 96]
            return out

        dtb8 = np.asarray(inputs["dt_bias"], f32)[hs]
        nega8 = -np.exp(np.asarray(inputs["A_log"], f32)[hs])
        in_maps.append(dict(
            xT=np.ascontiguousarray(np.asarray(inputs["x"], f32)[g].T),
            wq=padqk(np.asarray(inputs["Wq"], f32)[:, qs] * rms1[:, None]).astype(bf),
            wk=padqk(np.asarray(inputs["Wk"], f32)[:, qs] * rms1[:, None]).astype(bf),
            wv=padv(np.asarray(inputs["Wv"], f32)[:, vs] * rms1[:, None]).astype(bf),
            wg=np.ascontiguousarray(
                np.asarray(inputs["Wg"], f32)[:, vs] * rms1[:, None]).astype(bf),
            wab=np.ascontiguousarray(np.concatenate(
                [np.asarray(inputs["Wa"], f32)[:, hs],
                 np.asarray(inputs["Wb"], f32)[:, hs]], 1) * rms1[:, None]).astype(bf),
            cq=padcw(np.asarray(inputs["conv_q_w"], f32)[qs]),
            ck=padcw(np.asarray(inputs["conv_k_w"], f32)[qs]),
            cv=padv(np.asarray(inputs["conv_v_w"], f32)[vs]),
            dtb=np.tile(dtb8, 8).reshape(1, 64).copy(),
            nega=np.tile(nega8, 8).reshape(1, 64).copy(),
            wo=_pack_wo(np.asarray(inputs["Wo"], f32)[vs] * np.tile(gn, 8)[:, None]).astype(bf),
            w13=_pack_w13(np.asarray(inputs["W1"], f32)[:, isl] * rms2[:, None],
                          np.asarray(inputs["W3"], f32)[:, isl] * rms2[:, None]).astype(bf),
            w2=_pack_w2(np.asarray(inputs["W2"], f32)[isl]).astype(bf),
        ))
    return in_maps


def kernel(**inputs):
    if "nc" not in _cache:
        _cache["nc"] = _build(8)
    res = run_bass_kernel_spmd(_cache["nc"], _shard(inputs), list(range(8)))
    out = np.zeros((B, T, D), np.float32)
    for g in range(2):
        yT = sum(res.results[4 * g + m]["y"] for m in range(4))
        out[g] = yT.T
    return out


# revision 47
# speedup vs baseline: 1.8984x; 1.0323x over previous
"""GatedDeltaNet block kernel for 8 Trainium2 cores (Bass/Tile), bf16 rework.

Sharding: DP2 (batch) x TP4 (heads / MLP-inter). Core c: group g=c//4 runs
batch g; member m=c%4 owns heads [8m,8m+8), q/k cols [384m,..), v/g cols
[768m,..), INTER [1408m,..). Two half-token AllReduces per 4-core group after
o_proj (overlapped with o_proj compute); final down-proj partials summed on
the host.

Everything runs feature-major (host passes x transposed, takes y transposed)
so there are no PE transposes outside the delta-rule inner loop. All big
GEMM operands are bf16 (host-cast weights); psum accumulation, the delta-rule
state, decay/beta math and norms stay fp32.

Per-core dataflow:
  A: xT [D,T] -> rmsnorm via matmul-accumulated column sumsq -> hT bf16 (SBUF)
  B: bf16 projections off hT; q/k feature-major (heads padded to 64 rows)
     -> conv+silu+l2norm -> SBUF (+ token-major copies of k, v via PE
     transposes); gate token-major; a/b -> batched decay prep for all chunks
  C: chunked gated delta rule (C=128, UT transform via log-doubling inverse,
     bf16 matmuls / fp32 state); writes normed+gated o feature-major to SBUF
  D: o_projT in two token halves, each followed by its AllReduce (overlapped)
  E: h2T = xT + oT; rmsnorm -> ffT bf16 (reuses hT); MLP bf16; yT partials
"""
import sys
sys.path.insert(0, '/opt/trn_rl_repo')
import numpy as np
import ml_dtypes

import concourse.bass as bass
import concourse.bacc as bacc
import concourse.mybir as mybir
import concourse.tile as tile
from concourse.bass_isa import ReduceOp
from concourse.bass_utils import run_bass_kernel_spmd

F32 = mybir.dt.float32
BF16 = mybir.dt.bfloat16
AF = mybir.ActivationFunctionType
OP = mybir.AluOpType

B, T, D = 2, 1024, 2048
H, DK, DV = 32, 48, 96
HP = 8
QKP = 512
VD_C = 768
VP = 1024
INT_C = 1408
C = 128
NCHUNK = T // C
KT = D // 128
NTOK = T // 128

_cache = {}
import os
PHASES = os.environ.get("DN_PHASES", "ABCDE")
NCH = int(os.environ.get("DN_NCHUNK", str(T // C)))
DUMP = os.environ.get("DN_DUMP", "")


class _SkipRest(Exception):
    pass


class _Dumped(Exception):
    pass


def _build(n_cores=8):
    groups = [[0, 1, 2, 3], [4, 5, 6, 7]] if n_cores == 8 else [[0]]
    nc = bacc.Bacc("TRN2", target_bir_lowering=False, debug=False, num_devices=n_cores)

    xT_d = nc.dram_tensor("xT", [D, T], F32, kind="ExternalInput")
    wq_d = nc.dram_tensor("wq", [D, QKP], BF16, kind="ExternalInput")
    wk_d = nc.dram_tensor("wk", [D, QKP], BF16, kind="ExternalInput")
    wv_d = nc.dram_tensor("wv", [D, VP], BF16, kind="ExternalInput")
    wg_d = nc.dram_tensor("wg", [D, VD_C], BF16, kind="ExternalInput")
    wab_d = nc.dram_tensor("wab", [D, 16], BF16, kind="ExternalInput")
    cq_d = nc.dram_tensor("cq", [QKP, 4], F32, kind="ExternalInput")
    ck_d = nc.dram_tensor("ck", [QKP, 4], F32, kind="ExternalInput")
    cv_d = nc.dram_tensor("cv", [VP, 4], F32, kind="ExternalInput")
    dtb_d = nc.dram_tensor("dtb", [1, 64], F32, kind="ExternalInput")
    nega_d = nc.dram_tensor("nega", [1, 64], F32, kind="ExternalInput")
    wo_d = nc.dram_tensor("wo", [128, 16 * VD_C], BF16, kind="ExternalInput")
    w13_d = nc.dram_tensor("w13", [128, 11 * 4096], BF16, kind="ExternalInput")
    w2_d = nc.dram_tensor("w2", [128, 16 * INT_C], BF16, kind="ExternalInput")
    y_d = nc.dram_tensor("y", [D, T], F32, kind="ExternalOutput")

    ones = np.ones((128, 128), np.float32)
    idn_c = nc.inline_tensor(np.eye(128, dtype=np.float32), "idn_c")
    idnb_c = nc.inline_tensor(np.eye(128, dtype=ml_dtypes.bfloat16), "idnb_c")
    cum_c = nc.inline_tensor(np.triu(ones).copy(), "cum_c")
    mst_c = nc.inline_tensor(np.triu(ones, 1).astype(ml_dtypes.bfloat16), "mst_c")
    msi_c = nc.inline_tensor(np.triu(ones).copy(), "msi_c")
    negl_c = nc.inline_tensor((np.tril(ones, -1) * -1e30).copy(), "negl_c")
    # SELJ[r, 128j+p] = 1 iff (r%8==2j and p<48) or (r%8==2j+1 and 64<=p<112)
    selj_np = np.zeros((64, 512), np.float32)
    for r in range(64):
        for j in range(4):
            if r % 8 == 2 * j:
                selj_np[r, 128 * j:128 * j + 48] = 1.0
            if r % 8 == 2 * j + 1:
                selj_np[r, 128 * j + 64:128 * j + 112] = 1.0
    selj_c = nc.inline_tensor(selj_np, "selj_c")
    # CHK[8ci+h, ci] = 1
    chk_np = np.zeros((64, 8), np.float32)
    for ci in range(8):
        chk_np[8 * ci:8 * ci + 8, ci] = 1.0
    chk_c = nc.inline_tensor(chk_np, "chk_c")
    on48_np = np.zeros((128, 2), ml_dtypes.bfloat16)
    on48_np[0:48, 0] = 1.0
    on48_np[64:112, 1] = 1.0
    on48_c = nc.inline_tensor(on48_np, "on48_c")
    ones1_np = np.ones((128, 1), ml_dtypes.bfloat16)
    ones1_c = nc.inline_tensor(ones1_np, "ones1_c")

    with tile.TileContext(nc) as tc:
      live_pools = []

      def _pool(**kw):
          p = tc.alloc_tile_pool(**kw)
          live_pools.append(p)
          return p

      def _rel(p):
          p.release()
          live_pools.remove(p)

      try:
        cpool = _pool(name="consts", bufs=1)
        ps = _pool(name="ps", bufs=8, space="PSUM")

        def pst(p=128, f=512, dt=F32):
            return ps.tile([p, f], dt, tag="ps", name="pst")

        def dump_y(items):
            # items: list of (y_block_index, ap [p, <=1024]) — copy (cast) to y
            dbg = _pool(name="dbg", bufs=4)
            for bi, ap in items:
                p, n = ap.shape[0], ap.shape[1]
                st = dbg.tile([128, 1024], F32, tag="dbg", name="st")
                nc.vector.tensor_copy(st[0:p, 0:n], ap)
                nc.sync.dma_start(y_d[128 * bi:128 * bi + p, 0:n], st[0:p, 0:n])
            _rel(dbg)
            raise _Dumped()

        idn = cpool.tile([128, 128], F32)
        idnb = cpool.tile([128, 128], BF16)
        cum = cpool.tile([128, 128], F32)
        mstb = cpool.tile([128, 128], BF16)
        msi = cpool.tile([128, 128], F32)
        negl = cpool.tile([128, 128], F32)
        selj = cpool.tile([64, 512], F32)
        chk = cpool.tile([64, 8], F32)
        on48 = cpool.tile([128, 2], BF16)
        ones1 = cpool.tile([128, 1], BF16)
        for t_, s_ in [(idn, idn_c), (idnb, idnb_c), (cum, cum_c), (mstb, mst_c),
                       (msi, msi_c), (negl, negl_c), (selj, selj_c), (chk, chk_c),
                       (on48, on48_c), (ones1, ones1_c)]:
            nc.sync.dma_start(t_[:], s_[:])
        eps1 = cpool.tile([128, 1], F32)
        nc.vector.memset(eps1[:], 1e-5)
        epsq = cpool.tile([128, 1], F32)
        nc.vector.memset(epsq[:], 48e-6)
        epsk = cpool.tile([128, 1], F32)
        nc.vector.memset(epsk[:], 1e-6)
        epsg = cpool.tile([128, 1], F32)
        nc.vector.memset(epsg[:], 1e-5)
        dtb_r = cpool.tile([1, 64], F32)
        nega_r = cpool.tile([1, 64], F32)
        nc.sync.dma_start(dtb_r[:], dtb_d[:])
        nc.sync.dma_start(nega_r[:], nega_d[:])
        dtb_bc = cpool.tile([128, 64], F32)
        nega_bc = cpool.tile([128, 64], F32)
        nc.gpsimd.partition_broadcast(dtb_bc[:], dtb_r[:])
        nc.gpsimd.partition_broadcast(nega_bc[:], nega_r[:])
        cqw = cpool.tile([128, 16], F32)
        ckw = cpool.tile([128, 16], F32)
        cvw = cpool.tile([128, 32], F32)
        for j in range(4):
            nc.sync.dma_start(cqw[:, 4 * j:4 * j + 4], cq_d[128 * j:128 * j + 128, :])
            nc.sync.dma_start(ckw[:, 4 * j:4 * j + 4], ck_d[128 * j:128 * j + 128, :])
        for j in range(8):
            nc.sync.dma_start(cvw[:, 4 * j:4 * j + 4], cv_d[128 * j:128 * j + 128, :])
        ab_fm = cpool.tile([16, 1024], F32)

        big = _pool(name="big", bufs=1)
        hT = big.tile([128, KT * 1024], BF16)       # also ffT in phase E
        osb = big.tile([128, 6 * 1024], BF16)       # feature-major o: [feat%128, 1024*(f//128)+tok]
        pg = _pool(name="pg", bufs=1)
        g_tok = pg.tile([128, NTOK * VD_C], BF16, tag="gtok")

        wp = _pool(name="wp", bufs=4)
        dram = _pool(name="dram", bufs=1, space="DRAM")
        oT_in = [dram.tile([D, 512], BF16, name=f"oT_in{i}") for i in range(2)]
        oT_out = [dram.tile([D, 512], BF16, name=f"oT_out{i}") for i in range(2)]
        h2T_scr = dram.tile([D, T], F32)
        bfm_scr = dram.tile([64, 128], F32)

        bigq = _pool(name="bigq", bufs=1)
        qsb = bigq.tile([128, 4 * 1024], BF16)
        ksb = bigq.tile([128, 4 * 1024], BF16)
        ktok = bigq.tile([128, 8 * 512], BF16)      # token-major k: [tok, 512ci+128j]
        vtok = bigq.tile([128, 8 * VD_C], BF16)     # token-major v: [tok, 768ci+96h]
        nc.vector.memset(qsb[:], 0.0)
        nc.vector.memset(ksb[:], 0.0)

        # ============ Phase A: hT = rmsnorm(x)^T in bf16 ============
        stA = _pool(name="stA", bufs=16)
        sqp = _pool(name="sqp", bufs=3)
        p_ss = [pst(1, 512) for _ in range(2)]
        xts = []
        for k in range(KT):
            xa = stA.tile([128, 1024], F32, tag="xT")
            nc.sync.dma_start(xa[:], xT_d[128 * k:128 * k + 128, :])
            xts.append(xa)
            sq = sqp.tile([128, 1024], BF16, tag="sq")
            nc.vector.tensor_mul(sq[:], xa[:], xa[:])
            for n in range(2):
                nc.tensor.matmul(p_ss[n][:], ones1[:], sq[:, 512 * n:512 * n + 512],
                                 start=(k == 0), stop=(k == KT - 1))
        srowA = sqp.tile([1, 1024], F32, tag="srowA", bufs=1)
        for n in range(2):
            nc.scalar.activation(srowA[:, 512 * n:512 * n + 512], p_ss[n][:],
                                 AF.Sqrt, bias=eps1[0:1, :], scale=1.0 / D)
        nc.vector.reciprocal_approx_fast(srowA[:], srowA[:])
        sbcA = sqp.tile([128, 1024], F32, tag="sbcA", bufs=1)
        nc.gpsimd.partition_broadcast(sbcA[:], srowA[:])
        for k in range(KT):
            nc.vector.tensor_mul(hT[:, 1024 * k:1024 * k + 1024], xts[k][:], sbcA[:])
        _rel(sqp)
        _rel(stA)
        if DUMP == "hT":
            dump_y([(k, hT[:, 1024 * k:1024 * k + 1024]) for k in range(KT)])

        # ============ Phase B ============
        if "B" not in PHASES:
            raise _SkipRest()
        dk = _pool(name="dk", bufs=1)
        pb = _pool(name="pb", bufs=6)

        def conv_acc(pre, cw, j):
            acc = pb.tile([128, 1024], F32, tag="s1k")
            nc.scalar.activation(acc[:], pre[:], AF.Copy, scale=cw[:, 4 * j + 3:4 * j + 4])
            for s in (1, 2, 3):
                nc.vector.scalar_tensor_tensor(
                    acc[:, s:1024], pre[:, 0:1024 - s],
                    cw[:, 4 * j + 3 - s:4 * j + 4 - s], acc[:, s:1024],
                    OP.mult, OP.add)
            return acc

        def qkv_pass(w_dram, cw, eps_col, mult, kind, jbase, wcol0):
            pps = [[pst() for n in range(2)] for j in range(4)]
            for k in range(KT):
                wt = wp.tile([128, 512], BF16, tag="wwide")
                nc.sync.dma_start(wt[:], w_dram[128 * k:128 * k + 128, wcol0:wcol0 + 512])
                for j in range(4):
                    for n in range(2):
                        nc.tensor.matmul(
                            pps[j][n][:], wt[:, 128 * j:128 * j + 128],
                            hT[:, 1024 * k + 512 * n:1024 * k + 512 * n + 512],
                            start=(k == 0), stop=(k == KT - 1))
            for j in range(4):
                jj = jbase + j
                pre = pb.tile([128, 1024], F32, tag="s1k")
                for n in range(2):
                    nc.vector.tensor_copy(pre[:, 512 * n:512 * n + 512], pps[j][n][:])
                acc = conv_acc(pre, cw, jj)
                if kind == "v":
                    vb = pb.tile([128, 1024], BF16, tag="vb16", bufs=2)
                    nc.scalar.activation(vb[:], acc[:], AF.Silu)
                    for ci in range(8):
                        pv = pst(128, 96, BF16)
                        nc.tensor.transpose(pv[:], vb[0:96, 128 * ci:128 * ci + 128],
                                            idnb[0:96, 0:96])
                        nc.scalar.copy(
                            vtok[:, VD_C * ci + 96 * jj:VD_C * ci + 96 * jj + 96], pv[:])
                else:
                    blk = pb.tile([128, 1024], F32, tag="s1k")
                    nc.scalar.activation(blk[:], acc[:], AF.Silu)
                    sq = pb.tile([128, 1024], BF16, tag="sqb", bufs=2)
                    nc.vector.tensor_mul(sq[:], blk[:], blk[:])
                    dst = qsb if kind == "q" else ksb
                    for hh, rh in ((0, 0), (1, 64)):
                        srow = pb.tile([1, 1024], F32, tag="srow", bufs=2)
                        for n2 in range(2):
                            p_ssq = pst(1, 512)
                            nc.tensor.matmul(
                                p_ssq[:], on48[:, hh:hh + 1], sq[:, 512 * n2:512 * n2 + 512],
                                start=True, stop=True)
                            nc.scalar.activation(srow[:, 512 * n2:512 * n2 + 512], p_ssq[:],
                                                 AF.Sqrt, bias=eps_col[0:1, :], scale=mult)
                        nc.vector.reciprocal_approx_fast(srow[:], srow[:])
                        sbc = pb.tile([128, 1024], F32, tag="sbc", bufs=2)
                        nc.gpsimd.partition_broadcast(sbc[:], srow[:])
                        nc.vector.tensor_mul(dst[rh:rh + 48, 1024 * jj:1024 * jj + 1024],
                                             blk[rh:rh + 48, :], sbc[rh:rh + 48, :])
                    if kind == "k":
                        for ci in range(8):
                            pk = pst(128, 128, BF16)
                            nc.tensor.transpose(
                                pk[:], ksb[:, 1024 * jj + 128 * ci:1024 * jj + 128 * ci + 128],
                                idnb[:])
                            nc.scalar.copy(
                                ktok[:, 512 * ci + 128 * jj:512 * ci + 128 * jj + 128], pk[:])

        qkv_pass(wq_d, cqw, epsq, 48.0, "q", 0, 0)
        qkv_pass(wk_d, ckw, epsk, 1.0, "k", 0, 0)
        qkv_pass(wv_d, cvw, None, None, "v", 0, 0)
        qkv_pass(wv_d, cvw, None, None, "v", 4, 512)
        if DUMP == "qkv":
            dump_y([(j, qsb[:, 1024 * j:1024 * j + 1024]) for j in range(4)]
                   + [(4 + j, ksb[:, 1024 * j:1024 * j + 1024]) for j in range(4)]
                   + [(8 + b, vtok[:, 1024 * b:1024 * b + 1024]) for b in range(6)]
                   + [(14 + b, ktok[:, 1024 * b:1024 * b + 1024]) for b in range(2)])

        # gate token-major
        for n in range(2):
            pgs = [pst(128, 384) for _ in range(NTOK)]
            for k in range(KT):
                wt = wp.tile([128, 384], BF16, tag="wg384")
                nc.sync.dma_start(wt[:], wg_d[128 * k:128 * k + 128, 384 * n:384 * n + 384])
                for i in range(NTOK):
                    nc.tensor.matmul(
                        pgs[i][:], hT[:, 1024 * k + 128 * i:1024 * k + 128 * i + 128], wt[:],
                        start=(k == 0), stop=(k == KT - 1))
            for i in range(NTOK):
                nc.scalar.activation(
                    g_tok[:, VD_C * i + 384 * n:VD_C * i + 384 * n + 384], pgs[i][:], AF.Silu)

        # a/b projections, feature-major [16, 1024]
        ppab = [pst(16, 512) for _ in range(2)]
        for k in range(KT):
            wt = wp.tile([128, 16], BF16, tag="wab")
            nc.sync.dma_start(wt[:], wab_d[128 * k:128 * k + 128, :])
            for n in range(2):
                nc.tensor.matmul(ppab[n][:], wt[:], hT[:, 1024 * k + 512 * n:1024 * k + 512 * n + 512],
                                 start=(k == 0), stop=(k == KT - 1))
        for n in range(2):
            nc.vector.tensor_copy(ab_fm[:, 512 * n:512 * n + 512], ppab[n][:])

        # -------- batched decay prep for all chunks --------
        gta = dk.tile([128, 64], F32)
        bta = dk.tile([128, 64], F32)
        for ci in range(8):
            p_ab = pst(128, 16)
            nc.tensor.transpose(p_ab[:], ab_fm[:, 128 * ci:128 * ci + 128], idn[0:16, 0:16])
            nc.vector.tensor_copy(gta[:, 8 * ci:8 * ci + 8], p_ab[:, 0:8])
            nc.vector.tensor_copy(bta[:, 8 * ci:8 * ci + 8], p_ab[:, 8:16])
        nc.vector.tensor_add(gta[:], gta[:], dtb_bc[:])
        nc.scalar.activation(gta[:], gta[:], AF.Exp)
        nc.vector.tensor_scalar_add(gta[:], gta[:], 1.0)
        nc.scalar.activation(gta[:], gta[:], AF.Ln)
        nc.vector.tensor_mul(gta[:], gta[:], nega_bc[:])        # gt_all [128,64]
        beta_all = dk.tile([128, 64], F32)
        nc.scalar.activation(beta_all[:], bta[:], AF.Sigmoid)
        nbeta_all = dk.tile([128, 64], F32)
        nc.vector.tensor_scalar_mul(nbeta_all[:], beta_all[:], -1.0)
        p_bc = pst(128, 64)
        nc.tensor.matmul(p_bc[:], cum[:], gta[:], start=True, stop=True)
        bcum_tok = dk.tile([128, 64], F32)
        nc.vector.tensor_copy(bcum_tok[:], p_bc[:])
        lam_all = dk.tile([128, 64], F32)
        nc.scalar.activation(lam_all[:], p_bc[:], AF.Exp)
        nlam_all = dk.tile([128, 64], F32)
        nc.vector.tensor_scalar_mul(nlam_all[:], lam_all[:], -1.0)
        p_bf = pst(64, 128)
        nc.tensor.transpose(p_bf[:], bcum_tok[:], idn[:])
        b_fm = dk.tile([64, 128], F32)
        nc.vector.tensor_copy(b_fm[:], p_bf[:])
        nc.scalar.dma_start(bfm_scr[:], b_fm[:])
        wfm = dk.tile([64, 128], F32)
        nc.vector.tensor_scalar(wfm[:], b_fm[:], b_fm[:, 127:128], None, OP.subtract)
        nc.scalar.activation(wfm[:], wfm[:], AF.Exp, scale=-1.0)
        p_wt = pst(128, 64)
        nc.tensor.transpose(p_wt[:], wfm[:], idn[0:64, 0:64])
        w_tok = dk.tile([128, 64], F32)
        nc.vector.tensor_copy(w_tok[:], p_wt[:])
        ebc_all = dk.tile([64, 1], F32)
        nc.scalar.activation(ebc_all[:], b_fm[:, 127:128], AF.Exp)
        # EB[8ci+h, ci] = ebc_all[8ci+h]; ebcJ[j][p, ci] = per-(ci,j) state-decay col
        EB = dk.tile([64, 8], F32)
        nc.vector.tensor_scalar_mul(EB[:], chk[:], ebc_all[:, 0:1])
        ebcJ = []
        for j in range(4):
            p_ebj = pst(128, 8)
            nc.tensor.matmul(p_ebj[:], selj[:, 128 * j:128 * j + 128], EB[:],
                             start=True, stop=True)
            ej = dk.tile([128, 8], F32, tag=f"ebj{j}", name=f"ebj{j}")
            nc.vector.tensor_copy(ej[:], p_ebj[:])
            ebcJ.append(ej)
        _rel(pb)

        # ============ Phase C ============
        if "C" not in PHASES:
            raise _SkipRest()
        wod = _pool(name="wod", bufs=16)
        pd = _pool(name="pd", bufs=8)
        dput = _pool(name="dput", bufs=48)
        dpa = _pool(name="dpa", bufs=10)
        dpx = _pool(name="dpx", bufs=10)
        dpf = _pool(name="dpf", bufs=6)
        dp2 = _pool(name="dp2", bufs=2)
        spool = _pool(name="spool", bufs=2)

        def emit_D(half):
            t0 = 512 * half
            for db in range(16):
                wt = wod.tile([128, VD_C], BF16, tag="wo", name="wo_t", bufs=16)
                nc.scalar.dma_start(wt[:], wo_d[:, VD_C * db:VD_C * db + VD_C])
                pp = pst()
                for fb in range(6):
                    nc.tensor.matmul(pp[:], wt[:, 128 * fb:128 * fb + 128],
                                     osb[:, 1024 * fb + t0:1024 * fb + t0 + 512],
                                     start=(fb == 0), stop=(fb == 5))
                stg = pd.tile([128, 512], BF16, tag="s512b", name="stg", bufs=8)
                nc.scalar.copy(stg[:], pp[:])
                nc.sync.dma_start(oT_in[half][128 * db:128 * db + 128, :], stg[:])
            nc.gpsimd.collective_compute(
                "AllReduce", OP.add, ins=[oT_in[half][:]], outs=[oT_out[half][:]],
                replica_groups=groups)

        S_cur = {}
        for j in range(4):
            S_cur[j] = spool.tile([128, DV], F32, tag=f"s{j}", name=f"s{j}")
            nc.vector.memset(S_cur[j][:], 0.0)

        for ci in range(NCH):
            # ---- prep all 8 heads: abar, xx, xt ----
            ABAR, XX, XT = {}, {}, {}
            for j in range(4):
                for hh in range(2):
                    h = 2 * j + hh
                    rh = 64 * hh
                    kts = ksb[rh:rh + 48, 1024 * j + 128 * ci:1024 * j + 128 * ci + 128]
                    qts = qsb[rh:rh + 48, 1024 * j + 128 * ci:1024 * j + 128 * ci + 128]
                    p_kk = pst(128, 128)
                    nc.tensor.matmul(p_kk[:], kts, kts, start=True, stop=True)
                    p_kq = pst(128, 128)
                    nc.tensor.matmul(p_kq[:], kts, qts, start=True, stop=True)
                    bc128 = dpf.tile([128, 128], F32, tag="bc", name="bc128")
                    nc.gpsimd.dma_start(
                        bc128[:],
                        bfm_scr[8 * ci + h:8 * ci + h + 1, :].to_broadcast((128, 128)))
                    dte = dpf.tile([128, 128], F32, tag="dte", name="dte")
                    nc.vector.scalar_tensor_tensor(
                        dte[:], bc128[:], bcum_tok[:, 8 * ci + h:8 * ci + h + 1],
                        negl[:], OP.subtract, OP.add)
                    dincl = dput.tile([128, 128], BF16, tag="ut", name="dincl")
                    nc.scalar.activation(dincl[:], dte[:], AF.Exp)
                    abar = dpa.tile([128, 128], BF16, tag="abar", name="abar")
                    nc.vector.tensor_mul(abar[:], p_kq[:], dincl[:])
                    dstr = dput.tile([128, 128], BF16, tag="ut", name="dstr")
                    nc.vector.tensor_mul(dstr[:], dincl[:], mstb[:])
                    x0 = dput.tile([128, 128], BF16, tag="ut", name="x0")
                    nc.vector.tensor_mul(x0[:], p_kk[:], dstr[:])
                    xx = dpx.tile([128, 128], BF16, tag="xx", name="xx")
                    nc.vector.tensor_scalar_mul(xx[:], x0[:],
                                                nbeta_all[:, 8 * ci + h:8 * ci + h + 1])
                    p_x = pst(128, 128, BF16)
                    nc.tensor.transpose(p_x[:], xx[:], idnb[:])
                    xt = dpx.tile([128, 128], BF16, tag="xt", name="xt")
                    nc.scalar.copy(xt[:], p_x[:])
                    ABAR[h], XX[h], XT[h] = abar, xx, xt

            # ---- UT inverse, level-major across all 8 heads ----
            PM = {}
            for h in range(HP):
                pmat = dput.tile([128, 128], BF16, tag="ut", name="pmat")
                nc.vector.tensor_add(pmat[:], XX[h][:], idnb[:])
                PM[h] = pmat
            cur = {h: (XX[h], XT[h]) for h in range(HP)}
            for lvl in range(6):
                last = lvl == 5
                nxt = {}
                for h in range(HP):
                    xx, xt = cur[h]
                    x2 = None
                    if not last:
                        p_sq = pst(128, 128)
                        nc.tensor.matmul(p_sq[:], xt[:], xx[:], start=True, stop=True)
                        x2 = dput.tile([128, 128], BF16, tag="ut", name="x2")
                        nc.scalar.copy(x2[:], p_sq[:])
                    p_sqt = pst(128, 128)
                    nc.tensor.matmul(p_sqt[:], xx[:], xt[:], start=True, stop=True)
                    xt2 = dput.tile([128, 128], BF16, tag="ut", name="xt2")
                    if h % 2 == 0:
                        nc.scalar.copy(xt2[:], p_sqt[:])
                    else:
                        nc.vector.tensor_copy(xt2[:], p_sqt[:])
                    nxt[h] = (x2, xt2)
                for h in range(HP):
                    p_pr = pst(128, 128)
                    nc.tensor.matmul(p_pr[:], nxt[h][1][:], PM[h][:], start=True, stop=True)
                    pnew = dput.tile([128, 128], BF16, tag="ut", name="pnew")
                    if h % 2 == 0:
                        nc.vector.tensor_add(pnew[:], PM[h][:], p_pr[:])
                    else:
                        nc.vector.tensor_add(pnew[:], p_pr[:], PM[h][:])
                    PM[h] = pnew
                if not last:
                    cur = nxt

            # ---- state/output, step-major in 4-head waves ----
            ot_all = dp2.tile([128, VD_C], F32, tag="otall", name="ot_all")
            for jp in (0, 2):
                heads = [(j, hh) for j in (jp, jp + 1) for hh in (0, 1)]
                SB, PS, KW = {}, {}, {}
                for j in (jp, jp + 1):
                    S_bf = dp2.tile([128, DV], BF16, tag=f"sbf{j}", name="S_bf")
                    nc.vector.tensor_copy(S_bf[:], S_cur[j][:])
                    SB[j] = S_bf
                    PS[j] = pst(128, DV)
                for j, hh in heads:
                    h = 2 * j + hh
                    rh = 64 * hh
                    kw = dp2.tile([128, 48], BF16, tag="kw", name="kw", bufs=6)
                    nc.vector.tensor_scalar_mul(
                        kw[:], ktok[:, 512 * ci + 128 * j + rh:512 * ci + 128 * j + rh + 48],
                        w_tok[:, 8 * ci + h:8 * ci + h + 1])
                    KW[h] = kw
                RR, UU = {}, {}
                for j, hh in heads:
                    h = 2 * j + hh
                    rh = 64 * hh
                    kts = ksb[rh:rh + 48, 1024 * j + 128 * ci:1024 * j + 128 * ci + 128]
                    p_ks = pst(128, DV)
                    nc.tensor.matmul(p_ks[:], kts, SB[j][rh:rh + 48, :], start=True, stop=True)
                    r_ = dp2.tile([128, DV], BF16, tag="rr", name="r_", bufs=5)
                    nc.vector.scalar_tensor_tensor(
                        r_[:], p_ks[:], nlam_all[:, 8 * ci + h:8 * ci + h + 1],
                        vtok[:, VD_C * ci + 96 * h:VD_C * ci + 96 * h + 96],
                        OP.mult, OP.add)
                    RR[h] = r_
                for j, hh in heads:
                    h = 2 * j + hh
                    p_w = pst(128, DV)
                    nc.tensor.matmul(p_w[:], PM[h][:], RR[h][:], start=True, stop=True)
                    u_ = dp2.tile([128, DV], BF16, tag="uu", name="u_", bufs=5)
                    nc.vector.tensor_scalar_mul(u_[:], p_w[:],
                                                beta_all[:, 8 * ci + h:8 * ci + h + 1])
                    UU[h] = u_
                for j, hh in heads:
                    h = 2 * j + hh
                    rh = 64 * hh
                    qts = qsb[rh:rh + 48, 1024 * j + 128 * ci:1024 * j + 128 * ci + 128]
                    p_oi = pst(128, DV)
                    nc.tensor.matmul(p_oi[:], ABAR[h][:], UU[h][:], start=True, stop=True)
                    p_qs = pst(128, DV)
                    nc.tensor.matmul(p_qs[:], qts, SB[j][rh:rh + 48, :], start=True, stop=True)
                    ots = ot_all[:, 96 * h:96 * h + 96]
                    nc.vector.tensor_scalar_mul(ots, p_qs[:],
                                                lam_all[:, 8 * ci + h:8 * ci + h + 1])
                    nc.vector.tensor_add(ots, ots, p_oi[:])
                    nc.tensor.matmul(PS[j][rh:rh + 48, :], KW[h][:], UU[h][:],
                                     start=True, stop=True)
                for j in (jp, jp + 1):
                    s_new = spool.tile([128, DV], F32, tag=f"s{j}", name="s_new")
                    for rh2 in (0, 64):
                        nc.vector.scalar_tensor_tensor(
                            s_new[rh2:rh2 + 48, :], S_cur[j][rh2:rh2 + 48, :],
                            ebcJ[j][rh2:rh2 + 48, ci:ci + 1],
                            PS[j][rh2:rh2 + 48, :], OP.mult, OP.add)
                    S_cur[j] = s_new

            # ---- batched gated rmsnorm + gate + transpose to osb ----
            osq = dp2.tile([128, VD_C], F32, tag="osq", name="osq")
            nc.vector.tensor_mul(osq[:], ot_all[:], ot_all[:])
            rcol8 = dp2.tile([128, 8], F32, tag="rc8", name="rcol8")
            for h in range(HP):
                nc.vector.tensor_reduce(rcol8[:, h:h + 1], osq[:, 96 * h:96 * h + 96],
                                        mybir.AxisListType.X, OP.add)
            nc.scalar.activation(rcol8[:], rcol8[:], AF.Sqrt, bias=epsg[:], scale=1.0 / DV)
            nc.vector.reciprocal_approx_fast(rcol8[:], rcol8[:])
            ob = dp2.tile([128, VD_C], BF16, tag="ob", name="ob")
            for h in range(HP):
                nc.vector.scalar_tensor_tensor(
                    ob[:, 96 * h:96 * h + 96], ot_all[:, 96 * h:96 * h + 96],
                    rcol8[:, h:h + 1], g_tok[:, VD_C * ci + 96 * h:VD_C * ci + 96 * h + 96],
                    OP.mult, OP.mult)
            for b6 in range(6):
                p_ot = pst(128, 128, BF16)
                nc.tensor.transpose(p_ot[:], ob[:, 128 * b6:128 * b6 + 128], idnb[:])
                nc.scalar.copy(osb[:, 1024 * b6 + 128 * ci:1024 * b6 + 128 * ci + 128], p_ot[:])

            # ---- o_projT half + AllReduce as soon as its tokens exist ----
            if ci == 3:
                emit_D(0)
            if ci == 7:
                emit_D(1)

        for p in (spool, dp2, dpf, dpx, dpa, dput, pd, wod):
            _rel(p)
        if DUMP == "o":
            dump_y([(b, osb[:, 1024 * b:1024 * b + 1024]) for b in range(6)]
                   + [(6 + b, g_tok[:, 1024 * b:1024 * b + 1024]) for b in range(6)]
                   + [(12 + b, ktok[:, 1024 * b:1024 * b + 1024]) for b in range(4)])
        if DUMP == "dk":
            dump_y([(0, gta[:]), (1, beta_all[:]), (2, bcum_tok[:]),
                    (3, lam_all[:]), (4, w_tok[:]), (5, b_fm[:]),
                    (6, ebc_all[:]), (7, ebcJ[0][:]), (8, ebcJ[3][:])])
        _rel(dk)
        _rel(bigq)

        if "D" not in PHASES:
            raise _SkipRest()
        if DUMP in ("ar", "oin"):
            src = oT_out if DUMP == "ar" else oT_in
            dbg = _pool(name="dbg", bufs=4)
            for bi in range(16):
                st = dbg.tile([128, 1024], F32, tag="dbg", name="st")
                for half in range(2):
                    so = dbg.tile([128, 512], BF16, tag="dbg2", name="so")
                    nc.sync.dma_start(so[:], src[half][128 * bi:128 * bi + 128, :])
                    nc.vector.tensor_copy(st[:, 512 * half:512 * half + 512], so[:])
                nc.sync.dma_start(y_d[128 * bi:128 * bi + 128, :], st[:])
            _rel(dbg)
            raise _Dumped()

        # ============ Phase E ============
        if "E" not in PHASES:
            raise _SkipRest()
        seq = _pool(name="seq", bufs=3)
        pe = _pool(name="pe", bufs=4)
        wp13 = _pool(name="wp13", bufs=3)
        wp2 = _pool(name="wp2", bufs=3)
        mida = pg.tile([128, 6 * 1024], BF16, tag="gtok")
        pmid = _pool(name="pmid", bufs=1)
        midb = pmid.tile([128, 5 * 1024], BF16, tag="midb")

        def mid_ap(m, off, ln):
            if m < 6:
                return mida[:, 1024 * m + off:1024 * m + off + ln]
            return midb[:, 1024 * (m - 6) + off:1024 * (m - 6) + off + ln]

        ffT = hT
        for half in range(2):
            hs = slice(512 * half, 512 * half + 512)
            p_s2 = pst(1, 512)
            for k in range(KT):
                xe = seq.tile([128, 512], F32, tag="xe")
                nc.sync.dma_start(xe[:], xT_d[128 * k:128 * k + 128, hs])
                oe = seq.tile([128, 512], BF16, tag="oe")
                nc.gpsimd.dma_start(oe[:], oT_out[half][128 * k:128 * k + 128, :])
                h2 = seq.tile([128, 512], F32, tag="h2T")
                nc.vector.tensor_add(h2[:], xe[:], oe[:])
                nc.scalar.dma_start(h2T_scr[128 * k:128 * k + 128, hs], h2[:])
                sqe = seq.tile([128, 512], BF16, tag="sqe")
                nc.vector.tensor_mul(sqe[:], h2[:], h2[:])
                nc.tensor.matmul(p_s2[:], ones1[:], sqe[:],
                                 start=(k == 0), stop=(k == KT - 1))
            srowE = seq.tile([1, 512], F32, tag="srowE", bufs=2)
            nc.scalar.activation(srowE[:], p_s2[:], AF.Sqrt,
                                 bias=eps1[0:1, :], scale=1.0 / D)
            nc.vector.reciprocal_approx_fast(srowE[:], srowE[:])
            sbcE = seq.tile([128, 512], F32, tag="sbcE", bufs=2)
            nc.gpsimd.partition_broadcast(sbcE[:], srowE[:])
            for k in range(KT):
                h2r = seq.tile([128, 512], F32, tag="h2r", bufs=4)
                nc.sync.dma_start(h2r[:], h2T_scr[128 * k:128 * k + 128, hs])
                nc.vector.tensor_mul(ffT[:, 1024 * k + 512 * half:1024 * k + 512 * half + 512],
                                     h2r[:], sbcE[:])
            # W1/W3 for this half
            for m in range(11):
                pu1 = pst()
                pu3 = pst()
                wt13 = wp13.tile([128, 4096], BF16, tag="w13", name="wt13")
                nc.sync.dma_start(wt13[:], w13_d[:, 4096 * m:4096 * m + 4096])
                for k in range(KT):
                    rhs = ffT[:, 1024 * k + 512 * half:1024 * k + 512 * half + 512]
                    nc.tensor.matmul(pu1[:], wt13[:, 256 * k:256 * k + 128], rhs,
                                     start=(k == 0), stop=(k == KT - 1))
                    nc.tensor.matmul(pu3[:], wt13[:, 256 * k + 128:256 * k + 256], rhs,
                                     start=(k == 0), stop=(k == KT - 1))
                u1s = pe.tile([128, 512], F32, tag="s512", name="u1s")
                nc.scalar.activation(u1s[:], pu1[:], AF.Silu)
                nc.vector.tensor_mul(mid_ap(m, 512 * half, 512), u1s[:], pu3[:])
            if DUMP == "mid" and half == 1:
                dump_y([(m, mid_ap(m, 0, 1024)) for m in range(11)])
            # W2 for this half
            for db in range(16):
                wt2 = wp2.tile([128, INT_C], BF16, tag="w2", name="w2_t")
                nc.sync.dma_start(wt2[:], w2_d[:, INT_C * db:INT_C * db + INT_C])
                pps = pst()
                for m in range(11):
                    nc.tensor.matmul(pps[:], wt2[:, 128 * m:128 * m + 128],
                                     mid_ap(m, 512 * half, 512),
                                     start=(m == 0), stop=(m == 10))
                h2t = pe.tile([128, 512], F32, tag="s512", name="h2t")
                nc.sync.dma_start(h2t[:], h2T_scr[128 * db:128 * db + 128, hs])
                yst = pe.tile([128, 512], F32, tag="s512", name="yst")
                nc.vector.tensor_scalar_mul(yst[:], h2t[:], 0.25)
                nc.vector.tensor_add(yst[:], yst[:], pps[:])
                nc.sync.dma_start(y_d[128 * db:128 * db + 128, hs], yst[:])

        for p in (pmid, wp2, wp13, pe, seq, dram, wp, pg, big, ps, cpool):
            _rel(p)
      except _SkipRest:
        zst = _pool(name="zst", bufs=1)
        zt = zst.tile([128, 512], F32)
        nc.vector.memset(zt[:], 0.0)
        for i in range(16):
            for dh in range(2):
                nc.sync.dma_start(y_d[128 * i:128 * i + 128, 512 * dh:512 * dh + 512], zt[:])
        for p in reversed(live_pools):
            p.release()
      except _Dumped:
        for p in reversed(live_pools):
            p.release()

    nc.compile()
    return nc


def _pack_wo(wo):
    # [768, 2048] -> [128, 16*768]: col = 768*db + 128*fb + c
    return np.ascontiguousarray(
        wo.reshape(6, 128, 16, 128).transpose(1, 2, 0, 3).reshape(128, 16 * 768))


def _pack_w13(w1, w3):
    # [2048, 1408] x2 -> [128, 11*4096]: col = 4096*m + 256*k + 128*which + c
    a = w1.reshape(16, 128, 11, 128).transpose(1, 2, 0, 3)   # [128, 11, 16, 128]
    b = w3.reshape(16, 128, 11, 128).transpose(1, 2, 0, 3)
    return np.ascontiguousarray(
        np.stack([a, b], axis=3).reshape(128, 11 * 4096))


def _pack_w2(w2):
    # [1408, 2048] -> [128, 16*1408]: col = 1408*db + 128*m + c
    return np.ascontiguousarray(
        w2.reshape(11, 128, 16, 128).transpose(1, 2, 0, 3).reshape(128, 16 * 1408))


def _shard(inputs):
    f32 = np.float32
    bf = ml_dtypes.bfloat16
    rms1 = np.asarray(inputs["rms1_w"], f32)
    rms2 = np.asarray(inputs["rms2_w"], f32)
    gn = np.asarray(inputs["gnorm_w"], f32)
    in_maps = []
    for c in range(8):
        g, m = c // 4, c % 4
        qs = slice(384 * m, 384 * m + 384)
        vs = slice(768 * m, 768 * m + 768)
        hs = slice(8 * m, 8 * m + 8)
        isl = slice(1408 * m, 1408 * m + 1408)

        def padqk(w):
            wp_ = np.zeros((D, QKP), f32)
            for h in range(8):
                wp_[:, 64 * h:64 * h + 48] = w[:, 48 * h:48 * h + 48]
            return wp_

        def padcw(w):
            cp = np.zeros((QKP, 4), f32)
            for h in range(8):
                cp[64 * h:64 * h + 48] = w[48 * h:48 * h + 48]
            return cp

        def padv(w):
            colpad = w.shape[0] == D
            out = np.zeros((D, VP) if colpad else (VP, w.shape[1]), f32)
            for h in range(8):
                if colpad:
                    out[:, 128 * h:128 * h + 96] = w[:, 96 * h:96 * h + 96]
                else:
                    out[128 * h:128 * h + 96] = w[96 * h:96 * h + 96]
            return out

        dtb8 = np.asarray(inputs["dt_bias"], f32)[hs]
        nega8 = -np.exp(np.asarray(inputs["A_log"], f32)[hs])
        in_maps.append(dict(
            xT=np.ascontiguousarray(np.asarray(inputs["x"], f32)[g].T),
            wq=padqk(np.asarray(inputs["Wq"], f32)[:, qs] * rms1[:, None]).astype(bf),
            wk=padqk(np.asarray(inputs["Wk"], f32)[:, qs] * rms1[:, None]).astype(bf),
            wv=padv(np.asarray(inputs["Wv"], f32)[:, vs] * rms1[:, None]).astype(bf),
            wg=np.ascontiguousarray(
                np.asarray(inputs["Wg"], f32)[:, vs] * rms1[:, None]).astype(bf),
            wab=np.ascontiguousarray(np.concatenate(
                [np.asarray(inputs["Wa"], f32)[:, hs],
                 np.asarray(inputs["Wb"], f32)[:, hs]], 1) * rms1[:, None]).astype(bf),
            cq=padcw(np.asarray(inputs["conv_q_w"], f32)[qs]),
            ck=padcw(np.asarray(inputs["conv_k_w"], f32)[qs]),
            cv=padv(np.asarray(inputs["conv_v_w"], f32)[vs]),
            dtb=np.tile(dtb8, 8).reshape(1, 64).copy(),
            nega=np.tile(nega8, 8).reshape(1, 64).copy(),
            wo=_pack_wo(np.asarray(inputs["Wo"], f32)[vs] * np.tile(gn, 8)[:, None]).astype(bf),
            w13=_pack_w13(np.asarray(inputs["W1"], f32)[:, isl] * rms2[:, None],
                          np.asarray(inputs["W3"], f32)[:, isl] * rms2[:, None]).astype(bf),
            w2=_pack_w2(np.asarray(inputs["W2"], f32)[isl]).astype(bf),
        ))
    return in_maps


def kernel(**inputs):
    if "nc" not in _cache:
        _cache["nc"] = _build(8)
    res = run_bass_kernel_spmd(_cache["nc"], _shard(inputs), list(range(8)))
    out = np.zeros((B, T, D), np.float32)
    for g in range(2):
        yT = sum(res.results[4 * g + m]["y"] for m in range(4))
        out[g] = yT.T
    return out


# revision 48
# speedup vs baseline: 1.9005x; 1.0011x over previous
"""GatedDeltaNet block kernel for 8 Trainium2 cores (Bass/Tile), bf16 rework.

Sharding: DP2 (batch) x TP4 (heads / MLP-inter). Core c: group g=c//4 runs
batch g; member m=c%4 owns heads [8m,8m+8), q/k cols [384m,..), v/g cols
[768m,..), INTER [1408m,..). Two half-token AllReduces per 4-core group after
o_proj (overlapped with o_proj compute); final down-proj partials summed on
the host.

Everything runs feature-major (host passes x transposed, takes y transposed)
so there are no PE transposes outside the delta-rule inner loop. All big
GEMM operands are bf16 (host-cast weights); psum accumulation, the delta-rule
state, decay/beta math and norms stay fp32.

Per-core dataflow:
  A: xT [D,T] -> rmsnorm via matmul-accumulated column sumsq -> hT bf16 (SBUF)
  B: bf16 projections off hT; q/k feature-major (heads padded to 64 rows)
     -> conv+silu+l2norm -> SBUF (+ token-major copies of k, v via PE
     transposes); gate token-major; a/b -> batched decay prep for all chunks
  C: chunked gated delta rule (C=128, UT transform via log-doubling inverse,
     bf16 matmuls / fp32 state); writes normed+gated o feature-major to SBUF
  D: o_projT in two token halves, each followed by its AllReduce (overlapped)
  E: h2T = xT + oT; rmsnorm -> ffT bf16 (reuses hT); MLP bf16; yT partials
"""
import sys
sys.path.insert(0, '/opt/trn_rl_repo')
import numpy as np
import ml_dtypes

import concourse.bass as bass
import concourse.bacc as bacc
import concourse.mybir as mybir
import concourse.tile as tile
from concourse.bass_isa import ReduceOp
from concourse.bass_utils import run_bass_kernel_spmd

F32 = mybir.dt.float32
BF16 = mybir.dt.bfloat16
AF = mybir.ActivationFunctionType
OP = mybir.AluOpType

B, T, D = 2, 1024, 2048
H, DK, DV = 32, 48, 96
HP = 8
QKP = 512
VD_C = 768
VP = 1024
INT_C = 1408
C = 128
NCHUNK = T // C
KT = D // 128
NTOK = T // 128

_cache = {}
import os
PHASES = os.environ.get("DN_PHASES", "ABCDE")
NCH = int(os.environ.get("DN_NCHUNK", str(T // C)))
DUMP = os.environ.get("DN_DUMP", "")


class _SkipRest(Exception):
    pass


class _Dumped(Exception):
    pass


def _build(n_cores=8):
    groups = [[0, 1, 2, 3], [4, 5, 6, 7]] if n_cores == 8 else [[0]]
    nc = bacc.Bacc("TRN2", target_bir_lowering=False, debug=False, num_devices=n_cores)

    xT_d = nc.dram_tensor("xT", [D, T], F32, kind="ExternalInput")
    wq_d = nc.dram_tensor("wq", [D, QKP], BF16, kind="ExternalInput")
    wk_d = nc.dram_tensor("wk", [D, QKP], BF16, kind="ExternalInput")
    wv_d = nc.dram_tensor("wv", [D, VP], BF16, kind="ExternalInput")
    wg_d = nc.dram_tensor("wg", [D, VD_C], BF16, kind="ExternalInput")
    wab_d = nc.dram_tensor("wab", [D, 16], BF16, kind="ExternalInput")
    cq_d = nc.dram_tensor("cq", [QKP, 4], F32, kind="ExternalInput")
    ck_d = nc.dram_tensor("ck", [QKP, 4], F32, kind="ExternalInput")
    cv_d = nc.dram_tensor("cv", [VP, 4], F32, kind="ExternalInput")
    dtb_d = nc.dram_tensor("dtb", [1, 64], F32, kind="ExternalInput")
    nega_d = nc.dram_tensor("nega", [1, 64], F32, kind="ExternalInput")
    wo_d = nc.dram_tensor("wo", [128, 16 * VD_C], BF16, kind="ExternalInput")
    w13_d = nc.dram_tensor("w13", [128, 11 * 4096], BF16, kind="ExternalInput")
    w2_d = nc.dram_tensor("w2", [128, 16 * INT_C], BF16, kind="ExternalInput")
    y_d = nc.dram_tensor("y", [D, T], F32, kind="ExternalOutput")

    ones = np.ones((128, 128), np.float32)
    idn_c = nc.inline_tensor(np.eye(128, dtype=np.float32), "idn_c")
    idnb_c = nc.inline_tensor(np.eye(128, dtype=ml_dtypes.bfloat16), "idnb_c")
    cum_c = nc.inline_tensor(np.triu(ones).copy(), "cum_c")
    mst_c = nc.inline_tensor(np.triu(ones, 1).astype(ml_dtypes.bfloat16), "mst_c")
    msi_c = nc.inline_tensor(np.triu(ones).copy(), "msi_c")
    negl_c = nc.inline_tensor((np.tril(ones, -1) * -1e30).copy(), "negl_c")
    # SELJ[r, 128j+p] = 1 iff (r%8==2j and p<48) or (r%8==2j+1 and 64<=p<112)
    selj_np = np.zeros((64, 512), np.float32)
    for r in range(64):
        for j in range(4):
            if r % 8 == 2 * j:
                selj_np[r, 128 * j:128 * j + 48] = 1.0
            if r % 8 == 2 * j + 1:
                selj_np[r, 128 * j + 64:128 * j + 112] = 1.0
    selj_c = nc.inline_tensor(selj_np, "selj_c")
    # CHK[8ci+h, ci] = 1
    chk_np = np.zeros((64, 8), np.float32)
    for ci in range(8):
        chk_np[8 * ci:8 * ci + 8, ci] = 1.0
    chk_c = nc.inline_tensor(chk_np, "chk_c")
    on48_np = np.zeros((128, 2), ml_dtypes.bfloat16)
    on48_np[0:48, 0] = 1.0
    on48_np[64:112, 1] = 1.0
    on48_c = nc.inline_tensor(on48_np, "on48_c")
    ones1_np = np.ones((128, 1), ml_dtypes.bfloat16)
    ones1_c = nc.inline_tensor(ones1_np, "ones1_c")

    with tile.TileContext(nc) as tc:
      live_pools = []

      def _pool(**kw):
          p = tc.alloc_tile_pool(**kw)
          live_pools.append(p)
          return p

      def _rel(p):
          p.release()
          live_pools.remove(p)

      try:
        cpool = _pool(name="consts", bufs=1)
        ps = _pool(name="ps", bufs=8, space="PSUM")

        def pst(p=128, f=512, dt=F32):
            return ps.tile([p, f], dt, tag="ps", name="pst")

        def dump_y(items):
            # items: list of (y_block_index, ap [p, <=1024]) — copy (cast) to y
            dbg = _pool(name="dbg", bufs=4)
            for bi, ap in items:
                p, n = ap.shape[0], ap.shape[1]
                st = dbg.tile([128, 1024], F32, tag="dbg", name="st")
                nc.vector.tensor_copy(st[0:p, 0:n], ap)
                nc.sync.dma_start(y_d[128 * bi:128 * bi + p, 0:n], st[0:p, 0:n])
            _rel(dbg)
            raise _Dumped()

        idn = cpool.tile([128, 128], F32)
        idnb = cpool.tile([128, 128], BF16)
        cum = cpool.tile([128, 128], F32)
        mstb = cpool.tile([128, 128], BF16)
        msi = cpool.tile([128, 128], F32)
        negl = cpool.tile([128, 128], F32)
        selj = cpool.tile([64, 512], F32)
        chk = cpool.tile([64, 8], F32)
        on48 = cpool.tile([128, 2], BF16)
        ones1 = cpool.tile([128, 1], BF16)
        for t_, s_ in [(idn, idn_c), (idnb, idnb_c), (cum, cum_c), (mstb, mst_c),
                       (msi, msi_c), (negl, negl_c), (selj, selj_c), (chk, chk_c),
                       (on48, on48_c), (ones1, ones1_c)]:
            nc.sync.dma_start(t_[:], s_[:])
        eps1 = cpool.tile([128, 1], F32)
        nc.vector.memset(eps1[:], 1e-5)
        epsq = cpool.tile([128, 1], F32)
        nc.vector.memset(epsq[:], 48e-6)
        epsk = cpool.tile([128, 1], F32)
        nc.vector.memset(epsk[:], 1e-6)
        epsg = cpool.tile([128, 1], F32)
        nc.vector.memset(epsg[:], 1e-5)
        dtb_r = cpool.tile([1, 64], F32)
        nega_r = cpool.tile([1, 64], F32)
        nc.sync.dma_start(dtb_r[:], dtb_d[:])
        nc.sync.dma_start(nega_r[:], nega_d[:])
        dtb_bc = cpool.tile([128, 64], F32)
        nega_bc = cpool.tile([128, 64], F32)
        nc.gpsimd.partition_broadcast(dtb_bc[:], dtb_r[:])
        nc.gpsimd.partition_broadcast(nega_bc[:], nega_r[:])
        cqw = cpool.tile([128, 16], F32)
        ckw = cpool.tile([128, 16], F32)
        cvw = cpool.tile([128, 32], F32)
        for j in range(4):
            nc.sync.dma_start(cqw[:, 4 * j:4 * j + 4], cq_d[128 * j:128 * j + 128, :])
            nc.sync.dma_start(ckw[:, 4 * j:4 * j + 4], ck_d[128 * j:128 * j + 128, :])
        for j in range(8):
            nc.sync.dma_start(cvw[:, 4 * j:4 * j + 4], cv_d[128 * j:128 * j + 128, :])
        ab_fm = cpool.tile([16, 1024], F32)

        big = _pool(name="big", bufs=1)
        hT = big.tile([128, KT * 1024], BF16)       # also ffT in phase E
        osb = big.tile([128, 6 * 1024], BF16)       # feature-major o: [feat%128, 1024*(f//128)+tok]
        pg = _pool(name="pg", bufs=1)
        g_tok = pg.tile([128, NTOK * VD_C], BF16, tag="gtok")

        wp = _pool(name="wp", bufs=4)
        dram = _pool(name="dram", bufs=1, space="DRAM")
        oT_in = [dram.tile([D, 512], BF16, name=f"oT_in{i}") for i in range(2)]
        oT_out = [dram.tile([D, 512], BF16, name=f"oT_out{i}") for i in range(2)]
        h2T_scr = dram.tile([D, T], F32)
        bfm_scr = dram.tile([64, 128], F32)

        bigq = _pool(name="bigq", bufs=1)
        qsb = bigq.tile([128, 4 * 1024], BF16)
        ksb = bigq.tile([128, 4 * 1024], BF16)
        ktok = bigq.tile([128, 8 * 512], BF16)      # token-major k: [tok, 512ci+128j]
        vtok = bigq.tile([128, 8 * VD_C], BF16)     # token-major v: [tok, 768ci+96h]
        nc.vector.memset(qsb[:], 0.0)
        nc.vector.memset(ksb[:], 0.0)

        # ============ Phase A: hT = rmsnorm(x)^T in bf16 ============
        stA = _pool(name="stA", bufs=16)
        sqp = _pool(name="sqp", bufs=3)
        p_ss = [pst(1, 512) for _ in range(2)]
        xts = []
        for k in range(KT):
            xa = stA.tile([128, 1024], F32, tag="xT")
            nc.sync.dma_start(xa[:], xT_d[128 * k:128 * k + 128, :])
            xts.append(xa)
            sq = sqp.tile([128, 1024], BF16, tag="sq")
            nc.vector.tensor_mul(sq[:], xa[:], xa[:])
            for n in range(2):
                nc.tensor.matmul(p_ss[n][:], ones1[:], sq[:, 512 * n:512 * n + 512],
                                 start=(k == 0), stop=(k == KT - 1))
        srowA = sqp.tile([1, 1024], F32, tag="srowA", bufs=1)
        for n in range(2):
            nc.scalar.activation(srowA[:, 512 * n:512 * n + 512], p_ss[n][:],
                                 AF.Sqrt, bias=eps1[0:1, :], scale=1.0 / D)
        nc.vector.reciprocal_approx_fast(srowA[:], srowA[:])
        sbcA = sqp.tile([128, 1024], F32, tag="sbcA", bufs=1)
        nc.gpsimd.partition_broadcast(sbcA[:], srowA[:])
        for k in range(KT):
            nc.vector.tensor_mul(hT[:, 1024 * k:1024 * k + 1024], xts[k][:], sbcA[:])
        _rel(sqp)
        _rel(stA)
        if DUMP == "hT":
            dump_y([(k, hT[:, 1024 * k:1024 * k + 1024]) for k in range(KT)])

        # ============ Phase B ============
        if "B" not in PHASES:
            raise _SkipRest()
        dk = _pool(name="dk", bufs=1)
        pb = _pool(name="pb", bufs=6)

        def conv_acc(pre, cw, j):
            acc = pb.tile([128, 1024], F32, tag="s1k")
            nc.scalar.activation(acc[:], pre[:], AF.Copy, scale=cw[:, 4 * j + 3:4 * j + 4])
            for s in (1, 2, 3):
                nc.vector.scalar_tensor_tensor(
                    acc[:, s:1024], pre[:, 0:1024 - s],
                    cw[:, 4 * j + 3 - s:4 * j + 4 - s], acc[:, s:1024],
                    OP.mult, OP.add)
            return acc

        def qkv_pass(w_dram, cw, eps_col, mult, kind, jbase, wcol0):
            pps = [[pst() for n in range(2)] for j in range(4)]
            for k in range(KT):
                wt = wp.tile([128, 512], BF16, tag="wwide")
                nc.sync.dma_start(wt[:], w_dram[128 * k:128 * k + 128, wcol0:wcol0 + 512])
                for j in range(4):
                    for n in range(2):
                        nc.tensor.matmul(
                            pps[j][n][:], wt[:, 128 * j:128 * j + 128],
                            hT[:, 1024 * k + 512 * n:1024 * k + 512 * n + 512],
                            start=(k == 0), stop=(k == KT - 1))
            for j in range(4):
                jj = jbase + j
                pre = pb.tile([128, 1024], F32, tag="s1k")
                for n in range(2):
                    nc.vector.tensor_copy(pre[:, 512 * n:512 * n + 512], pps[j][n][:])
                acc = conv_acc(pre, cw, jj)
                if kind == "v":
                    vb = pb.tile([128, 1024], BF16, tag="vb16", bufs=2)
                    nc.scalar.activation(vb[:], acc[:], AF.Silu)
                    for ci in range(8):
                        pv = pst(128, 96, BF16)
                        nc.tensor.transpose(pv[:], vb[0:96, 128 * ci:128 * ci + 128],
                                            idnb[0:96, 0:96])
                        nc.scalar.copy(
                            vtok[:, VD_C * ci + 96 * jj:VD_C * ci + 96 * jj + 96], pv[:])
                else:
                    blk = pb.tile([128, 1024], F32, tag="s1k")
                    nc.scalar.activation(blk[:], acc[:], AF.Silu)
                    sq = pb.tile([128, 1024], BF16, tag="sqb", bufs=2)
                    nc.vector.tensor_mul(sq[:], blk[:], blk[:])
                    dst = qsb if kind == "q" else ksb
                    for hh, rh in ((0, 0), (1, 64)):
                        srow = pb.tile([1, 1024], F32, tag="srow", bufs=2)
                        for n2 in range(2):
                            p_ssq = pst(1, 512)
                            nc.tensor.matmul(
                                p_ssq[:], on48[:, hh:hh + 1], sq[:, 512 * n2:512 * n2 + 512],
                                start=True, stop=True)
                            nc.scalar.activation(srow[:, 512 * n2:512 * n2 + 512], p_ssq[:],
                                                 AF.Sqrt, bias=eps_col[0:1, :], scale=mult)
                        nc.vector.reciprocal_approx_fast(srow[:], srow[:])
                        sbc = pb.tile([128, 1024], F32, tag="sbc", bufs=2)
                        nc.gpsimd.partition_broadcast(sbc[:], srow[:])
                        nc.vector.tensor_mul(dst[rh:rh + 48, 1024 * jj:1024 * jj + 1024],
                                             blk[rh:rh + 48, :], sbc[rh:rh + 48, :])
                    if kind == "k":
                        for ci in range(8):
                            pk = pst(128, 128, BF16)
                            nc.tensor.transpose(
                                pk[:], ksb[:, 1024 * jj + 128 * ci:1024 * jj + 128 * ci + 128],
                                idnb[:])
                            nc.scalar.copy(
                                ktok[:, 512 * ci + 128 * jj:512 * ci + 128 * jj + 128], pk[:])

        qkv_pass(wq_d, cqw, epsq, 48.0, "q", 0, 0)
        qkv_pass(wk_d, ckw, epsk, 1.0, "k", 0, 0)
        qkv_pass(wv_d, cvw, None, None, "v", 0, 0)
        qkv_pass(wv_d, cvw, None, None, "v", 4, 512)
        if DUMP == "qkv":
            dump_y([(j, qsb[:, 1024 * j:1024 * j + 1024]) for j in range(4)]
                   + [(4 + j, ksb[:, 1024 * j:1024 * j + 1024]) for j in range(4)]
                   + [(8 + b, vtok[:, 1024 * b:1024 * b + 1024]) for b in range(6)]
                   + [(14 + b, ktok[:, 1024 * b:1024 * b + 1024]) for b in range(2)])

        # gate token-major
        for n in range(2):
            pgs = [pst(128, 384) for _ in range(NTOK)]
            for k in range(KT):
                wt = wp.tile([128, 384], BF16, tag="wg384")
                nc.sync.dma_start(wt[:], wg_d[128 * k:128 * k + 128, 384 * n:384 * n + 384])
                for i in range(NTOK):
                    nc.tensor.matmul(
                        pgs[i][:], hT[:, 1024 * k + 128 * i:1024 * k + 128 * i + 128], wt[:],
                        start=(k == 0), stop=(k == KT - 1))
            for i in range(NTOK):
                nc.scalar.activation(
                    g_tok[:, VD_C * i + 384 * n:VD_C * i + 384 * n + 384], pgs[i][:], AF.Silu)

        # a/b projections, feature-major [16, 1024]
        ppab = [pst(16, 512) for _ in range(2)]
        for k in range(KT):
            wt = wp.tile([128, 16], BF16, tag="wab")
            nc.sync.dma_start(wt[:], wab_d[128 * k:128 * k + 128, :])
            for n in range(2):
                nc.tensor.matmul(ppab[n][:], wt[:], hT[:, 1024 * k + 512 * n:1024 * k + 512 * n + 512],
                                 start=(k == 0), stop=(k == KT - 1))
        for n in range(2):
            nc.vector.tensor_copy(ab_fm[:, 512 * n:512 * n + 512], ppab[n][:])

        # -------- batched decay prep for all chunks --------
        gta = dk.tile([128, 64], F32)
        bta = dk.tile([128, 64], F32)
        for ci in range(8):
            p_ab = pst(128, 16)
            nc.tensor.transpose(p_ab[:], ab_fm[:, 128 * ci:128 * ci + 128], idn[0:16, 0:16])
            nc.vector.tensor_copy(gta[:, 8 * ci:8 * ci + 8], p_ab[:, 0:8])
            nc.vector.tensor_copy(bta[:, 8 * ci:8 * ci + 8], p_ab[:, 8:16])
        nc.vector.tensor_add(gta[:], gta[:], dtb_bc[:])
        nc.scalar.activation(gta[:], gta[:], AF.Exp)
        nc.vector.tensor_scalar_add(gta[:], gta[:], 1.0)
        nc.scalar.activation(gta[:], gta[:], AF.Ln)
        nc.vector.tensor_mul(gta[:], gta[:], nega_bc[:])        # gt_all [128,64]
        beta_all = dk.tile([128, 64], F32)
        nc.scalar.activation(beta_all[:], bta[:], AF.Sigmoid)
        nbeta_all = dk.tile([128, 64], F32)
        nc.vector.tensor_scalar_mul(nbeta_all[:], beta_all[:], -1.0)
        p_bc = pst(128, 64)
        nc.tensor.matmul(p_bc[:], cum[:], gta[:], start=True, stop=True)
        bcum_tok = dk.tile([128, 64], F32)
        nc.vector.tensor_copy(bcum_tok[:], p_bc[:])
        lam_all = dk.tile([128, 64], F32)
        nc.scalar.activation(lam_all[:], p_bc[:], AF.Exp)
        nlam_all = dk.tile([128, 64], F32)
        nc.vector.tensor_scalar_mul(nlam_all[:], lam_all[:], -1.0)
        p_bf = pst(64, 128)
        nc.tensor.transpose(p_bf[:], bcum_tok[:], idn[:])
        b_fm = dk.tile([64, 128], F32)
        nc.vector.tensor_copy(b_fm[:], p_bf[:])
        nc.scalar.dma_start(bfm_scr[:], b_fm[:])
        wfm = dk.tile([64, 128], F32)
        nc.vector.tensor_scalar(wfm[:], b_fm[:], b_fm[:, 127:128], None, OP.subtract)
        nc.scalar.activation(wfm[:], wfm[:], AF.Exp, scale=-1.0)
        p_wt = pst(128, 64)
        nc.tensor.transpose(p_wt[:], wfm[:], idn[0:64, 0:64])
        w_tok = dk.tile([128, 64], F32)
        nc.vector.tensor_copy(w_tok[:], p_wt[:])
        ebc_all = dk.tile([64, 1], F32)
        nc.scalar.activation(ebc_all[:], b_fm[:, 127:128], AF.Exp)
        # EB[8ci+h, ci] = ebc_all[8ci+h]; ebcJ[j][p, ci] = per-(ci,j) state-decay col
        EB = dk.tile([64, 8], F32)
        nc.vector.tensor_scalar_mul(EB[:], chk[:], ebc_all[:, 0:1])
        ebcJ = []
        for j in range(4):
            p_ebj = pst(128, 8)
            nc.tensor.matmul(p_ebj[:], selj[:, 128 * j:128 * j + 128], EB[:],
                             start=True, stop=True)
            ej = dk.tile([128, 8], F32, tag=f"ebj{j}", name=f"ebj{j}")
            nc.vector.tensor_copy(ej[:], p_ebj[:])
            ebcJ.append(ej)
        _rel(pb)

        # ============ Phase C ============
        if "C" not in PHASES:
            raise _SkipRest()
        wod = _pool(name="wod", bufs=16)
        pd = _pool(name="pd", bufs=8)
        dput = _pool(name="dput", bufs=48)
        dpa = _pool(name="dpa", bufs=10)
        dpx = _pool(name="dpx", bufs=10)
        dpf = _pool(name="dpf", bufs=6)
        dp2 = _pool(name="dp2", bufs=2)
        spool = _pool(name="spool", bufs=2)

        def emit_D(half):
            t0 = 512 * half
            for db in range(16):
                wt = wod.tile([128, VD_C], BF16, tag="wo", name="wo_t", bufs=16)
                nc.scalar.dma_start(wt[:], wo_d[:, VD_C * db:VD_C * db + VD_C])
                pp = pst()
                for fb in range(6):
                    nc.tensor.matmul(pp[:], wt[:, 128 * fb:128 * fb + 128],
                                     osb[:, 1024 * fb + t0:1024 * fb + t0 + 512],
                                     start=(fb == 0), stop=(fb == 5))
                stg = pd.tile([128, 512], BF16, tag="s512b", name="stg", bufs=8)
                nc.scalar.copy(stg[:], pp[:])
                nc.sync.dma_start(oT_in[half][128 * db:128 * db + 128, :], stg[:])
            nc.gpsimd.collective_compute(
                "AllReduce", OP.add, ins=[oT_in[half][:]], outs=[oT_out[half][:]],
                replica_groups=groups)

        S_cur = {}
        for j in range(4):
            S_cur[j] = spool.tile([128, DV], F32, tag=f"s{j}", name=f"s{j}")
            nc.vector.memset(S_cur[j][:], 0.0)

        for ci in range(NCH):
            # ---- prep all 8 heads: abar, xx, xt ----
            ABAR, XX, XT = {}, {}, {}
            for j in range(4):
                for hh in range(2):
                    h = 2 * j + hh
                    rh = 64 * hh
                    kts = ksb[rh:rh + 48, 1024 * j + 128 * ci:1024 * j + 128 * ci + 128]
                    qts = qsb[rh:rh + 48, 1024 * j + 128 * ci:1024 * j + 128 * ci + 128]
                    p_kk = pst(128, 128)
                    nc.tensor.matmul(p_kk[:], kts, kts, start=True, stop=True)
                    p_kq = pst(128, 128)
                    nc.tensor.matmul(p_kq[:], kts, qts, start=True, stop=True)
                    bc128 = dpf.tile([128, 128], F32, tag="bc", name="bc128")
                    nc.gpsimd.dma_start(
                        bc128[:],
                        bfm_scr[8 * ci + h:8 * ci + h + 1, :].to_broadcast((128, 128)))
                    dte = dpf.tile([128, 128], F32, tag="dte", name="dte")
                    nc.vector.scalar_tensor_tensor(
                        dte[:], bc128[:], bcum_tok[:, 8 * ci + h:8 * ci + h + 1],
                        negl[:], OP.subtract, OP.add)
                    dincl = dput.tile([128, 128], BF16, tag="ut", name="dincl")
                    nc.scalar.activation(dincl[:], dte[:], AF.Exp)
                    abar = dpa.tile([128, 128], BF16, tag="abar", name="abar")
                    nc.vector.tensor_mul(abar[:], p_kq[:], dincl[:])
                    dstr = dput.tile([128, 128], BF16, tag="ut", name="dstr")
                    nc.gpsimd.tensor_mul(dstr[:], dincl[:], mstb[:])
                    x0 = dput.tile([128, 128], BF16, tag="ut", name="x0")
                    nc.vector.tensor_mul(x0[:], p_kk[:], dstr[:])
                    xx = dpx.tile([128, 128], BF16, tag="xx", name="xx")
                    nc.vector.tensor_scalar_mul(xx[:], x0[:],
                                                nbeta_all[:, 8 * ci + h:8 * ci + h + 1])
                    p_x = pst(128, 128, BF16)
                    nc.tensor.transpose(p_x[:], xx[:], idnb[:])
                    xt = dpx.tile([128, 128], BF16, tag="xt", name="xt")
                    nc.scalar.copy(xt[:], p_x[:])
                    ABAR[h], XX[h], XT[h] = abar, xx, xt

            # ---- UT inverse, level-major across all 8 heads ----
            PM = {}
            for h in range(HP):
                pmat = dput.tile([128, 128], BF16, tag="ut", name="pmat")
                nc.vector.tensor_add(pmat[:], XX[h][:], idnb[:])
                PM[h] = pmat
            cur = {h: (XX[h], XT[h]) for h in range(HP)}
            for lvl in range(6):
                last = lvl == 5
                nxt = {}
                for h in range(HP):
                    xx, xt = cur[h]
                    x2 = None
                    if not last:
                        p_sq = pst(128, 128)
                        nc.tensor.matmul(p_sq[:], xt[:], xx[:], start=True, stop=True)
                        x2 = dput.tile([128, 128], BF16, tag="ut", name="x2")
                        nc.scalar.copy(x2[:], p_sq[:])
                    p_sqt = pst(128, 128)
                    nc.tensor.matmul(p_sqt[:], xx[:], xt[:], start=True, stop=True)
                    xt2 = dput.tile([128, 128], BF16, tag="ut", name="xt2")
                    if h % 2 == 0:
                        nc.scalar.copy(xt2[:], p_sqt[:])
                    else:
                        nc.vector.tensor_copy(xt2[:], p_sqt[:])
                    nxt[h] = (x2, xt2)
                for h in range(HP):
                    p_pr = pst(128, 128)
                    nc.tensor.matmul(p_pr[:], nxt[h][1][:], PM[h][:], start=True, stop=True)
                    pnew = dput.tile([128, 128], BF16, tag="ut", name="pnew")
                    if h % 2 == 0:
                        nc.vector.tensor_add(pnew[:], PM[h][:], p_pr[:])
                    else:
                        nc.vector.tensor_add(pnew[:], p_pr[:], PM[h][:])
                    PM[h] = pnew
                if not last:
                    cur = nxt

            # ---- state/output, step-major in 4-head waves ----
            ot_all = dp2.tile([128, VD_C], F32, tag="otall", name="ot_all")
            for jp in (0, 2):
                heads = [(j, hh) for j in (jp, jp + 1) for hh in (0, 1)]
                SB, PS, KW = {}, {}, {}
                for j in (jp, jp + 1):
                    S_bf = dp2.tile([128, DV], BF16, tag=f"sbf{j}", name="S_bf")
                    nc.gpsimd.tensor_copy(S_bf[:], S_cur[j][:])
                    SB[j] = S_bf
                    PS[j] = pst(128, DV)
                for j, hh in heads:
                    h = 2 * j + hh
                    rh = 64 * hh
                    kw = dp2.tile([128, 48], BF16, tag="kw", name="kw", bufs=6)
                    nc.gpsimd.tensor_scalar_mul(
                        kw[:], ktok[:, 512 * ci + 128 * j + rh:512 * ci + 128 * j + rh + 48],
                        w_tok[:, 8 * ci + h:8 * ci + h + 1])
                    KW[h] = kw
                RR, UU = {}, {}
                for j, hh in heads:
                    h = 2 * j + hh
                    rh = 64 * hh
                    kts = ksb[rh:rh + 48, 1024 * j + 128 * ci:1024 * j + 128 * ci + 128]
                    p_ks = pst(128, DV)
                    nc.tensor.matmul(p_ks[:], kts, SB[j][rh:rh + 48, :], start=True, stop=True)
                    r_ = dp2.tile([128, DV], BF16, tag="rr", name="r_", bufs=5)
                    nc.vector.scalar_tensor_tensor(
                        r_[:], p_ks[:], nlam_all[:, 8 * ci + h:8 * ci + h + 1],
                        vtok[:, VD_C * ci + 96 * h:VD_C * ci + 96 * h + 96],
                        OP.mult, OP.add)
                    RR[h] = r_
                for j, hh in heads:
                    h = 2 * j + hh
                    p_w = pst(128, DV)
                    nc.tensor.matmul(p_w[:], PM[h][:], RR[h][:], start=True, stop=True)
                    u_ = dp2.tile([128, DV], BF16, tag="uu", name="u_", bufs=5)
                    nc.vector.tensor_scalar_mul(u_[:], p_w[:],
                                                beta_all[:, 8 * ci + h:8 * ci + h + 1])
                    UU[h] = u_
                for j, hh in heads:
                    h = 2 * j + hh
                    rh = 64 * hh
                    qts = qsb[rh:rh + 48, 1024 * j + 128 * ci:1024 * j + 128 * ci + 128]
                    p_oi = pst(128, DV)
                    nc.tensor.matmul(p_oi[:], ABAR[h][:], UU[h][:], start=True, stop=True)
                    p_qs = pst(128, DV)
                    nc.tensor.matmul(p_qs[:], qts, SB[j][rh:rh + 48, :], start=True, stop=True)
                    ots = ot_all[:, 96 * h:96 * h + 96]
                    nc.vector.tensor_scalar_mul(ots, p_qs[:],
                                                lam_all[:, 8 * ci + h:8 * ci + h + 1])
                    nc.vector.tensor_add(ots, ots, p_oi[:])
                    nc.tensor.matmul(PS[j][rh:rh + 48, :], KW[h][:], UU[h][:],
                                     start=True, stop=True)
                for j in (jp, jp + 1):
                    s_new = spool.tile([128, DV], F32, tag=f"s{j}", name="s_new")
                    for rh2 in (0, 64):
                        nc.vector.scalar_tensor_tensor(
                            s_new[rh2:rh2 + 48, :], S_cur[j][rh2:rh2 + 48, :],
                            ebcJ[j][rh2:rh2 + 48, ci:ci + 1],
                            PS[j][rh2:rh2 + 48, :], OP.mult, OP.add)
                    S_cur[j] = s_new

            # ---- batched gated rmsnorm + gate + transpose to osb ----
            osq = dp2.tile([128, VD_C], F32, tag="osq", name="osq")
            nc.vector.tensor_mul(osq[:], ot_all[:], ot_all[:])
            rcol8 = dp2.tile([128, 8], F32, tag="rc8", name="rcol8")
            for h in range(HP):
                nc.vector.tensor_reduce(rcol8[:, h:h + 1], osq[:, 96 * h:96 * h + 96],
                                        mybir.AxisListType.X, OP.add)
            nc.scalar.activation(rcol8[:], rcol8[:], AF.Sqrt, bias=epsg[:], scale=1.0 / DV)
            nc.vector.reciprocal_approx_fast(rcol8[:], rcol8[:])
            ob = dp2.tile([128, VD_C], BF16, tag="ob", name="ob")
            for h in range(HP):
                nc.vector.scalar_tensor_tensor(
                    ob[:, 96 * h:96 * h + 96], ot_all[:, 96 * h:96 * h + 96],
                    rcol8[:, h:h + 1], g_tok[:, VD_C * ci + 96 * h:VD_C * ci + 96 * h + 96],
                    OP.mult, OP.mult)
            for b6 in range(6):
                p_ot = pst(128, 128, BF16)
                nc.tensor.transpose(p_ot[:], ob[:, 128 * b6:128 * b6 + 128], idnb[:])
                nc.scalar.copy(osb[:, 1024 * b6 + 128 * ci:1024 * b6 + 128 * ci + 128], p_ot[:])

            # ---- o_projT half + AllReduce as soon as its tokens exist ----
            if ci == 3:
                emit_D(0)
            if ci == 7:
                emit_D(1)

        for p in (spool, dp2, dpf, dpx, dpa, dput, pd, wod):
            _rel(p)
        if DUMP == "o":
            dump_y([(b, osb[:, 1024 * b:1024 * b + 1024]) for b in range(6)]
                   + [(6 + b, g_tok[:, 1024 * b:1024 * b + 1024]) for b in range(6)]
                   + [(12 + b, ktok[:, 1024 * b:1024 * b + 1024]) for b in range(4)])
        if DUMP == "dk":
            dump_y([(0, gta[:]), (1, beta_all[:]), (2, bcum_tok[:]),
                    (3, lam_all[:]), (4, w_tok[:]), (5, b_fm[:]),
                    (6, ebc_all[:]), (7, ebcJ[0][:]), (8, ebcJ[3][:])])
        _rel(dk)
        _rel(bigq)

        if "D" not in PHASES:
            raise _SkipRest()
        if DUMP in ("ar", "oin"):
            src = oT_out if DUMP == "ar" else oT_in
            dbg = _pool(name="dbg", bufs=4)
            for bi in range(16):
                st = dbg.tile([128, 1024], F32, tag="dbg", name="st")
                for half in range(2):
                    so = dbg.tile([128, 512], BF16, tag="dbg2", name="so")
                    nc.sync.dma_start(so[:], src[half][128 * bi:128 * bi + 128, :])
                    nc.vector.tensor_copy(st[:, 512 * half:512 * half + 512], so[:])
                nc.sync.dma_start(y_d[128 * bi:128 * bi + 128, :], st[:])
            _rel(dbg)
            raise _Dumped()

        # ============ Phase E ============
        if "E" not in PHASES:
            raise _SkipRest()
        seq = _pool(name="seq", bufs=3)
        pe = _pool(name="pe", bufs=4)
        wp13 = _pool(name="wp13", bufs=3)
        wp2 = _pool(name="wp2", bufs=3)
        mida = pg.tile([128, 6 * 1024], BF16, tag="gtok")
        pmid = _pool(name="pmid", bufs=1)
        midb = pmid.tile([128, 5 * 1024], BF16, tag="midb")

        def mid_ap(m, off, ln):
            if m < 6:
                return mida[:, 1024 * m + off:1024 * m + off + ln]
            return midb[:, 1024 * (m - 6) + off:1024 * (m - 6) + off + ln]

        ffT = hT
        for half in range(2):
            hs = slice(512 * half, 512 * half + 512)
            p_s2 = pst(1, 512)
            for k in range(KT):
                xe = seq.tile([128, 512], F32, tag="xe")
                nc.sync.dma_start(xe[:], xT_d[128 * k:128 * k + 128, hs])
                oe = seq.tile([128, 512], BF16, tag="oe")
                nc.sync.dma_start(oe[:], oT_out[half][128 * k:128 * k + 128, :])
                h2 = seq.tile([128, 512], F32, tag="h2T")
                nc.vector.tensor_add(h2[:], xe[:], oe[:])
                nc.scalar.dma_start(h2T_scr[128 * k:128 * k + 128, hs], h2[:])
                sqe = seq.tile([128, 512], BF16, tag="sqe")
                nc.vector.tensor_mul(sqe[:], h2[:], h2[:])
                nc.tensor.matmul(p_s2[:], ones1[:], sqe[:],
                                 start=(k == 0), stop=(k == KT - 1))
            srowE = seq.tile([1, 512], F32, tag="srowE", bufs=2)
            nc.scalar.activation(srowE[:], p_s2[:], AF.Sqrt,
                                 bias=eps1[0:1, :], scale=1.0 / D)
            nc.vector.reciprocal_approx_fast(srowE[:], srowE[:])
            sbcE = seq.tile([128, 512], F32, tag="sbcE", bufs=2)
            nc.gpsimd.partition_broadcast(sbcE[:], srowE[:])
            for k in range(KT):
                h2r = seq.tile([128, 512], F32, tag="h2r", bufs=4)
                nc.sync.dma_start(h2r[:], h2T_scr[128 * k:128 * k + 128, hs])
                nc.vector.tensor_mul(ffT[:, 1024 * k + 512 * half:1024 * k + 512 * half + 512],
                                     h2r[:], sbcE[:])
            # W1/W3 for this half
            for m in range(11):
                pu1 = pst()
                pu3 = pst()
                wt13 = wp13.tile([128, 4096], BF16, tag="w13", name="wt13")
                nc.sync.dma_start(wt13[:], w13_d[:, 4096 * m:4096 * m + 4096])
                for k in range(KT):
                    rhs = ffT[:, 1024 * k + 512 * half:1024 * k + 512 * half + 512]
                    nc.tensor.matmul(pu1[:], wt13[:, 256 * k:256 * k + 128], rhs,
                                     start=(k == 0), stop=(k == KT - 1))
                    nc.tensor.matmul(pu3[:], wt13[:, 256 * k + 128:256 * k + 256], rhs,
                                     start=(k == 0), stop=(k == KT - 1))
                u1s = pe.tile([128, 512], F32, tag="s512", name="u1s")
                nc.scalar.activation(u1s[:], pu1[:], AF.Silu)
                nc.vector.tensor_mul(mid_ap(m, 512 * half, 512), u1s[:], pu3[:])
            if DUMP == "mid" and half == 1:
                dump_y([(m, mid_ap(m, 0, 1024)) for m in range(11)])
            # W2 for this half
            for db in range(16):
                wt2 = wp2.tile([128, INT_C], BF16, tag="w2", name="w2_t")
                nc.sync.dma_start(wt2[:], w2_d[:, INT_C * db:INT_C * db + INT_C])
                pps = pst()
                for m in range(11):
                    nc.tensor.matmul(pps[:], wt2[:, 128 * m:128 * m + 128],
                                     mid_ap(m, 512 * half, 512),
                                     start=(m == 0), stop=(m == 10))
                h2t = pe.tile([128, 512], F32, tag="s512", name="h2t")
                nc.sync.dma_start(h2t[:], h2T_scr[128 * db:128 * db + 128, hs])
                yst = pe.tile([128, 512], F32, tag="s512", name="yst")
                nc.vector.tensor_scalar_mul(yst[:], h2t[:], 0.25)
                nc.vector.tensor_add(yst[:], yst[:], pps[:])
                nc.sync.dma_start(y_d[128 * db:128 * db + 128, hs], yst[:])

        for p in (pmid, wp2, wp13, pe, seq, dram, wp, pg, big, ps, cpool):
            _rel(p)
      except _SkipRest:
        zst = _pool(name="zst", bufs=1)
        zt = zst.tile([128, 512], F32)
        nc.vector.memset(zt[:], 0.0)
        for i in range(16):
            for dh in range(2):
                nc.sync.dma_start(y_d[128 * i:128 * i + 128, 512 * dh:512 * dh + 512], zt[:])
        for p in reversed(live_pools):
            p.release()
      except _Dumped:
        for p in reversed(live_pools):
            p.release()

    nc.compile()
    return nc


def _pack_wo(wo):
    # [768, 2048] -> [128, 16*768]: col = 768*db + 128*fb + c
    return np.ascontiguousarray(
        wo.reshape(6, 128, 16, 128).transpose(1, 2, 0, 3).reshape(128, 16 * 768))


def _pack_w13(w1, w3):
    # [2048, 1408] x2 -> [128, 11*4096]: col = 4096*m + 256*k + 128*which + c
    a = w1.reshape(16, 128, 11, 128).transpose(1, 2, 0, 3)   # [128, 11, 16, 128]
    b = w3.reshape(16, 128, 11, 128).transpose(1, 2, 0, 3)
    return np.ascontiguousarray(
        np.stack([a, b], axis=3).reshape(128, 11 * 4096))


def _pack_w2(w2):
    # [1408, 2048] -> [128, 16*1408]: col = 1408*db + 128*m + c
    return np.ascontiguousarray(
        w2.reshape(11, 128, 16, 128).transpose(1, 2, 0, 3).reshape(128, 16 * 1408))


def _shard(inputs):
    f32 = np.float32
    bf = ml_dtypes.bfloat16
    rms1 = np.asarray(inputs["rms1_w"], f32)
    rms2 = np.asarray(inputs["rms2_w"], f32)
    gn = np.asarray(inputs["gnorm_w"], f32)
    in_maps = []
    for c in range(8):
        g, m = c // 4, c % 4
        qs = slice(384 * m, 384 * m + 384)
        vs = slice(768 * m, 768 * m + 768)
        hs = slice(8 * m, 8 * m + 8)
        isl = slice(1408 * m, 1408 * m + 1408)

        def padqk(w):
            wp_ = np.zeros((D, QKP), f32)
            for h in range(8):
                wp_[:, 64 * h:64 * h + 48] = w[:, 48 * h:48 * h + 48]
            return wp_

        def padcw(w):
            cp = np.zeros((QKP, 4), f32)
            for h in range(8):
                cp[64 * h:64 * h + 48] = w[48 * h:48 * h + 48]
            return cp

        def padv(w):
            colpad = w.shape[0] == D
            out = np.zeros((D, VP) if colpad else (VP, w.shape[1]), f32)
            for h in range(8):
                if colpad:
                    out[:, 128 * h:128 * h + 96] = w[:, 96 * h:96 * h + 96]
                else:
                    out[128 * h:128 * h + 96] = w[96 * h:96 * h + 96]
            return out

        dtb8 = np.asarray(inputs["dt_bias"], f32)[hs]
        nega8 = -np.exp(np.asarray(inputs["A_log"], f32)[hs])
        in_maps.append(dict(
            xT=np.ascontiguousarray(np.asarray(inputs["x"], f32)[g].T),
            wq=padqk(np.asarray(inputs["Wq"], f32)[:, qs] * rms1[:, None]).astype(bf),
            wk=padqk(np.asarray(inputs["Wk"], f32)[:, qs] * rms1[:, None]).astype(bf),
            wv=padv(np.asarray(inputs["Wv"], f32)[:, vs] * rms1[:, None]).astype(bf),
            wg=np.ascontiguousarray(
                np.asarray(inputs["Wg"], f32)[:, vs] * rms1[:, None]).astype(bf),
            wab=np.ascontiguousarray(np.concatenate(
                [np.asarray(inputs["Wa"], f32)[:, hs],
                 np.asarray(inputs["Wb"], f32)[:, hs]], 1) * rms1[:, None]).astype(bf),
            cq=padcw(np.asarray(inputs["conv_q_w"], f32)[qs]),
            ck=padcw(np.asarray(inputs["conv_k_w"], f32)[qs]),
            cv=padv(np.asarray(inputs["conv_v_w"], f32)[vs]),
            dtb=np.tile(dtb8, 8).reshape(1, 64).copy(),
            nega=np.tile(nega8, 8).reshape(1, 64).copy(),
            wo=_pack_wo(np.asarray(inputs["Wo"], f32)[vs] * np.tile(gn, 8)[:, None]).astype(bf),
            w13=_pack_w13(np.asarray(inputs["W1"], f32)[:, isl] * rms2[:, None],
                          np.asarray(inputs["W3"], f32)[:, isl] * rms2[:, None]).astype(bf),
            w2=_pack_w2(np.asarray(inputs["W2"], f32)[isl]).astype(bf),
        ))
    return in_maps


def kernel(**inputs):
    if "nc" not in _cache:
        _cache["nc"] = _build(8)
    res = run_bass_kernel_spmd(_cache["nc"], _shard(inputs), list(range(8)))
    out = np.zeros((B, T, D), np.float32)
    for g in range(2):
        yT = sum(res.results[4 * g + m]["y"] for m in range(4))
        out[g] = yT.T
    return out


# revision 49
# speedup vs baseline: 1.9119x; 1.0060x over previous
"""GatedDeltaNet block kernel for 8 Trainium2 cores (Bass/Tile), bf16 rework.

Sharding: DP2 (batch) x TP4 (heads / MLP-inter). Core c: group g=c//4 runs
batch g; member m=c%4 owns heads [8m,8m+8), q/k cols [384m,..), v/g cols
[768m,..), INTER [1408m,..). Two half-token AllReduces per 4-core group after
o_proj (overlapped with o_proj compute); final down-proj partials summed on
the host.

Everything runs feature-major (host passes x transposed, takes y transposed)
so there are no PE transposes outside the delta-rule inner loop. All big
GEMM operands are bf16 (host-cast weights); psum accumulation, the delta-rule
state, decay/beta math and norms stay fp32.

Per-core dataflow:
  A: xT [D,T] -> rmsnorm via matmul-accumulated column sumsq -> hT bf16 (SBUF)
  B: bf16 projections off hT; q/k feature-major (heads padded to 64 rows)
     -> conv+silu+l2norm -> SBUF (+ token-major copies of k, v via PE
     transposes); gate token-major; a/b -> batched decay prep for all chunks
  C: chunked gated delta rule (C=128, UT transform via log-doubling inverse,
     bf16 matmuls / fp32 state); writes normed+gated o feature-major to SBUF
  D: o_projT in two token halves, each followed by its AllReduce (overlapped)
  E: h2T = xT + oT; rmsnorm -> ffT bf16 (reuses hT); MLP bf16; yT partials
"""
import sys
sys.path.insert(0, '/opt/trn_rl_repo')
import numpy as np
import ml_dtypes

import concourse.bass as bass
import concourse.bacc as bacc
import concourse.mybir as mybir
import concourse.tile as tile
from concourse.bass_isa import ReduceOp
from concourse.bass_utils import run_bass_kernel_spmd

F32 = mybir.dt.float32
BF16 = mybir.dt.bfloat16
AF = mybir.ActivationFunctionType
OP = mybir.AluOpType

B, T, D = 2, 1024, 2048
H, DK, DV = 32, 48, 96
HP = 8
QKP = 512
VD_C = 768
VP = 1024
INT_C = 1408
C = 128
NCHUNK = T // C
KT = D // 128
NTOK = T // 128

_cache = {}
import os
PHASES = os.environ.get("DN_PHASES", "ABCDE")
NCH = int(os.environ.get("DN_NCHUNK", str(T // C)))
DUMP = os.environ.get("DN_DUMP", "")


class _SkipRest(Exception):
    pass


class _Dumped(Exception):
    pass


def _build(n_cores=8):
    groups = [[0, 1, 2, 3], [4, 5, 6, 7]] if n_cores == 8 else [[0]]
    nc = bacc.Bacc("TRN2", target_bir_lowering=False, debug=False, num_devices=n_cores)

    xT_d = nc.dram_tensor("xT", [D, T], F32, kind="ExternalInput")
    wq_d = nc.dram_tensor("wq", [D, QKP], BF16, kind="ExternalInput")
    wk_d = nc.dram_tensor("wk", [D, QKP], BF16, kind="ExternalInput")
    wv_d = nc.dram_tensor("wv", [D, VP], BF16, kind="ExternalInput")
    wg_d = nc.dram_tensor("wg", [D, VD_C], BF16, kind="ExternalInput")
    wab_d = nc.dram_tensor("wab", [D, 16], BF16, kind="ExternalInput")
    cq_d = nc.dram_tensor("cq", [QKP, 4], F32, kind="ExternalInput")
    ck_d = nc.dram_tensor("ck", [QKP, 4], F32, kind="ExternalInput")
    cv_d = nc.dram_tensor("cv", [VP, 4], F32, kind="ExternalInput")
    dtb_d = nc.dram_tensor("dtb", [1, 64], F32, kind="ExternalInput")
    nega_d = nc.dram_tensor("nega", [1, 64], F32, kind="ExternalInput")
    wo_d = nc.dram_tensor("wo", [128, 16 * VD_C], BF16, kind="ExternalInput")
    w13_d = nc.dram_tensor("w13", [128, 11 * 4096], BF16, kind="ExternalInput")
    w2_d = nc.dram_tensor("w2", [128, 16 * INT_C], BF16, kind="ExternalInput")
    y_d = nc.dram_tensor("y", [D, T], F32, kind="ExternalOutput")

    ones = np.ones((128, 128), np.float32)
    idn_c = nc.inline_tensor(np.eye(128, dtype=np.float32), "idn_c")
    idnb_c = nc.inline_tensor(np.eye(128, dtype=ml_dtypes.bfloat16), "idnb_c")
    cum_c = nc.inline_tensor(np.triu(ones).copy(), "cum_c")
    mst_c = nc.inline_tensor(np.triu(ones, 1).astype(ml_dtypes.bfloat16), "mst_c")
    msi_c = nc.inline_tensor(np.triu(ones).copy(), "msi_c")
    negl_c = nc.inline_tensor((np.tril(ones, -1) * -1e30).copy(), "negl_c")
    # SELJ[r, 128j+p] = 1 iff (r%8==2j and p<48) or (r%8==2j+1 and 64<=p<112)
    selj_np = np.zeros((64, 512), np.float32)
    for r in range(64):
        for j in range(4):
            if r % 8 == 2 * j:
                selj_np[r, 128 * j:128 * j + 48] = 1.0
            if r % 8 == 2 * j + 1:
                selj_np[r, 128 * j + 64:128 * j + 112] = 1.0
    selj_c = nc.inline_tensor(selj_np, "selj_c")
    # CHK[8ci+h, ci] = 1
    chk_np = np.zeros((64, 8), np.float32)
    for ci in range(8):
        chk_np[8 * ci:8 * ci + 8, ci] = 1.0
    chk_c = nc.inline_tensor(chk_np, "chk_c")
    on48_np = np.zeros((128, 2), ml_dtypes.bfloat16)
    on48_np[0:48, 0] = 1.0
    on48_np[64:112, 1] = 1.0
    on48_c = nc.inline_tensor(on48_np, "on48_c")
    ones1_np = np.ones((128, 1), ml_dtypes.bfloat16)
    ones1_c = nc.inline_tensor(ones1_np, "ones1_c")

    with tile.TileContext(nc) as tc:
      live_pools = []

      def _pool(**kw):
          p = tc.alloc_tile_pool(**kw)
          live_pools.append(p)
          return p

      def _rel(p):
          p.release()
          live_pools.remove(p)

      try:
        cpool = _pool(name="consts", bufs=1)
        ps = _pool(name="ps", bufs=8, space="PSUM")

        def pst(p=128, f=512, dt=F32):
            return ps.tile([p, f], dt, tag="ps", name="pst")

        def dump_y(items):
            # items: list of (y_block_index, ap [p, <=1024]) — copy (cast) to y
            dbg = _pool(name="dbg", bufs=4)
            for bi, ap in items:
                p, n = ap.shape[0], ap.shape[1]
                st = dbg.tile([128, 1024], F32, tag="dbg", name="st")
                nc.vector.tensor_copy(st[0:p, 0:n], ap)
                nc.sync.dma_start(y_d[128 * bi:128 * bi + p, 0:n], st[0:p, 0:n])
            _rel(dbg)
            raise _Dumped()

        idn = cpool.tile([128, 128], F32)
        idnb = cpool.tile([128, 128], BF16)
        cum = cpool.tile([128, 128], F32)
        mstb = cpool.tile([128, 128], BF16)
        msi = cpool.tile([128, 128], F32)
        negl = cpool.tile([128, 128], F32)
        selj = cpool.tile([64, 512], F32)
        chk = cpool.tile([64, 8], F32)
        on48 = cpool.tile([128, 2], BF16)
        ones1 = cpool.tile([128, 1], BF16)
        for t_, s_ in [(idn, idn_c), (idnb, idnb_c), (cum, cum_c), (mstb, mst_c),
                       (msi, msi_c), (negl, negl_c), (selj, selj_c), (chk, chk_c),
                       (on48, on48_c), (ones1, ones1_c)]:
            nc.sync.dma_start(t_[:], s_[:])
        eps1 = cpool.tile([128, 1], F32)
        nc.vector.memset(eps1[:], 1e-5)
        epsq = cpool.tile([128, 1], F32)
        nc.vector.memset(epsq[:], 48e-6)
        epsk = cpool.tile([128, 1], F32)
        nc.vector.memset(epsk[:], 1e-6)
        epsg = cpool.tile([128, 1], F32)
        nc.vector.memset(epsg[:], 1e-5)
        dtb_r = cpool.tile([1, 64], F32)
        nega_r = cpool.tile([1, 64], F32)
        nc.sync.dma_start(dtb_r[:], dtb_d[:])
        nc.sync.dma_start(nega_r[:], nega_d[:])
        dtb_bc = cpool.tile([128, 64], F32)
        nega_bc = cpool.tile([128, 64], F32)
        nc.gpsimd.partition_broadcast(dtb_bc[:], dtb_r[:])
        nc.gpsimd.partition_broadcast(nega_bc[:], nega_r[:])
        cqw = cpool.tile([128, 16], F32)
        ckw = cpool.tile([128, 16], F32)
        cvw = cpool.tile([128, 32], F32)
        for j in range(4):
            nc.sync.dma_start(cqw[:, 4 * j:4 * j + 4], cq_d[128 * j:128 * j + 128, :])
            nc.sync.dma_start(ckw[:, 4 * j:4 * j + 4], ck_d[128 * j:128 * j + 128, :])
        for j in range(8):
            nc.sync.dma_start(cvw[:, 4 * j:4 * j + 4], cv_d[128 * j:128 * j + 128, :])
        ab_fm = cpool.tile([16, 1024], F32)

        big = _pool(name="big", bufs=1)
        hT = big.tile([128, KT * 1024], BF16)       # also ffT in phase E
        osb = big.tile([128, 6 * 1024], BF16)       # feature-major o: [feat%128, 1024*(f//128)+tok]
        pg = _pool(name="pg", bufs=1)
        g_tok = pg.tile([128, NTOK * VD_C], BF16, tag="gtok")

        wp = _pool(name="wp", bufs=4)
        dram = _pool(name="dram", bufs=1, space="DRAM")
        _ow = [512, 256, 256]
        oT_in = [dram.tile([D, _ow[i]], BF16, name=f"oT_in{i}") for i in range(3)]
        oT_out = [dram.tile([D, _ow[i]], BF16, name=f"oT_out{i}") for i in range(3)]
        h2T_scr = dram.tile([D, T], F32)
        bfm_scr = dram.tile([64, 128], F32)

        bigq = _pool(name="bigq", bufs=1)
        qsb = bigq.tile([128, 4 * 1024], BF16)
        ksb = bigq.tile([128, 4 * 1024], BF16)
        ktok = bigq.tile([128, 8 * 512], BF16)      # token-major k: [tok, 512ci+128j]
        vtok = bigq.tile([128, 8 * VD_C], BF16)     # token-major v: [tok, 768ci+96h]
        nc.vector.memset(qsb[:], 0.0)
        nc.vector.memset(ksb[:], 0.0)

        # ============ Phase A: hT = rmsnorm(x)^T in bf16 ============
        stA = _pool(name="stA", bufs=16)
        sqp = _pool(name="sqp", bufs=3)
        p_ss = [pst(1, 512) for _ in range(2)]
        xts = []
        for k in range(KT):
            xa = stA.tile([128, 1024], F32, tag="xT")
            nc.sync.dma_start(xa[:], xT_d[128 * k:128 * k + 128, :])
            xts.append(xa)
            sq = sqp.tile([128, 1024], BF16, tag="sq")
            nc.vector.tensor_mul(sq[:], xa[:], xa[:])
            for n in range(2):
                nc.tensor.matmul(p_ss[n][:], ones1[:], sq[:, 512 * n:512 * n + 512],
                                 start=(k == 0), stop=(k == KT - 1))
        srowA = sqp.tile([1, 1024], F32, tag="srowA", bufs=1)
        for n in range(2):
            nc.scalar.activation(srowA[:, 512 * n:512 * n + 512], p_ss[n][:],
                                 AF.Sqrt, bias=eps1[0:1, :], scale=1.0 / D)
        nc.vector.reciprocal_approx_fast(srowA[:], srowA[:])
        sbcA = sqp.tile([128, 1024], F32, tag="sbcA", bufs=1)
        nc.gpsimd.partition_broadcast(sbcA[:], srowA[:])
        for k in range(KT):
            nc.vector.tensor_mul(hT[:, 1024 * k:1024 * k + 1024], xts[k][:], sbcA[:])
        _rel(sqp)
        _rel(stA)
        if DUMP == "hT":
            dump_y([(k, hT[:, 1024 * k:1024 * k + 1024]) for k in range(KT)])

        # ============ Phase B ============
        if "B" not in PHASES:
            raise _SkipRest()
        dk = _pool(name="dk", bufs=1)
        pb = _pool(name="pb", bufs=6)

        def conv_acc(pre, cw, j):
            acc = pb.tile([128, 1024], F32, tag="s1k")
            nc.scalar.activation(acc[:], pre[:], AF.Copy, scale=cw[:, 4 * j + 3:4 * j + 4])
            for s in (1, 2, 3):
                nc.vector.scalar_tensor_tensor(
                    acc[:, s:1024], pre[:, 0:1024 - s],
                    cw[:, 4 * j + 3 - s:4 * j + 4 - s], acc[:, s:1024],
                    OP.mult, OP.add)
            return acc

        def qkv_pass(w_dram, cw, eps_col, mult, kind, jbase, wcol0):
            pps = [[pst() for n in range(2)] for j in range(4)]
            for k in range(KT):
                wt = wp.tile([128, 512], BF16, tag="wwide")
                nc.sync.dma_start(wt[:], w_dram[128 * k:128 * k + 128, wcol0:wcol0 + 512])
                for j in range(4):
                    for n in range(2):
                        nc.tensor.matmul(
                            pps[j][n][:], wt[:, 128 * j:128 * j + 128],
                            hT[:, 1024 * k + 512 * n:1024 * k + 512 * n + 512],
                            start=(k == 0), stop=(k == KT - 1))
            for j in range(4):
                jj = jbase + j
                pre = pb.tile([128, 1024], F32, tag="s1k")
                for n in range(2):
                    nc.vector.tensor_copy(pre[:, 512 * n:512 * n + 512], pps[j][n][:])
                acc = conv_acc(pre, cw, jj)
                if kind == "v":
                    vb = pb.tile([128, 1024], BF16, tag="vb16", bufs=2)
                    nc.scalar.activation(vb[:], acc[:], AF.Silu)
                    for ci in range(8):
                        pv = pst(128, 96, BF16)
                        nc.tensor.transpose(pv[:], vb[0:96, 128 * ci:128 * ci + 128],
                                            idnb[0:96, 0:96])
                        nc.scalar.copy(
                            vtok[:, VD_C * ci + 96 * jj:VD_C * ci + 96 * jj + 96], pv[:])
                else:
                    blk = pb.tile([128, 1024], F32, tag="s1k")
                    nc.scalar.activation(blk[:], acc[:], AF.Silu)
                    sq = pb.tile([128, 1024], BF16, tag="sqb", bufs=2)
                    nc.vector.tensor_mul(sq[:], blk[:], blk[:])
                    dst = qsb if kind == "q" else ksb
                    for hh, rh in ((0, 0), (1, 64)):
                        srow = pb.tile([1, 1024], F32, tag="srow", bufs=2)
                        for n2 in range(2):
                            p_ssq = pst(1, 512)
                            nc.tensor.matmul(
                                p_ssq[:], on48[:, hh:hh + 1], sq[:, 512 * n2:512 * n2 + 512],
                                start=True, stop=True)
                            nc.scalar.activation(srow[:, 512 * n2:512 * n2 + 512], p_ssq[:],
                                                 AF.Sqrt, bias=eps_col[0:1, :], scale=mult)
                        nc.vector.reciprocal_approx_fast(srow[:], srow[:])
                        sbc = pb.tile([128, 1024], F32, tag="sbc", bufs=2)
                        nc.gpsimd.partition_broadcast(sbc[:], srow[:])
                        nc.vector.tensor_mul(dst[rh:rh + 48, 1024 * jj:1024 * jj + 1024],
                                             blk[rh:rh + 48, :], sbc[rh:rh + 48, :])
                    if kind == "k":
                        for ci in range(8):
                            pk = pst(128, 128, BF16)
                            nc.tensor.transpose(
                                pk[:], ksb[:, 1024 * jj + 128 * ci:1024 * jj + 128 * ci + 128],
                                idnb[:])
                            nc.scalar.copy(
                                ktok[:, 512 * ci + 128 * jj:512 * ci + 128 * jj + 128], pk[:])

        qkv_pass(wq_d, cqw, epsq, 48.0, "q", 0, 0)
        qkv_pass(wk_d, ckw, epsk, 1.0, "k", 0, 0)
        qkv_pass(wv_d, cvw, None, None, "v", 0, 0)
        qkv_pass(wv_d, cvw, None, None, "v", 4, 512)
        if DUMP == "qkv":
            dump_y([(j, qsb[:, 1024 * j:1024 * j + 1024]) for j in range(4)]
                   + [(4 + j, ksb[:, 1024 * j:1024 * j + 1024]) for j in range(4)]
                   + [(8 + b, vtok[:, 1024 * b:1024 * b + 1024]) for b in range(6)]
                   + [(14 + b, ktok[:, 1024 * b:1024 * b + 1024]) for b in range(2)])

        # gate token-major
        for n in range(2):
            pgs = [pst(128, 384) for _ in range(NTOK)]
            for k in range(KT):
                wt = wp.tile([128, 384], BF16, tag="wg384")
                nc.sync.dma_start(wt[:], wg_d[128 * k:128 * k + 128, 384 * n:384 * n + 384])
                for i in range(NTOK):
                    nc.tensor.matmul(
                        pgs[i][:], hT[:, 1024 * k + 128 * i:1024 * k + 128 * i + 128], wt[:],
                        start=(k == 0), stop=(k == KT - 1))
            for i in range(NTOK):
                nc.scalar.activation(
                    g_tok[:, VD_C * i + 384 * n:VD_C * i + 384 * n + 384], pgs[i][:], AF.Silu)

        # a/b projections, feature-major [16, 1024]
        ppab = [pst(16, 512) for _ in range(2)]
        for k in range(KT):
            wt = wp.tile([128, 16], BF16, tag="wab")
            nc.sync.dma_start(wt[:], wab_d[128 * k:128 * k + 128, :])
            for n in range(2):
                nc.tensor.matmul(ppab[n][:], wt[:], hT[:, 1024 * k + 512 * n:1024 * k + 512 * n + 512],
                                 start=(k == 0), stop=(k == KT - 1))
        for n in range(2):
            nc.vector.tensor_copy(ab_fm[:, 512 * n:512 * n + 512], ppab[n][:])

        # -------- batched decay prep for all chunks --------
        gta = dk.tile([128, 64], F32)
        bta = dk.tile([128, 64], F32)
        for ci in range(8):
            p_ab = pst(128, 16)
            nc.tensor.transpose(p_ab[:], ab_fm[:, 128 * ci:128 * ci + 128], idn[0:16, 0:16])
            nc.vector.tensor_copy(gta[:, 8 * ci:8 * ci + 8], p_ab[:, 0:8])
            nc.vector.tensor_copy(bta[:, 8 * ci:8 * ci + 8], p_ab[:, 8:16])
        nc.vector.tensor_add(gta[:], gta[:], dtb_bc[:])
        nc.scalar.activation(gta[:], gta[:], AF.Exp)
        nc.vector.tensor_scalar_add(gta[:], gta[:], 1.0)
        nc.scalar.activation(gta[:], gta[:], AF.Ln)
        nc.vector.tensor_mul(gta[:], gta[:], nega_bc[:])        # gt_all [128,64]
        beta_all = dk.tile([128, 64], F32)
        nc.scalar.activation(beta_all[:], bta[:], AF.Sigmoid)
        nbeta_all = dk.tile([128, 64], F32)
        nc.vector.tensor_scalar_mul(nbeta_all[:], beta_all[:], -1.0)
        p_bc = pst(128, 64)
        nc.tensor.matmul(p_bc[:], cum[:], gta[:], start=True, stop=True)
        bcum_tok = dk.tile([128, 64], F32)
        nc.vector.tensor_copy(bcum_tok[:], p_bc[:])
        lam_all = dk.tile([128, 64], F32)
        nc.scalar.activation(lam_all[:], p_bc[:], AF.Exp)
        nlam_all = dk.tile([128, 64], F32)
        nc.vector.tensor_scalar_mul(nlam_all[:], lam_all[:], -1.0)
        p_bf = pst(64, 128)
        nc.tensor.transpose(p_bf[:], bcum_tok[:], idn[:])
        b_fm = dk.tile([64, 128], F32)
        nc.vector.tensor_copy(b_fm[:], p_bf[:])
        nc.scalar.dma_start(bfm_scr[:], b_fm[:])
        wfm = dk.tile([64, 128], F32)
        nc.vector.tensor_scalar(wfm[:], b_fm[:], b_fm[:, 127:128], None, OP.subtract)
        nc.scalar.activation(wfm[:], wfm[:], AF.Exp, scale=-1.0)
        p_wt = pst(128, 64)
        nc.tensor.transpose(p_wt[:], wfm[:], idn[0:64, 0:64])
        w_tok = dk.tile([128, 64], F32)
        nc.vector.tensor_copy(w_tok[:], p_wt[:])
        ebc_all = dk.tile([64, 1], F32)
        nc.scalar.activation(ebc_all[:], b_fm[:, 127:128], AF.Exp)
        # EB[8ci+h, ci] = ebc_all[8ci+h]; ebcJ[j][p, ci] = per-(ci,j) state-decay col
        EB = dk.tile([64, 8], F32)
        nc.vector.tensor_scalar_mul(EB[:], chk[:], ebc_all[:, 0:1])
        ebcJ = []
        for j in range(4):
            p_ebj = pst(128, 8)
            nc.tensor.matmul(p_ebj[:], selj[:, 128 * j:128 * j + 128], EB[:],
                             start=True, stop=True)
            ej = dk.tile([128, 8], F32, tag=f"ebj{j}", name=f"ebj{j}")
            nc.vector.tensor_copy(ej[:], p_ebj[:])
            ebcJ.append(ej)
        _rel(pb)

        # ============ Phase C ============
        if "C" not in PHASES:
            raise _SkipRest()
        wod = _pool(name="wod", bufs=16)
        pd = _pool(name="pd", bufs=8)
        dput = _pool(name="dput", bufs=48)
        dpa = _pool(name="dpa", bufs=10)
        dpx = _pool(name="dpx", bufs=10)
        dpf = _pool(name="dpf", bufs=6)
        dp2 = _pool(name="dp2", bufs=2)
        spool = _pool(name="spool", bufs=2)

        def emit_D(part, t0, width):
            for db in range(16):
                wt = wod.tile([128, VD_C], BF16, tag="wo", name="wo_t", bufs=16)
                nc.scalar.dma_start(wt[:], wo_d[:, VD_C * db:VD_C * db + VD_C])
                pp = pst(128, width)
                for fb in range(6):
                    nc.tensor.matmul(pp[:], wt[:, 128 * fb:128 * fb + 128],
                                     osb[:, 1024 * fb + t0:1024 * fb + t0 + width],
                                     start=(fb == 0), stop=(fb == 5))
                stg = pd.tile([128, width], BF16, tag="s512b", name="stg", bufs=8)
                nc.scalar.copy(stg[:], pp[:])
                nc.sync.dma_start(oT_in[part][128 * db:128 * db + 128, :], stg[:])
            nc.gpsimd.collective_compute(
                "AllReduce", OP.add, ins=[oT_in[part][:]], outs=[oT_out[part][:]],
                replica_groups=groups)

        S_cur = {}
        for j in range(4):
            S_cur[j] = spool.tile([128, DV], F32, tag=f"s{j}", name=f"s{j}")
            nc.vector.memset(S_cur[j][:], 0.0)

        for ci in range(NCH):
            # ---- prep all 8 heads: abar, xx, xt ----
            ABAR, XX, XT = {}, {}, {}
            for j in range(4):
                for hh in range(2):
                    h = 2 * j + hh
                    rh = 64 * hh
                    kts = ksb[rh:rh + 48, 1024 * j + 128 * ci:1024 * j + 128 * ci + 128]
                    qts = qsb[rh:rh + 48, 1024 * j + 128 * ci:1024 * j + 128 * ci + 128]
                    p_kk = pst(128, 128)
                    nc.tensor.matmul(p_kk[:], kts, kts, start=True, stop=True)
                    p_kq = pst(128, 128)
                    nc.tensor.matmul(p_kq[:], kts, qts, start=True, stop=True)
                    bc128 = dpf.tile([128, 128], F32, tag="bc", name="bc128")
                    nc.gpsimd.dma_start(
                        bc128[:],
                        bfm_scr[8 * ci + h:8 * ci + h + 1, :].to_broadcast((128, 128)))
                    dte = dpf.tile([128, 128], F32, tag="dte", name="dte")
                    nc.vector.scalar_tensor_tensor(
                        dte[:], bc128[:], bcum_tok[:, 8 * ci + h:8 * ci + h + 1],
                        negl[:], OP.subtract, OP.add)
                    dincl = dput.tile([128, 128], BF16, tag="ut", name="dincl")
                    nc.scalar.activation(dincl[:], dte[:], AF.Exp)
                    abar = dpa.tile([128, 128], BF16, tag="abar", name="abar")
                    nc.vector.tensor_mul(abar[:], p_kq[:], dincl[:])
                    dstr = dput.tile([128, 128], BF16, tag="ut", name="dstr")
                    nc.gpsimd.tensor_mul(dstr[:], dincl[:], mstb[:])
                    x0 = dput.tile([128, 128], BF16, tag="ut", name="x0")
                    nc.vector.tensor_mul(x0[:], p_kk[:], dstr[:])
                    xx = dpx.tile([128, 128], BF16, tag="xx", name="xx")
                    nc.vector.tensor_scalar_mul(xx[:], x0[:],
                                                nbeta_all[:, 8 * ci + h:8 * ci + h + 1])
                    p_x = pst(128, 128, BF16)
                    nc.tensor.transpose(p_x[:], xx[:], idnb[:])
                    xt = dpx.tile([128, 128], BF16, tag="xt", name="xt")
                    nc.scalar.copy(xt[:], p_x[:])
                    ABAR[h], XX[h], XT[h] = abar, xx, xt

            # ---- UT inverse, level-major across all 8 heads ----
            PM = {}
            for h in range(HP):
                pmat = dput.tile([128, 128], BF16, tag="ut", name="pmat")
                nc.vector.tensor_add(pmat[:], XX[h][:], idnb[:])
                PM[h] = pmat
            cur = {h: (XX[h], XT[h]) for h in range(HP)}
            for lvl in range(6):
                last = lvl == 5
                nxt = {}
                for h in range(HP):
                    xx, xt = cur[h]
                    x2 = None
                    if not last:
                        p_sq = pst(128, 128)
                        nc.tensor.matmul(p_sq[:], xt[:], xx[:], start=True, stop=True)
                        x2 = dput.tile([128, 128], BF16, tag="ut", name="x2")
                        nc.scalar.copy(x2[:], p_sq[:])
                    p_sqt = pst(128, 128)
                    nc.tensor.matmul(p_sqt[:], xx[:], xt[:], start=True, stop=True)
                    xt2 = dput.tile([128, 128], BF16, tag="ut", name="xt2")
                    if h % 2 == 0:
                        nc.scalar.copy(xt2[:], p_sqt[:])
                    else:
                        nc.vector.tensor_copy(xt2[:], p_sqt[:])
                    nxt[h] = (x2, xt2)
                for h in range(HP):
                    p_pr = pst(128, 128)
                    nc.tensor.matmul(p_pr[:], nxt[h][1][:], PM[h][:], start=True, stop=True)
                    pnew = dput.tile([128, 128], BF16, tag="ut", name="pnew")
                    if h % 2 == 0:
                        nc.vector.tensor_add(pnew[:], PM[h][:], p_pr[:])
                    else:
                        nc.vector.tensor_add(pnew[:], p_pr[:], PM[h][:])
                    PM[h] = pnew
                if not last:
                    cur = nxt

            # ---- state/output, step-major in 4-head waves ----
            ot_all = dp2.tile([128, VD_C], F32, tag="otall", name="ot_all")
            for jp in (0, 2):
                heads = [(j, hh) for j in (jp, jp + 1) for hh in (0, 1)]
                SB, PS, KW = {}, {}, {}
                for j in (jp, jp + 1):
                    S_bf = dp2.tile([128, DV], BF16, tag=f"sbf{j}", name="S_bf")
                    nc.gpsimd.tensor_copy(S_bf[:], S_cur[j][:])
                    SB[j] = S_bf
                    PS[j] = pst(128, DV)
                for j, hh in heads:
                    h = 2 * j + hh
                    rh = 64 * hh
                    kw = dp2.tile([128, 48], BF16, tag="kw", name="kw", bufs=6)
                    nc.gpsimd.tensor_scalar_mul(
                        kw[:], ktok[:, 512 * ci + 128 * j + rh:512 * ci + 128 * j + rh + 48],
                        w_tok[:, 8 * ci + h:8 * ci + h + 1])
                    KW[h] = kw
                RR, UU = {}, {}
                for j, hh in heads:
                    h = 2 * j + hh
                    rh = 64 * hh
                    kts = ksb[rh:rh + 48, 1024 * j + 128 * ci:1024 * j + 128 * ci + 128]
                    p_ks = pst(128, DV)
                    nc.tensor.matmul(p_ks[:], kts, SB[j][rh:rh + 48, :], start=True, stop=True)
                    r_ = dp2.tile([128, DV], BF16, tag="rr", name="r_", bufs=5)
                    nc.vector.scalar_tensor_tensor(
                        r_[:], p_ks[:], nlam_all[:, 8 * ci + h:8 * ci + h + 1],
                        vtok[:, VD_C * ci + 96 * h:VD_C * ci + 96 * h + 96],
                        OP.mult, OP.add)
                    RR[h] = r_
                for j, hh in heads:
                    h = 2 * j + hh
                    p_w = pst(128, DV)
                    nc.tensor.matmul(p_w[:], PM[h][:], RR[h][:], start=True, stop=True)
                    u_ = dp2.tile([128, DV], BF16, tag="uu", name="u_", bufs=5)
                    nc.vector.tensor_scalar_mul(u_[:], p_w[:],
                                                beta_all[:, 8 * ci + h:8 * ci + h + 1])
                    UU[h] = u_
                for j, hh in heads:
                    h = 2 * j + hh
                    rh = 64 * hh
                    qts = qsb[rh:rh + 48, 1024 * j + 128 * ci:1024 * j + 128 * ci + 128]
                    p_oi = pst(128, DV)
                    nc.tensor.matmul(p_oi[:], ABAR[h][:], UU[h][:], start=True, stop=True)
                    p_qs = pst(128, DV)
                    nc.tensor.matmul(p_qs[:], qts, SB[j][rh:rh + 48, :], start=True, stop=True)
                    ots = ot_all[:, 96 * h:96 * h + 96]
                    nc.vector.tensor_scalar_mul(ots, p_qs[:],
                                                lam_all[:, 8 * ci + h:8 * ci + h + 1])
                    nc.vector.tensor_add(ots, ots, p_oi[:])
                    nc.tensor.matmul(PS[j][rh:rh + 48, :], KW[h][:], UU[h][:],
                                     start=True, stop=True)
                for j in (jp, jp + 1):
                    s_new = spool.tile([128, DV], F32, tag=f"s{j}", name="s_new")
                    for rh2 in (0, 64):
                        nc.vector.scalar_tensor_tensor(
                            s_new[rh2:rh2 + 48, :], S_cur[j][rh2:rh2 + 48, :],
                            ebcJ[j][rh2:rh2 + 48, ci:ci + 1],
                            PS[j][rh2:rh2 + 48, :], OP.mult, OP.add)
                    S_cur[j] = s_new

            # ---- batched gated rmsnorm + gate + transpose to osb ----
            osq = dp2.tile([128, VD_C], F32, tag="osq", name="osq")
            nc.vector.tensor_mul(osq[:], ot_all[:], ot_all[:])
            rcol8 = dp2.tile([128, 8], F32, tag="rc8", name="rcol8")
            for h in range(HP):
                nc.vector.tensor_reduce(rcol8[:, h:h + 1], osq[:, 96 * h:96 * h + 96],
                                        mybir.AxisListType.X, OP.add)
            nc.scalar.activation(rcol8[:], rcol8[:], AF.Sqrt, bias=epsg[:], scale=1.0 / DV)
            nc.vector.reciprocal_approx_fast(rcol8[:], rcol8[:])
            ob = dp2.tile([128, VD_C], BF16, tag="ob", name="ob")
            for h in range(HP):
                nc.vector.scalar_tensor_tensor(
                    ob[:, 96 * h:96 * h + 96], ot_all[:, 96 * h:96 * h + 96],
                    rcol8[:, h:h + 1], g_tok[:, VD_C * ci + 96 * h:VD_C * ci + 96 * h + 96],
                    OP.mult, OP.mult)
            for b6 in range(6):
                p_ot = pst(128, 128, BF16)
                nc.tensor.transpose(p_ot[:], ob[:, 128 * b6:128 * b6 + 128], idnb[:])
                nc.scalar.copy(osb[:, 1024 * b6 + 128 * ci:1024 * b6 + 128 * ci + 128], p_ot[:])

            # ---- o_projT parts + AllReduce as soon as their tokens exist ----
            if ci == 3:
                emit_D(0, 0, 512)
            if ci == 5:
                emit_D(1, 512, 256)
            if ci == 7:
                emit_D(2, 768, 256)

        for p in (spool, dp2, dpf, dpx, dpa, dput, pd, wod):
            _rel(p)
        if DUMP == "o":
            dump_y([(b, osb[:, 1024 * b:1024 * b + 1024]) for b in range(6)]
                   + [(6 + b, g_tok[:, 1024 * b:1024 * b + 1024]) for b in range(6)]
                   + [(12 + b, ktok[:, 1024 * b:1024 * b + 1024]) for b in range(4)])
        if DUMP == "dk":
            dump_y([(0, gta[:]), (1, beta_all[:]), (2, bcum_tok[:]),
                    (3, lam_all[:]), (4, w_tok[:]), (5, b_fm[:]),
                    (6, ebc_all[:]), (7, ebcJ[0][:]), (8, ebcJ[3][:])])
        _rel(dk)
        _rel(bigq)

        if "D" not in PHASES:
            raise _SkipRest()

        # ============ Phase E ============
        if "E" not in PHASES:
            raise _SkipRest()
        seq = _pool(name="seq", bufs=3)
        pe = _pool(name="pe", bufs=4)
        wp13 = _pool(name="wp13", bufs=3)
        wp2 = _pool(name="wp2", bufs=3)
        mida = pg.tile([128, 6 * 1024], BF16, tag="gtok")
        pmid = _pool(name="pmid", bufs=1)
        midb = pmid.tile([128, 5 * 1024], BF16, tag="midb")

        def mid_ap(m, off, ln):
            if m < 6:
                return mida[:, 1024 * m + off:1024 * m + off + ln]
            return midb[:, 1024 * (m - 6) + off:1024 * (m - 6) + off + ln]

        ffT = hT
        for half in range(2):
            hs = slice(512 * half, 512 * half + 512)
            p_s2 = pst(1, 512)
            for k in range(KT):
                xe = seq.tile([128, 512], F32, tag="xe")
                nc.sync.dma_start(xe[:], xT_d[128 * k:128 * k + 128, hs])
                oe = seq.tile([128, 512], BF16, tag="oe")
                if half == 0:
                    nc.sync.dma_start(oe[:], oT_out[0][128 * k:128 * k + 128, :])
                else:
                    nc.sync.dma_start(oe[:, 0:256], oT_out[1][128 * k:128 * k + 128, :])
                    nc.sync.dma_start(oe[:, 256:512], oT_out[2][128 * k:128 * k + 128, :])
                h2 = seq.tile([128, 512], F32, tag="h2T")
                nc.vector.tensor_add(h2[:], xe[:], oe[:])
                nc.scalar.dma_start(h2T_scr[128 * k:128 * k + 128, hs], h2[:])
                sqe = seq.tile([128, 512], BF16, tag="sqe")
                nc.vector.tensor_mul(sqe[:], h2[:], h2[:])
                nc.tensor.matmul(p_s2[:], ones1[:], sqe[:],
                                 start=(k == 0), stop=(k == KT - 1))
            srowE = seq.tile([1, 512], F32, tag="srowE", bufs=2)
            nc.scalar.activation(srowE[:], p_s2[:], AF.Sqrt,
                                 bias=eps1[0:1, :], scale=1.0 / D)
            nc.vector.reciprocal_approx_fast(srowE[:], srowE[:])
            sbcE = seq.tile([128, 512], F32, tag="sbcE", bufs=2)
            nc.gpsimd.partition_broadcast(sbcE[:], srowE[:])
            for k in range(KT):
                h2r = seq.tile([128, 512], F32, tag="h2r", bufs=4)
                nc.sync.dma_start(h2r[:], h2T_scr[128 * k:128 * k + 128, hs])
                nc.vector.tensor_mul(ffT[:, 1024 * k + 512 * half:1024 * k + 512 * half + 512],
                                     h2r[:], sbcE[:])
            # W1/W3 for this half
            for m in range(11):
                pu1 = pst()
                pu3 = pst()
                wt13 = wp13.tile([128, 4096], BF16, tag="w13", name="wt13")
                nc.sync.dma_start(wt13[:], w13_d[:, 4096 * m:4096 * m + 4096])
                for k in range(KT):
                    rhs = ffT[:, 1024 * k + 512 * half:1024 * k + 512 * half + 512]
                    nc.tensor.matmul(pu1[:], wt13[:, 256 * k:256 * k + 128], rhs,
                                     start=(k == 0), stop=(k == KT - 1))
                    nc.tensor.matmul(pu3[:], wt13[:, 256 * k + 128:256 * k + 256], rhs,
                                     start=(k == 0), stop=(k == KT - 1))
                u1s = pe.tile([128, 512], F32, tag="s512", name="u1s")
                nc.scalar.activation(u1s[:], pu1[:], AF.Silu)
                nc.vector.tensor_mul(mid_ap(m, 512 * half, 512), u1s[:], pu3[:])
            if DUMP == "mid" and half == 1:
                dump_y([(m, mid_ap(m, 0, 1024)) for m in range(11)])
            # W2 for this half
            for db in range(16):
                wt2 = wp2.tile([128, INT_C], BF16, tag="w2", name="w2_t")
                nc.sync.dma_start(wt2[:], w2_d[:, INT_C * db:INT_C * db + INT_C])
                pps = pst()
                for m in range(11):
                    nc.tensor.matmul(pps[:], wt2[:, 128 * m:128 * m + 128],
                                     mid_ap(m, 512 * half, 512),
                                     start=(m == 0), stop=(m == 10))
                h2t = pe.tile([128, 512], F32, tag="s512", name="h2t")
                nc.sync.dma_start(h2t[:], h2T_scr[128 * db:128 * db + 128, hs])
                yst = pe.tile([128, 512], F32, tag="s512", name="yst")
                nc.vector.tensor_scalar_mul(yst[:], h2t[:], 0.25)
                nc.vector.tensor_add(yst[:], yst[:], pps[:])
                nc.sync.dma_start(y_d[128 * db:128 * db + 128, hs], yst[:])

        for p in (pmid, wp2, wp13, pe, seq, dram, wp, pg, big, ps, cpool):
            _rel(p)
      except _SkipRest:
        zst = _pool(name="zst", bufs=1)
        zt = zst.tile([128, 512], F32)
        nc.vector.memset(zt[:], 0.0)
        for i in range(16):
            for dh in range(2):
                nc.sync.dma_start(y_d[128 * i:128 * i + 128, 512 * dh:512 * dh + 512], zt[:])
        for p in reversed(live_pools):
            p.release()
      except _Dumped:
        for p in reversed(live_pools):
            p.release()

    nc.compile()
    return nc


def _pack_wo(wo):
    # [768, 2048] -> [128, 16*768]: col = 768*db + 128*fb + c
    return np.ascontiguousarray(
        wo.reshape(6, 128, 16, 128).transpose(1, 2, 0, 3).reshape(128, 16 * 768))


def _pack_w13(w1, w3):
    # [2048, 1408] x2 -> [128, 11*4096]: col = 4096*m + 256*k + 128*which + c
    a = w1.reshape(16, 128, 11, 128).transpose(1, 2, 0, 3)   # [128, 11, 16, 128]
    b = w3.reshape(16, 128, 11, 128).transpose(1, 2, 0, 3)
    return np.ascontiguousarray(
        np.stack([a, b], axis=3).reshape(128, 11 * 4096))


def _pack_w2(w2):
    # [1408, 2048] -> [128, 16*1408]: col = 1408*db + 128*m + c
    return np.ascontiguousarray(
        w2.reshape(11, 128, 16, 128).transpose(1, 2, 0, 3).reshape(128, 16 * 1408))


def _shard(inputs):
    f32 = np.float32
    bf = ml_dtypes.bfloat16
    rms1 = np.asarray(inputs["rms1_w"], f32)
    rms2 = np.asarray(inputs["rms2_w"], f32)
    gn = np.asarray(inputs["gnorm_w"], f32)
    in_maps = []
    for c in range(8):
        g, m = c // 4, c % 4
        qs = slice(384 * m, 384 * m + 384)
        vs = slice(768 * m, 768 * m + 768)
        hs = slice(8 * m, 8 * m + 8)
        isl = slice(1408 * m, 1408 * m + 1408)

        def padqk(w):
            wp_ = np.zeros((D, QKP), f32)
            for h in range(8):
                wp_[:, 64 * h:64 * h + 48] = w[:, 48 * h:48 * h + 48]
            return wp_

        def padcw(w):
            cp = np.zeros((QKP, 4), f32)
            for h in range(8):
                cp[64 * h:64 * h + 48] = w[48 * h:48 * h + 48]
            return cp

        def padv(w):
            colpad = w.shape[0] == D
            out = np.zeros((D, VP) if colpad else (VP, w.shape[1]), f32)
            for h in range(8):
                if colpad:
                    out[:, 128 * h:128 * h + 96] = w[:, 96 * h:96 * h + 96]
                else:
                    out[128 * h:128 * h + 96] = w[96 * h:96 * h + 96]
            return out

        dtb8 = np.asarray(inputs["dt_bias"], f32)[hs]
        nega8 = -np.exp(np.asarray(inputs["A_log"], f32)[hs])
        in_maps.append(dict(
            xT=np.ascontiguousarray(np.asarray(inputs["x"], f32)[g].T),
            wq=padqk(np.asarray(inputs["Wq"], f32)[:, qs] * rms1[:, None]).astype(bf),
            wk=padqk(np.asarray(inputs["Wk"], f32)[:, qs] * rms1[:, None]).astype(bf),
            wv=padv(np.asarray(inputs["Wv"], f32)[:, vs] * rms1[:, None]).astype(bf),
            wg=np.ascontiguousarray(
                np.asarray(inputs["Wg"], f32)[:, vs] * rms1[:, None]).astype(bf),
            wab=np.ascontiguousarray(np.concatenate(
                [np.asarray(inputs["Wa"], f32)[:, hs],
                 np.asarray(inputs["Wb"], f32)[:, hs]], 1) * rms1[:, None]).astype(bf),
            cq=padcw(np.asarray(inputs["conv_q_w"], f32)[qs]),
            ck=padcw(np.asarray(inputs["conv_k_w"], f32)[qs]),
            cv=padv(np.asarray(inputs["conv_v_w"], f32)[vs]),
            dtb=np.tile(dtb8, 8).reshape(1, 64).copy(),
            nega=np.tile(nega8, 8).reshape(1, 64).copy(),
            wo=_pack_wo(np.asarray(inputs["Wo"], f32)[vs] * np.tile(gn, 8)[:, None]).astype(bf),
            w13=_pack_w13(np.asarray(inputs["W1"], f32)[:, isl] * rms2[:, None],
                          np.asarray(inputs["W3"], f32)[:, isl] * rms2[:, None]).astype(bf),
            w2=_pack_w2(np.asarray(inputs["W2"], f32)[isl]).astype(bf),
        ))
    return in_maps


def kernel(**inputs):
    if "nc" not in _cache:
        _cache["nc"] = _build(8)
    res = run_bass_kernel_spmd(_cache["nc"], _shard(inputs), list(range(8)))
    out = np.zeros((B, T, D), np.float32)
    for g in range(2):
        yT = sum(res.results[4 * g + m]["y"] for m in range(4))
        out[g] = yT.T
    return out
